# revision 32
# baseline (speedup 1.0000x reference)
import os
import sys

for _p in ("/opt/trn_rl_repo", "/root/.axon_site/_ro/trn_rl_repo"):
    if _p not in sys.path:
        sys.path.insert(0, _p)

import numpy as np
import ml_dtypes

import concourse.bass as bass
import concourse.bacc as bacc
import concourse.mybir as mybir
import concourse.tile as tile
from concourse.masks import make_identity

F32 = mybir.dt.float32
BF16 = mybir.dt.bfloat16
I32 = mybir.dt.int32
I16 = mybir.dt.int16
ALU = mybir.AluOpType
ACT = mybir.ActivationFunctionType
AX = mybir.AxisListType

P = 128
EPS = 1e-5
SLOPE = 0.2
HEADS = 4


class Cfg:
    def __init__(self, ncores=8, nreal=40000, npc=5120, in_dim=64, hid=256,
                 eb=11, gw=224, ks=64, g=1500):
        self.NCORES = ncores
        self.N = nreal                      # real nodes total
        self.NR = nreal // ncores           # real nodes per core
        self.NP = npc                       # padded node slots per core
        self.NT = npc * ncores              # total table slots
        self.IN = in_dim
        self.HID = hid
        self.C = hid // HEADS
        self.EB = eb                        # edge blocks per dst-tile
        self.ET = eb * P                    # edge slot capacity per dst-tile
        self.NPT = npc // P                 # dst-tiles per core
        self.ECB = self.NPT * eb            # edge array free blocks
        self.GW = gw                        # graph window per core
        self.KS = ks                        # slots per graph
        self.RN = gw * ks                   # readout gather count
        self.G = g
        assert npc % P == 0 and self.RN % 512 == 0
        self.PAD_SLOT = self.NR             # global slot with zeroed table row


FULL = Cfg()


# ---------------------------------------------------------------------------
# device program
# ---------------------------------------------------------------------------

def build_nc(cfg: Cfg, dbg=False):
    c = cfg
    HID = c.HID
    nc = bacc.Bacc("TRN2", target_bir_lowering=False, debug=False,
                   num_devices=c.NCORES, num_swdge_queues=1)

    x_in = nc.dram_tensor("x_in", [c.NP, c.IN], BF16, kind="ExternalInput")
    wcat0 = nc.dram_tensor("wcat0", [c.IN, 2 * HID], BF16, kind="ExternalInput")
    w1 = nc.dram_tensor("w1", [HID, HID], BF16, kind="ExternalInput")
    w2 = nc.dram_tensor("w2", [HID, HID], BF16, kind="ExternalInput")
    pvec = nc.dram_tensor("pvec", [21, P], F32, kind="ExternalInput")
    asd = nc.dram_tensor("asd", [6, HID], F32, kind="ExternalInput")
    esrc = nc.dram_tensor("esrc", [P, c.ECB], mybir.dt.uint16, kind="ExternalInput")
    edst32 = nc.dram_tensor("edst32", [P, c.ECB], I16, kind="ExternalInput")
    dstf = nc.dram_tensor("dstf", [P, c.ECB], mybir.dt.int8, kind="ExternalInput")
    ridxs = nc.dram_tensor("ridxs", [16, c.RN // 16], I16, kind="ExternalInput")
    ridxm = nc.dram_tensor("ridxm", [16, c.RN // 16], I16, kind="ExternalInput")

    rsum_o = nc.dram_tensor("rsum_o", [2, P, c.GW], BF16, kind="ExternalOutput")
    rmax_o = nc.dram_tensor("rmax_o", [2, P, c.GW], BF16, kind="ExternalOutput")
    if dbg:
        d_h = [nc.dram_tensor(f"d_h{l}", [c.NP, c.HID], F32,
                              kind="ExternalOutput") for l in range(3)]
        d_o = nc.dram_tensor("d_o", [c.NP, c.HID], F32, kind="ExternalOutput")
        d_g1 = nc.dram_tensor("d_g1", [P, c.EB, c.HID + 16], F32, kind="ExternalOutput")
        d_g2 = nc.dram_tensor("d_g2", [P, c.EB, 8], F32, kind="ExternalOutput")
        d_ex = nc.dram_tensor("d_ex", [P, c.EB, 4], F32, kind="ExternalOutput")
        d_pay = nc.dram_tensor("d_pay", [P, c.EB, 260], F32, kind="ExternalOutput")
        d_pt = nc.dram_tensor("d_pt", [P, 260], F32, kind="ExternalOutput")

    RG = [list(range(c.NCORES))]

    with tile.TileContext(nc) as tc:
        with (
            tc.tile_pool(name="dram", bufs=1, space="DRAM") as dram,
            tc.tile_pool(name="const", bufs=1) as cpool,
            tc.tile_pool(name="persist", bufs=1) as pers,
        ):
            # ---------------- constants / loads ----------------
            ident = cpool.tile([P, P], BF16)
            make_identity(nc, ident[:])
            ones = cpool.tile([P, 1], F32)
            nc.gpsimd.memset(ones[:], 1.0)
            m39 = cpool.tile([P, 1], F32)
            nc.gpsimd.memset(m39[:], 0.0)
            npad = c.NR - (c.NPT - 1) * P   # real rows in the last tile
            nc.gpsimd.memset(m39[0:npad, :], 1.0)

            wc0_sb = cpool.tile([c.IN, 2 * HID], BF16)
            nc.sync.dma_start(wc0_sb[:], wcat0[:, :])
            w1_sb = cpool.tile([P, 2, HID], BF16)
            w2_sb = cpool.tile([P, 2, HID], BF16)
            for hf in range(2):
                nc.sync.dma_start(w1_sb[:, hf, :], w1[hf * P:(hf + 1) * P, :])
                nc.sync.dma_start(w2_sb[:, hf, :], w2[hf * P:(hf + 1) * P, :])

            def bcast256(pool, dst, src0, src1):
                scr = pool.tile([1, 2 * P], F32, tag="bscr", bufs=2)
                nc.sync.dma_start(scr[0:1, 0:P], src0)
                nc.sync.dma_start(scr[0:1, P:2 * P], src1)
                nc.gpsimd.partition_broadcast(dst[:, :], scr[0:1, :])

            as_bc, ad_bc, gb_bc = [], [], []
            for l in range(3):
                a_s = pers.tile([P, HID], F32, name=f"as_bc{l}")
                a_d = pers.tile([P, HID], F32, name=f"ad_bc{l}")
                bcast256(pers, a_s, asd[2 * l:2 * l + 1, 0:P],
                         asd[2 * l:2 * l + 1, P:2 * P])
                bcast256(pers, a_d, asd[2 * l + 1:2 * l + 2, 0:P],
                         asd[2 * l + 1:2 * l + 2, P:2 * P])
                as_bc.append(a_s)
                ad_bc.append(a_d)
                g_b = pers.tile([P, HID], F32, name=f"gb_bc{l}")
                bcast256(pers, g_b, pvec[2 + 6 * l:3 + 6 * l, :],
                         pvec[3 + 6 * l:4 + 6 * l, :])
                gb_bc.append(g_b)
            bin_bc = pers.tile([P, HID], F32)
            bcast256(pers, bin_bc, pvec[0:1, :], pvec[1:2, :])
            iota_bc = pers.tile([P, P], F32)
            scr0 = pers.tile([1, P], F32, name="scr0")
            nc.sync.dma_start(scr0[0:1, :], pvec[20:21, :])
            nc.gpsimd.partition_broadcast(iota_bc[:, :], scr0[0:1, :])

            esrc_16 = pers.tile([P, c.ECB], mybir.dt.uint16, name="esrc16")
            nc.sync.dma_start(esrc_16[:], esrc[:, :])
            esrc_sb = pers.tile([P, c.ECB], I32)
            nc.vector.tensor_copy(esrc_sb[:], esrc_16[:])
            edst_16 = pers.tile([P, c.ECB], I16, name="edst16t")
            nc.sync.dma_start(edst_16[:], edst32[:, :])
            edst32_sb = pers.tile([P, c.ECB], I32)
            nc.vector.tensor_copy(edst32_sb[:], edst_16[:])
            dstf_8 = pers.tile([P, c.ECB], mybir.dt.int8, name="dstf8")
            nc.sync.dma_start(dstf_8[:], dstf[:, :])
            dstf_sb = pers.tile([P, c.ECB], F32)
            nc.vector.tensor_copy(dstf_sb[:], dstf_8[:])
            ridxs_sb = pers.tile([P, c.RN // 16], I16)
            ridxm_sb = pers.tile([P, c.RN // 16], I16)
            for k in range(8):
                nc.sync.dma_start(ridxs_sb[16 * k:16 * (k + 1), :], ridxs[:, :])
                nc.sync.dma_start(ridxm_sb[16 * k:16 * (k + 1), :], ridxm[:, :])

            x_bf = pers.tile([P, c.NPT, c.IN], BF16)
            for t in range(c.NPT):
                nc.sync.dma_start(x_bf[:, t, :], x_in[t * P:(t + 1) * P, :])

            h_bf = pers.tile([P, c.NPT, HID], BF16)
            o_sb = pers.tile([P, c.NPT, HID], F32)

            hshort = dram.tile([c.NP, HID], F32)
            h_cur = [dram.tile([c.NP, HID], F32, name=f"h{l}") for l in range(3)]

            # ---------------- layers ----------------
            for l in range(3):
                tbl_own = dram.tile([c.NP, HID + 16], BF16, name=f"tblo{l}")
                tblS_own = dram.tile([c.NP, 8], F32, name=f"tblso{l}")
                tbl = dram.tile([c.NT, HID + 16], BF16, name=f"tbl{l}",
                                addr_space="Shared")
                stats_own = dram.tile([4, P], F32, name=f"sto{l}")
                stats_all = dram.tile([4, P], F32, name=f"sta{l}",
                                      addr_space="Shared")

                with (
                    tc.tile_pool(name=f"tl{l}", bufs=2) as tl,
                    tc.tile_pool(name=f"tps{l}", bufs=2, space="PSUM") as tps,
                    tc.tile_pool(name=f"sps{l}", bufs=1, space="PSUM") as sps,
                    tc.tile_pool(name=f"ep{l}", bufs=2) as ep,
                    tc.tile_pool(name=f"sm{l}", bufs=1) as sm,
                ):
                    # ---- table build (own slice) ----
                    for t in range(c.NPT):
                        if l == 0:
                            xt = x_bf[:, t, :]
                            tp = tps.tile([P, P], BF16, tag="tp")
                            hT = tl.tile([P, 2, P], BF16, tag="hT")
                            nc.tensor.transpose(tp[0:c.IN, :], xt, ident[:])
                            nc.vector.tensor_copy(hT[0:c.IN, 0, :], tp[0:c.IN, :])
                            xw_ps = tps.tile([P, 2 * HID], F32, tag="xwps")
                            nc.tensor.matmul(xw_ps[:, :], hT[0:c.IN, 0, :],
                                             wc0_sb[:, :], start=True, stop=True)
                            hs = tl.tile([P, HID], F32, tag="hs")
                            nc.vector.tensor_add(hs[:], xw_ps[:, 0:HID], bin_bc[:])
                            nc.sync.dma_start(hshort[t * P:(t + 1) * P, :], hs[:])
                            xw = xw_ps[:, HID:2 * HID]
                        else:
                            ht = h_bf[:, t, :]
                            wsb = w1_sb if l == 1 else w2_sb
                            xw_ps = tps.tile([P, 2 * HID], F32, tag="xwps")
                            hT = tl.tile([P, 2, P], BF16, tag="hT")
                            for hf in range(2):
                                tp = tps.tile([P, P], BF16, tag="tp")
                                nc.tensor.transpose(
                                    tp[:, :], ht[:, hf * P:(hf + 1) * P], ident[:])
                                nc.vector.tensor_copy(hT[:, hf, :], tp[:, :])
                            for hf in range(2):
                                nc.tensor.matmul(xw_ps[:, 0:HID], hT[:, hf, :],
                                                 wsb[:, hf, :], start=(hf == 0),
                                                 stop=(hf == 1))
                            xw = xw_ps[:, 0:HID]

                        xwb = tl.tile([P, HID], BF16, tag="xwb")
                        if t == c.NPT - 1:
                            nc.vector.tensor_scalar_mul(xwb[:], xw, m39[:, 0:1])
                        else:
                            nc.vector.tensor_copy(xwb[:], xw)
                        nc.sync.dma_start(tbl_own[t * P:(t + 1) * P, 0:HID],
                                          xwb[:])

                        sd = tl.tile([P, 8], F32, tag="sd")
                        sc = tl.tile([P, c.C], F32, tag="sc")
                        for hd in range(4):
                            sl = slice(hd * c.C, (hd + 1) * c.C)
                            nc.vector.tensor_tensor(
                                out=sc[:], in0=xw[:, sl], in1=as_bc[l][:, sl],
                                op=ALU.mult)
                            nc.vector.tensor_reduce(
                                sd[:, hd:hd + 1], sc[:], axis=AX.X, op=ALU.add)
                            nc.vector.tensor_tensor(
                                out=sc[:], in0=xw[:, sl], in1=ad_bc[l][:, sl],
                                op=ALU.mult)
                            nc.vector.tensor_reduce(
                                sd[:, 4 + hd:5 + hd], sc[:], axis=AX.X,
                                op=ALU.add)
                        nc.sync.dma_start(tblS_own[t * P:(t + 1) * P, :], sd[:])
                        nc.sync.dma_start(
                            tbl_own[t * P:(t + 1) * P, HID:HID + 16],
                            sd[:].bitcast(BF16))

                    nc.gpsimd.collective_compute(
                        "AllGather", ALU.bypass, replica_groups=RG,
                        ins=[tbl_own[:, :].opt()], outs=[tbl[:, :].opt()])

                    # ---- edge phase: gather / attention / segment matmul ----
                    s_o = sm.tile([P, HID], F32, name=f"s_o{l}")
                    s_q = sm.tile([P, HID], F32, name=f"s_q{l}")
                    nc.vector.memset(s_o[:], 0.0)
                    nc.vector.memset(s_q[:], 0.0)
                    for t in range(c.NPT):
                        g1 = ep.tile([P, c.EB, HID + 16], BF16, tag="g1")
                        g2 = ep.tile([P, c.EB, 8], F32, tag="g2")
                        for b in range(c.EB):
                            col = t * c.EB + b
                            nc.gpsimd.indirect_dma_start(
                                out=g1[:, b, :], out_offset=None, in_=tbl[:, :],
                                in_offset=bass.IndirectOffsetOnAxis(
                                    ap=esrc_sb[:, col:col + 1], axis=0))
                            nc.gpsimd.indirect_dma_start(
                                out=g2[:, b, :], out_offset=None,
                                in_=tblS_own[:, :],
                                in_offset=bass.IndirectOffsetOnAxis(
                                    ap=edst32_sb[:, col:col + 1], axis=0))

                        g1sd = g1[:, :, HID:HID + 16].bitcast(F32)
                        ee = ep.tile([P, c.EB, 4], F32, tag="ee")
                        nc.vector.tensor_add(ee[:], g1sd[:, :, 0:4],
                                             g2[:, :, 4:8])
                        e2 = ep.tile([P, c.EB, 4], F32, tag="e2")
                        nc.scalar.mul(e2[:], ee[:], SLOPE)
                        nc.vector.tensor_tensor(out=ee[:], in0=ee[:], in1=e2[:],
                                                op=ALU.max)
                        ex = ep.tile([P, c.EB, 4], F32, tag="ex")
                        nc.scalar.activation(ex[:], ee[:], ACT.Exp)

                        pay = ep.tile([P, c.EB, 260], BF16, tag="pay")
                        for hd in range(4):
                            sl = slice(hd * c.C, (hd + 1) * c.C)
                            nc.vector.tensor_tensor(
                                out=pay[:, :, sl], in0=g1[:, :, sl],
                                in1=ex[:, :, hd:hd + 1].to_broadcast(
                                    [P, c.EB, c.C]),
                                op=ALU.mult)
                        nc.vector.tensor_copy(pay[:, :, 256:260], ex[:])

                        pt = tps.tile([P, 260], F32, tag="pt")
                        for b in range(c.EB):
                            S = ep.tile([P, P], BF16, tag="S")
                            nc.vector.tensor_scalar(
                                out=S[:], in0=iota_bc[:],
                                scalar1=dstf_sb[:, t * c.EB + b:
                                                t * c.EB + b + 1],
                                scalar2=None, op0=ALU.is_equal)
                            nc.tensor.matmul(pt[:, :], S[:], pay[:, b, :],
                                             start=(b == 0),
                                             stop=(b == c.EB - 1))

                        if dbg and l == 0 and t == 0:
                            dcp = tl.tile([P, c.EB, c.HID + 16], F32, tag="dcp")
                            nc.vector.tensor_copy(dcp[:], g1[:])
                            nc.sync.dma_start(d_g1[:, :, :], dcp[:])
                            nc.sync.dma_start(d_g2[:, :, :], g2[:])
                            nc.sync.dma_start(d_ex[:, :, :], ex[:])
                            dcq = tl.tile([P, c.EB, 260], F32, tag="dcq")
                            nc.vector.tensor_copy(dcq[:], pay[:])
                            nc.sync.dma_start(d_pay[:, :, :], dcq[:])
                            dcr = tl.tile([P, 260], F32, tag="dcr")
                            nc.vector.tensor_copy(dcr[:], pt[:])
                            nc.sync.dma_start(d_pt[:, :], dcr[:])

                        # ---- finalize tile: alpha-div, bias, stats ----
                        den = tl.tile([P, 4], F32, tag="den")
                        nc.vector.tensor_scalar_max(den[:], pt[:, 256:260],
                                                    1e-20)
                        rec = tl.tile([P, 4], F32, tag="rec")
                        nc.vector.reciprocal(rec[:], den[:])
                        ot = o_sb[:, t, :]
                        for hd in range(4):
                            sl = slice(hd * c.C, (hd + 1) * c.C)
                            nc.vector.tensor_scalar_mul(ot[:, sl], pt[:, sl],
                                                        rec[:, hd:hd + 1])
                        nc.vector.tensor_add(ot, ot, gb_bc[l][:])
                        if t == c.NPT - 1:
                            om = tl.tile([P, HID], F32, tag="om")
                            nc.vector.tensor_scalar_mul(om[:], ot, m39[:, 0:1])
                            stat_in = om[:]
                        else:
                            stat_in = ot
                        sq = tl.tile([P, HID], F32, tag="sq")
                        nc.vector.tensor_tensor(out=sq[:], in0=stat_in,
                                                in1=stat_in, op=ALU.mult)
                        nc.vector.tensor_add(s_o[:], s_o[:], stat_in)
                        nc.vector.tensor_add(s_q[:], s_q[:], sq[:])

                    if dbg and l == 0:
                        for t in range(c.NPT):
                            nc.sync.dma_start(d_o[t * P:(t + 1) * P, :],
                                              o_sb[:, t, :])

                    # ---- BN stats reduce + collective ----
                    st_ps = sps.tile([P, 4], F32, name=f"stp{l}")
                    for hf in range(2):
                        nc.tensor.matmul(
                            st_ps[:, hf:hf + 1], s_o[:, hf * P:(hf + 1) * P],
                            ones[:], start=True, stop=True,
                            skip_group_check=True)
                        nc.tensor.matmul(
                            st_ps[:, 2 + hf:3 + hf], s_q[:, hf * P:(hf + 1) * P],
                            ones[:], start=True, stop=True,
                            skip_group_check=True)
                    stq = sm.tile([P, 4], F32, name=f"stq{l}")
                    nc.vector.tensor_copy(stq[:], st_ps[:])
                    nc.sync.dma_start(stats_own[:, :].transpose([1, 0]), stq[:])
                    nc.gpsimd.collective_compute(
                        "AllReduce", ALU.add, replica_groups=RG,
                        ins=[stats_own[:, :].opt()], outs=[stats_all[:, :].opt()])

                    st_s = sm.tile([2, P], F32, name=f"sts{l}")
                    nc.sync.dma_start(st_s[:], stats_all[0:2, :])
                    st_q = sm.tile([2, P], F32, name=f"stq2{l}")
                    nc.sync.dma_start(st_q[:], stats_all[2:4, :])
                    g2t = sm.tile([2, P], F32, name=f"g2t{l}")
                    nc.sync.dma_start(g2t[:], pvec[4 + 6 * l:6 + 6 * l, :])
                    be2t = sm.tile([2, P], F32, name=f"be2t{l}")
                    nc.sync.dma_start(be2t[:], pvec[6 + 6 * l:8 + 6 * l, :])
                    mu = sm.tile([2, P], F32, name=f"mu{l}")
                    nc.scalar.mul(mu[:], st_s[:], 1.0 / c.N)
                    va = sm.tile([2, P], F32, name=f"va{l}")
                    nc.scalar.mul(va[:], st_q[:], 1.0 / c.N)
                    mu2 = sm.tile([2, P], F32, name=f"mu2{l}")
                    nc.vector.tensor_tensor(out=mu2[:], in0=mu[:], in1=mu[:],
                                            op=ALU.mult)
                    nc.vector.tensor_tensor(out=va[:], in0=va[:], in1=mu2[:],
                                            op=ALU.subtract)
                    nc.vector.tensor_scalar_add(va[:], va[:], EPS)
                    sdv = sm.tile([2, P], F32, name=f"sdv{l}")
                    nc.scalar.sqrt(sdv[:], va[:])
                    rs = sm.tile([2, P], F32, name=f"rs{l}")
                    nc.vector.reciprocal(rs[:], sdv[:])
                    A2 = sm.tile([2, P], F32, name=f"A2{l}")
                    nc.vector.tensor_tensor(out=A2[:], in0=rs[:], in1=g2t[:],
                                            op=ALU.mult)
                    B2 = sm.tile([2, P], F32, name=f"B2{l}")
                    nc.vector.tensor_tensor(out=B2[:], in0=mu[:], in1=A2[:],
                                            op=ALU.mult)
                    nc.vector.tensor_tensor(out=B2[:], in0=be2t[:],
                                            in1=B2[:], op=ALU.subtract)
                    A_bc = sm.tile([P, HID], F32, name=f"Abc{l}")
                    B_bc = sm.tile([P, HID], F32, name=f"Bbc{l}")
                    bcast256(sm, A_bc, A2[0:1, :], A2[1:2, :])
                    bcast256(sm, B_bc, B2[0:1, :], B2[1:2, :])

                    # ---- apply pass ----
                    for t in range(c.NPT):
                        u1 = tl.tile([P, HID], F32, tag="u1")
                        nc.vector.tensor_tensor(out=u1[:], in0=o_sb[:, t, :],
                                                in1=A_bc[:], op=ALU.mult)
                        nc.vector.tensor_add(u1[:], u1[:], B_bc[:])
                        u2 = tl.tile([P, HID], F32, tag="u2")
                        nc.vector.tensor_scalar_min(u2[:], u1[:], 0.0)
                        nc.scalar.activation(u2[:], u2[:], ACT.Exp)
                        rl = tl.tile([P, HID], F32, tag="rl")
                        nc.scalar.activation(rl[:], u1[:], ACT.Relu)
                        nc.vector.tensor_add(u2[:], u2[:], rl[:])
                        nc.vector.tensor_scalar_add(u2[:], u2[:], -1.0)
                        hp = tl.tile([P, HID], F32, tag="hp")
                        src = hshort if l == 0 else h_cur[l - 1]
                        nc.sync.dma_start(hp[:], src[t * P:(t + 1) * P, :])
                        nc.vector.tensor_add(u2[:], u2[:], hp[:])
                        nc.sync.dma_start(h_cur[l][t * P:(t + 1) * P, :], u2[:])
                        if dbg:
                            nc.sync.dma_start(d_h[l][t * P:(t + 1) * P, :],
                                              u2[:])
                        nc.vector.tensor_copy(h_bf[:, t, :], u2[:])

            # ---------------- readout ----------------
            h3tbl = dram.tile([c.NP + P, HID], BF16)
            for t in range(c.NPT):
                nc.sync.dma_start(h3tbl[t * P:(t + 1) * P, :], h_bf[:, t, :])
            with (
                tc.tile_pool(name="rd", bufs=2) as rd,
                tc.tile_pool(name="rs1", bufs=1) as rs1,
            ):
                sent0 = rs1.tile([1, HID], BF16, name="sent0")
                nc.gpsimd.memset(sent0[:], 0.0)
                sent1 = rs1.tile([1, HID], BF16, name="sent1")
                nc.gpsimd.memset(sent1[:], -1e30)
                nc.sync.dma_start(h3tbl[c.NP:c.NP + 1, :], sent0[:])
                nc.sync.dma_start(h3tbl[c.NP + 1:c.NP + 2, :], sent1[:])

                rsum_sb = rs1.tile([P, 2, c.GW], F32, name="rsum_sb")
                rmax_sb = rs1.tile([P, 2, c.GW], F32, name="rmax_sb")
                GSZ = 512
                gpg = GSZ // c.KS
                for j in range(c.RN // GSZ):
                    isl = slice(j * (GSZ // 16), (j + 1) * (GSZ // 16))
                    gsl = slice(j * gpg, (j + 1) * gpg)
                    gr = rd.tile([P, 2, GSZ], BF16, tag="gr")
                    nc.gpsimd.dma_gather(
                        out_ap=gr[:], in_ap=h3tbl[:, :],
                        idxs_ap=ridxs_sb[:, isl],
                        num_idxs=GSZ, num_idxs_reg=GSZ, elem_size=HID,
                        transpose=True)
                    nc.vector.tensor_reduce(
                        rsum_sb[:, :, gsl],
                        gr[:].rearrange("p b (g k) -> p b g k", k=c.KS),
                        axis=AX.X, op=ALU.add)
                    gm = rd.tile([P, 2, GSZ], BF16, tag="gm")
                    nc.gpsimd.dma_gather(
                        out_ap=gm[:], in_ap=h3tbl[:, :],
                        idxs_ap=ridxm_sb[:, isl],
                        num_idxs=GSZ, num_idxs_reg=GSZ, elem_size=HID,
                        transpose=True)
                    nc.vector.tensor_reduce(
                        rmax_sb[:, :, gsl],
                        gm[:].rearrange("p b (g k) -> p b g k", k=c.KS),
                        axis=AX.X, op=ALU.max)

                rsum_bf = rs1.tile([P, 2, c.GW], BF16, name="rsum_bf")
                nc.vector.tensor_copy(rsum_bf[:], rsum_sb[:])
                rmax_bf = rs1.tile([P, 2, c.GW], BF16, name="rmax_bf")
                nc.vector.tensor_copy(rmax_bf[:], rmax_sb[:])
                nc.sync.dma_start(rsum_o[:, :, :].transpose([1, 0, 2]),
                                  rsum_bf[:])
                nc.sync.dma_start(rmax_o[:, :, :].transpose([1, 0, 2]),
                                  rmax_bf[:])

    nc.compile()
    return nc


# ---------------------------------------------------------------------------
# host-side prep
# ---------------------------------------------------------------------------

def host_prep(cfg: Cfg, x, edge_index, batch):
    c = cfg
    bf = ml_dtypes.bfloat16
    n = c.N
    loop = np.arange(n, dtype=np.int64)
    src = np.concatenate([np.asarray(edge_index[0], np.int64), loop])
    dst = np.concatenate([np.asarray(edge_index[1], np.int64), loop])
    src_slot = ((src // c.NR) * c.NP + src % c.NR).astype(np.int64)
    dst_core = dst // c.NR
    dst_local = (dst % c.NR).astype(np.int64)
    batch = np.asarray(batch, np.int64)

    per_core = []
    gfirsts = []
    for cc in range(c.NCORES):
        m = dst_core == cc
        es = src_slot[m]
        ed = dst_local[m]
        order = np.argsort(ed, kind="stable")
        es, ed = es[order], ed[order]
        tile_id = ed // P
        counts = np.bincount(tile_id, minlength=c.NPT)
        if counts.max() > c.ET:
            raise OverflowError("edge tile capacity exceeded")
        starts = np.zeros(c.NPT, np.int64)
        np.cumsum(counts[:-1], out=starts[1:])
        pos_in_tile = np.arange(len(ed)) - starts[tile_id]
        es_f = np.full(c.NPT * c.ET, c.PAD_SLOT, np.int64)
        ed_f = np.zeros(c.NPT * c.ET, np.int64)
        df_f = np.full(c.NPT * c.ET, -1, np.int64)
        slot = tile_id * c.ET + pos_in_tile
        es_f[slot] = es
        ed_f[slot] = ed
        df_f[slot] = ed % P
        # slot s of tile t -> (p = s % 128, col = t*EB + s//128)
        es2d = np.ascontiguousarray(
            es_f.reshape(c.NPT, c.EB, P).transpose(2, 0, 1).reshape(
                P, c.ECB)).astype(np.uint16)
        ed2d = np.ascontiguousarray(
            ed_f.reshape(c.NPT, c.EB, P).transpose(2, 0, 1).reshape(
                P, c.ECB)).astype(np.int16)
        df2d = np.ascontiguousarray(
            df_f.reshape(c.NPT, c.EB, P).transpose(2, 0, 1).reshape(
                P, c.ECB)).astype(np.int8)

        bsl = batch[cc * c.NR:(cc + 1) * c.NR]
        gfirst = int(bsl[0])
        gfirsts.append(gfirst)
        w = (bsl - gfirst).astype(np.int64)
        uniq, first_idx = np.unique(w, return_index=True)
        fi = np.zeros(int(w[-1]) + 1, np.int64)
        fi[uniq] = first_idx
        kwi = np.arange(c.NR) - fi[w]
        if int(w[-1]) >= c.GW or int(kwi.max()) >= c.KS:
            raise OverflowError("readout window exceeded")
        sidx_s = np.full(c.RN, c.NP, np.int64)
        sidx_m = np.full(c.RN, c.NP + 1, np.int64)
        pos = w * c.KS + kwi
        sidx_s[pos] = np.arange(c.NR)
        sidx_m[pos] = np.arange(c.NR)
        ridxs_a = sidx_s.reshape(c.RN // 16, 16).T.astype(np.int16)
        ridxm_a = sidx_m.reshape(c.RN // 16, 16).T.astype(np.int16)

        xp = np.zeros((c.NP, c.IN), np.float32)
        xp[:c.NR] = x[cc * c.NR:(cc + 1) * c.NR]

        per_core.append(dict(
            x_in=xp.astype(bf), esrc=es2d, edst32=ed2d, dstf=df2d,
            ridxs=ridxs_a, ridxm=ridxm_a))
    return per_core, gfirsts


def host_weights(cfg: Cfg, W_in, gW0, gW1, gW2, b_in, gb, bng, bnb, a_s, a_d):
    bf = ml_dtypes.bfloat16
    wcat0 = np.concatenate([np.asarray(W_in, np.float32),
                            np.asarray(gW0, np.float32)], axis=1).astype(bf)
    pvec = np.zeros((21, P), np.float32)
    pvec[0:2] = np.asarray(b_in, np.float32).reshape(2, P)
    for l in range(3):
        pvec[2 + 6 * l:4 + 6 * l] = np.asarray(gb[l], np.float32).reshape(2, P)
        pvec[4 + 6 * l:6 + 6 * l] = np.asarray(bng[l], np.float32).reshape(2, P)
        pvec[6 + 6 * l:8 + 6 * l] = np.asarray(bnb[l], np.float32).reshape(2, P)
    pvec[20] = np.arange(P, dtype=np.float32)
    asd = np.zeros((6, cfg.HID), np.float32)
    for l in range(3):
        asd[2 * l] = np.asarray(a_s[l], np.float32).reshape(-1)
        asd[2 * l + 1] = np.asarray(a_d[l], np.float32).reshape(-1)
    return dict(wcat0=wcat0, w1=np.asarray(gW1, np.float32).astype(bf),
                w2=np.asarray(gW2, np.float32).astype(bf), pvec=pvec, asd=asd)


def host_finish(cfg: Cfg, outs, gfirsts, batch, mW1, mb1, mg1, mbeta1,
                mW2, mb2, mg2, mbeta2, hW, hb):
    c = cfg
    batch = np.asarray(batch, np.int64)
    cnt = np.bincount(batch, minlength=c.G).astype(np.float32)
    hsum = np.zeros((c.G, c.HID), np.float32)
    hmax = np.full((c.G, c.HID), -np.inf, np.float32)
    for cc in range(c.NCORES):
        g0 = gfirsts[cc]
        ng = min(c.GW, c.G - g0)
        rs = np.asarray(outs[cc]["rsum_o"], np.float32).reshape(
            2 * P, c.GW)[:c.HID, :ng].T
        rm = np.asarray(outs[cc]["rmax_o"], np.float32).reshape(
            2 * P, c.GW)[:c.HID, :ng].T
        hsum[g0:g0 + ng] += rs
        hmax[g0:g0 + ng] = np.maximum(hmax[g0:g0 + ng], rm)
    hmean = hsum / np.maximum(cnt, 1.0)[:, None]
    hmax = np.where((cnt[:, None] > 0) & (hmax > -1e29), hmax, 0.0)
    hg = np.concatenate([hmean, hmax], axis=1).astype(np.float32)

    def bn(h, g, b):
        mu = h.mean(0, dtype=np.float32)
        v = ((h - mu) ** 2).mean(0, dtype=np.float32)
        return (h - mu) / np.sqrt(v + EPS) * g + b

    s = np.maximum(bn(hg @ np.asarray(mW1, np.float32) + mb1, mg1, mbeta1), 0.0)
    s = np.maximum(bn(s @ np.asarray(mW2, np.float32) + mb2, mg2, mbeta2), 0.0)
    return (s @ np.asarray(hW, np.float32) + hb).astype(np.float32)


# ---------------------------------------------------------------------------
# persistent PJRT runner (compile once, reuse)
# ---------------------------------------------------------------------------

class Runner:
    def __init__(self, nc, n_cores):
        import jax
        from jax.sharding import Mesh, PartitionSpec
        from jax.experimental.shard_map import shard_map
        from concourse import bass2jax
        try:
            jax.config.update("jax_compilation_cache_dir", "/tmp/jax_pcc")
            jax.config.update("jax_persistent_cache_min_entry_size_bytes", -1)
            jax.config.update("jax_persistent_cache_min_compile_time_secs", 0)
        except Exception:
            pass
        bass2jax.install_neuronx_cc_hook()
        self.nc = nc
        self.n_cores = n_cores
        partition_name = (nc.partition_id_tensor.name
                          if getattr(nc, "partition_id_tensor", None) is not None
                          else None)
        in_names, out_names, out_avals, zero_shapes = [], [], [], []
        self.in_specs = {}
        for alloc in nc.m.functions[0].allocations:
            if not isinstance(alloc, mybir.MemoryLocationSet):
                continue
            name = alloc.memorylocations[0].name
            if alloc.kind == "ExternalInput":
                if name == partition_name:
                    continue
                in_names.append(name)
                self.in_specs[name] = (tuple(alloc.tensor_shape),
                                       mybir.dt.np(alloc.dtype))
            elif alloc.kind == "ExternalOutput":
                shape = tuple(alloc.tensor_shape)
                dtype = mybir.dt.np(alloc.dtype)
                out_names.append(name)
                out_avals.append(jax.core.ShapedArray(shape, dtype))
                zero_shapes.append((shape, dtype))
        self.in_names = in_names
        self.out_names = out_names
        self.out_avals = out_avals
        self.zero_shapes = zero_shapes
        n_params = len(in_names)
        all_names = list(in_names) + list(out_names)
        if partition_name is not None:
            all_names.append(partition_name)
        donate = tuple(range(n_params, n_params + len(out_names)))

        def _body(*args):
            operands = list(args)
            if partition_name is not None:
                operands.append(bass2jax.partition_id_tensor())
            outs = bass2jax._bass_exec_p.bind(
                *operands,
                out_avals=tuple(out_avals),
                in_names=tuple(all_names),
                out_names=tuple(out_names),
                lowering_input_output_aliases=(),
                sim_require_finite=False,
                sim_require_nnan=False,
                nc=nc,
            )
            return tuple(outs)

        devices = jax.devices()[:n_cores]
        mesh = Mesh(np.asarray(devices), ("core",))
        from jax.sharding import NamedSharding
        self._zero_sh = NamedSharding(mesh, PartitionSpec("core"))
        nin = n_params + len(out_names)
        self._fn = jax.jit(
            shard_map(_body, mesh=mesh,
                      in_specs=(PartitionSpec("core"),) * nin,
                      out_specs=(PartitionSpec("core"),) * len(out_names),
                      check_rep=False),
            donate_argnums=donate, keep_unused=True)

    def run(self, in_maps):
        import jax.numpy as jnp
        concat = [np.concatenate([np.asarray(m[nm]) for m in in_maps], axis=0)
                  for nm in self.in_names]
        zeros = [jnp.zeros((self.n_cores * s[0], *s[1:]), d,
                           device=self._zero_sh)
                 for s, d in self.zero_shapes]
        out_arrs = self._fn(*concat, *zeros)
        res = []
        for cc in range(self.n_cores):
            res.append({nm: np.asarray(out_arrs[i]).reshape(
                self.n_cores, *self.out_avals[i].shape)[cc]
                for i, nm in enumerate(self.out_names)})
        return res

    def warm(self):
        in_maps = []
        for cc in range(self.n_cores):
            m = {nm: np.zeros(sh, dt)
                 for nm, (sh, dt) in self.in_specs.items()}
            in_maps.append(m)
        self.run(in_maps)


_RUNNER = None


def _ensure_runner():
    global _RUNNER
    if _RUNNER is None:
        nc = build_nc(FULL)
        _RUNNER = Runner(nc, FULL.NCORES)
        _RUNNER.warm()
    return _RUNNER


# ---------------------------------------------------------------------------
# entry point
# ---------------------------------------------------------------------------

def _kernel_numpy(x, edge_index, batch, W_in, b_in, gW, gas, gad, gb, bng,
                  bnb, mW1, mb1, mg1, mbeta1, mW2, mb2, mg2, mbeta2, hW, hb):
    # pure-host fallback (slow) in case device capacity assumptions fail
    n = x.shape[0]
    G = FULL.G
    loop = np.arange(n)
    src = np.concatenate([edge_index[0], loop])
    dst = np.concatenate([edge_index[1], loop])
    order = np.argsort(dst, kind="stable")
    srcs, dsts = src[order], dst[order]
    counts = np.bincount(dsts, minlength=n)
    starts = np.zeros(n, np.int64)
    np.cumsum(counts[:-1], out=starts[1:])

    def bn(h, g, b):
        mu = h.mean(0)
        v = ((h - mu) ** 2).mean(0)
        return (h - mu) / np.sqrt(v + EPS) * g + b

    h_short = x @ W_in + b_in
    h = x
    for i in range(3):
        xw = (h @ gW[i]).reshape(n, HEADS, -1)
        ssum = np.einsum("nhc,hc->nh", xw, gas[i])
        dsum = np.einsum("nhc,hc->nh", xw, gad[i])
        e = ssum[srcs] + dsum[dsts]
        e = np.where(e > 0, e, SLOPE * e)
        m = np.maximum.reduceat(e, starts, axis=0)
        ex = np.exp(e - m[dsts])
        den = np.add.reduceat(ex, starts, axis=0)
        alpha = ex / den[dsts]
        out = np.add.reduceat(xw[srcs] * alpha[:, :, None], starts,
                              axis=0).reshape(n, -1) + gb[i]
        hn = bn(out, bng[i], bnb[i])
        hn = np.where(hn > 0, hn, np.expm1(np.minimum(hn, 0)))
        h = hn + (h_short if i == 0 else h)
    cnt = np.bincount(batch, minlength=G).astype(np.float32)
    hsum = np.zeros((G, h.shape[1]), np.float32)
    np.add.at(hsum, batch, h)
    hmax = np.full((G, h.shape[1]), -np.inf, np.float32)
    np.maximum.at(hmax, batch, h)
    hmax = np.where(cnt[:, None] > 0, hmax, 0.0)
    hg = np.concatenate([hsum / np.maximum(cnt, 1.0)[:, None], hmax], axis=1)
    s = np.maximum(bn(hg @ mW1 + mb1, mg1, mbeta1), 0.0)
    s = np.maximum(bn(s @ mW2 + mb2, mg2, mbeta2), 0.0)
    return (s @ hW + hb).astype(np.float32)


def kernel(x, edge_index, batch, W_in, b_in, gW0, gas0, gad0, gb0, bng0, bnb0,
           gW1, gas1, gad1, gb1, bng1, bnb1, gW2, gas2, gad2, gb2, bng2, bnb2,
           mW1, mb1, mg1, mbeta1, mW2, mb2, mg2, mbeta2, hW, hb):
    c = FULL
    x = np.asarray(x, np.float32)
    edge_index = np.asarray(edge_index)
    batch = np.asarray(batch)
    try:
        runner = _ensure_runner()
        per_core, gfirsts = host_prep(c, x, edge_index, batch)
        wmap = host_weights(c, W_in, gW0, gW1, gW2, b_in,
                            [gb0, gb1, gb2], [bng0, bng1, bng2],
                            [bnb0, bnb1, bnb2],
                            [gas0, gas1, gas2], [gad0, gad1, gad2])
        in_maps = [dict(pc, **wmap) for pc in per_core]
        outs = runner.run(in_maps)
        return host_finish(c, outs, gfirsts, batch, mW1, mb1, mg1, mbeta1,
                           mW2, mb2, mg2, mbeta2, hW, hb)
    except OverflowError:
        return _kernel_numpy(
            x, edge_index, batch,
            np.asarray(W_in, np.float32), np.asarray(b_in, np.float32),
            [np.asarray(w, np.float32) for w in (gW0, gW1, gW2)],
            [np.asarray(w, np.float32) for w in (gas0, gas1, gas2)],
            [np.asarray(w, np.float32) for w in (gad0, gad1, gad2)],
            [np.asarray(w, np.float32) for w in (gb0, gb1, gb2)],
            [np.asarray(w, np.float32) for w in (bng0, bng1, bng2)],
            [np.asarray(w, np.float32) for w in (bnb0, bnb1, bnb2)],
            np.asarray(mW1, np.float32), mb1, mg1, mbeta1,
            np.asarray(mW2, np.float32), mb2, mg2, mbeta2,
            np.asarray(hW, np.float32), hb)


if os.environ.get("BASS_GNN_LAZY", "") != "1":
    _ensure_runner()


# revision 33
# speedup vs baseline: 1.2973x; 1.2973x over previous
import os
import sys

for _p in ("/opt/trn_rl_repo", "/root/.axon_site/_ro/trn_rl_repo"):
    if _p not in sys.path:
        sys.path.insert(0, _p)

import numpy as np
import ml_dtypes

import concourse.bass as bass
import concourse.bacc as bacc
import concourse.mybir as mybir
import concourse.tile as tile
from concourse.masks import make_identity

F32 = mybir.dt.float32
BF16 = mybir.dt.bfloat16
I32 = mybir.dt.int32
I16 = mybir.dt.int16
ALU = mybir.AluOpType
ACT = mybir.ActivationFunctionType
AX = mybir.AxisListType

P = 128
EPS = 1e-5
SLOPE = 0.2
HEADS = 4


class Cfg:
    def __init__(self, ncores=8, nreal=40000, npc=5120, in_dim=64, hid=256,
                 eb=11, gw=224, ks=64, g=1500):
        self.NCORES = ncores
        self.N = nreal                      # real nodes total
        self.NR = nreal // ncores           # real nodes per core
        self.NP = npc                       # padded node slots per core
        self.NT = npc * ncores              # total table slots
        self.IN = in_dim
        self.HID = hid
        self.C = hid // HEADS
        self.EB = eb                        # edge blocks per dst-tile
        self.ET = eb * P                    # edge slot capacity per dst-tile
        self.NPT = npc // P                 # dst-tiles per core
        self.ECB = self.NPT * eb            # edge array free blocks
        self.GW = gw                        # graph window per core
        self.KS = ks                        # slots per graph
        self.RN = gw * ks                   # readout gather count
        self.G = g
        assert npc % P == 0 and self.RN % 512 == 0
        self.PAD_SLOT = self.NR             # global slot with zeroed table row


FULL = Cfg()


# ---------------------------------------------------------------------------
# device program
# ---------------------------------------------------------------------------

def build_nc(cfg: Cfg, dbg=False):
    c = cfg
    HID = c.HID
    nc = bacc.Bacc("TRN2", target_bir_lowering=False, debug=False,
                   num_devices=c.NCORES, num_swdge_queues=1)

    x_in = nc.dram_tensor("x_in", [c.NP, c.IN], BF16, kind="ExternalInput")
    wcat0 = nc.dram_tensor("wcat0", [c.IN, 2 * HID], BF16, kind="ExternalInput")
    w1 = nc.dram_tensor("w1", [HID, HID], BF16, kind="ExternalInput")
    w2 = nc.dram_tensor("w2", [HID, HID], BF16, kind="ExternalInput")
    pvec = nc.dram_tensor("pvec", [21, P], F32, kind="ExternalInput")
    asd = nc.dram_tensor("asd", [6, HID], F32, kind="ExternalInput")
    esrc = nc.dram_tensor("esrc", [P, c.ECB], mybir.dt.uint16, kind="ExternalInput")
    edst32 = nc.dram_tensor("edst32", [P, c.ECB], I16, kind="ExternalInput")
    dstf = nc.dram_tensor("dstf", [P, c.ECB], mybir.dt.int8, kind="ExternalInput")
    ridxs = nc.dram_tensor("ridxs", [16, c.RN // 16], I16, kind="ExternalInput")
    ridxm = nc.dram_tensor("ridxm", [16, c.RN // 16], I16, kind="ExternalInput")

    rsum_o = nc.dram_tensor("rsum_o", [2, P, c.GW], BF16, kind="ExternalOutput")
    rmax_o = nc.dram_tensor("rmax_o", [2, P, c.GW], BF16, kind="ExternalOutput")
    if dbg:
        d_h = [nc.dram_tensor(f"d_h{l}", [c.NP, c.HID], F32,
                              kind="ExternalOutput") for l in range(3)]
        d_o = nc.dram_tensor("d_o", [c.NP, c.HID], F32, kind="ExternalOutput")
        d_g1 = nc.dram_tensor("d_g1", [P, c.EB, c.HID + 16], F32, kind="ExternalOutput")
        d_g2 = nc.dram_tensor("d_g2", [P, c.EB, 8], F32, kind="ExternalOutput")
        d_ex = nc.dram_tensor("d_ex", [P, c.EB, 4], F32, kind="ExternalOutput")
        d_pay = nc.dram_tensor("d_pay", [P, c.EB, 260], F32, kind="ExternalOutput")
        d_pt = nc.dram_tensor("d_pt", [P, 260], F32, kind="ExternalOutput")

    RG = [list(range(c.NCORES))]

    with tile.TileContext(nc) as tc:
        with (
            tc.tile_pool(name="dram", bufs=1, space="DRAM") as dram,
            tc.tile_pool(name="const", bufs=1) as cpool,
            tc.tile_pool(name="persist", bufs=1) as pers,
        ):
            # ---------------- constants / loads ----------------
            ident = cpool.tile([P, P], BF16)
            make_identity(nc, ident[:])
            ones = cpool.tile([P, 1], F32)
            nc.gpsimd.memset(ones[:], 1.0)
            m39 = cpool.tile([P, 1], F32)
            nc.gpsimd.memset(m39[:], 0.0)
            npad = c.NR - (c.NPT - 1) * P   # real rows in the last tile
            nc.gpsimd.memset(m39[0:npad, :], 1.0)

            wc0_sb = cpool.tile([c.IN, 2 * HID], BF16)
            nc.sync.dma_start(wc0_sb[:], wcat0[:, :])
            w1_sb = cpool.tile([P, 2, HID], BF16)
            w2_sb = cpool.tile([P, 2, HID], BF16)
            for hf in range(2):
                nc.sync.dma_start(w1_sb[:, hf, :], w1[hf * P:(hf + 1) * P, :])
                nc.sync.dma_start(w2_sb[:, hf, :], w2[hf * P:(hf + 1) * P, :])

            def bcast256(pool, dst, src0, src1):
                scr = pool.tile([1, 2 * P], F32, tag="bscr", bufs=2)
                nc.sync.dma_start(scr[0:1, 0:P], src0)
                nc.sync.dma_start(scr[0:1, P:2 * P], src1)
                nc.gpsimd.partition_broadcast(dst[:, :], scr[0:1, :])

            as_bc, ad_bc, gb_bc = [], [], []
            for l in range(3):
                a_s = pers.tile([P, HID], F32, name=f"as_bc{l}")
                a_d = pers.tile([P, HID], F32, name=f"ad_bc{l}")
                bcast256(pers, a_s, asd[2 * l:2 * l + 1, 0:P],
                         asd[2 * l:2 * l + 1, P:2 * P])
                bcast256(pers, a_d, asd[2 * l + 1:2 * l + 2, 0:P],
                         asd[2 * l + 1:2 * l + 2, P:2 * P])
                as_bc.append(a_s)
                ad_bc.append(a_d)
                g_b = pers.tile([P, HID], F32, name=f"gb_bc{l}")
                bcast256(pers, g_b, pvec[2 + 6 * l:3 + 6 * l, :],
                         pvec[3 + 6 * l:4 + 6 * l, :])
                gb_bc.append(g_b)
            bin_bc = pers.tile([P, HID], F32)
            bcast256(pers, bin_bc, pvec[0:1, :], pvec[1:2, :])
            iota_bc = pers.tile([P, P], F32)
            scr0 = pers.tile([1, P], F32, name="scr0")
            nc.sync.dma_start(scr0[0:1, :], pvec[20:21, :])
            nc.gpsimd.partition_broadcast(iota_bc[:, :], scr0[0:1, :])

            esrc_16 = pers.tile([P, c.ECB], mybir.dt.uint16, name="esrc16")
            nc.sync.dma_start(esrc_16[:], esrc[:, :])
            esrc_sb = pers.tile([P, c.ECB], I32)
            nc.vector.tensor_copy(esrc_sb[:], esrc_16[:])
            edst_16 = pers.tile([P, c.ECB], I16, name="edst16t")
            nc.sync.dma_start(edst_16[:], edst32[:, :])
            edst32_sb = pers.tile([P, c.ECB], I32)
            nc.vector.tensor_copy(edst32_sb[:], edst_16[:])
            dstf_8 = pers.tile([P, c.ECB], mybir.dt.int8, name="dstf8")
            nc.sync.dma_start(dstf_8[:], dstf[:, :])
            dstf_sb = pers.tile([P, c.ECB], F32)
            nc.vector.tensor_copy(dstf_sb[:], dstf_8[:])
            ridxs_sb = pers.tile([P, c.RN // 16], I16)
            ridxm_sb = pers.tile([P, c.RN // 16], I16)
            for k in range(8):
                nc.sync.dma_start(ridxs_sb[16 * k:16 * (k + 1), :], ridxs[:, :])
                nc.sync.dma_start(ridxm_sb[16 * k:16 * (k + 1), :], ridxm[:, :])

            x_bf = pers.tile([P, c.NPT, c.IN], BF16)
            for t in range(c.NPT):
                nc.sync.dma_start(x_bf[:, t, :], x_in[t * P:(t + 1) * P, :])

            h_bf = pers.tile([P, c.NPT, HID], BF16)
            o_sb = pers.tile([P, c.NPT, HID], F32)

            hshort = dram.tile([c.NP, HID], F32)
            h_cur = [dram.tile([c.NP, HID], F32, name=f"h{l}") for l in range(3)]

            # ---------------- layers ----------------
            for l in range(3):
                tbl_own = dram.tile([c.NP, HID + 16], BF16, name=f"tblo{l}")
                tblS_own = dram.tile([c.NP, 8], F32, name=f"tblso{l}")
                tbl = dram.tile([c.NT, HID + 16], BF16, name=f"tbl{l}",
                                addr_space="Shared")
                stats_own = dram.tile([4, P], F32, name=f"sto{l}")
                stats_all = dram.tile([4, P], F32, name=f"sta{l}",
                                      addr_space="Shared")

                with (
                    tc.tile_pool(name=f"tl{l}", bufs=2) as tl,
                    tc.tile_pool(name=f"tps{l}", bufs=2, space="PSUM") as tps,
                    tc.tile_pool(name=f"sps{l}", bufs=1, space="PSUM") as sps,
                    tc.tile_pool(name=f"ep{l}", bufs=2) as ep,
                    tc.tile_pool(name=f"sm{l}", bufs=1) as sm,
                ):
                    # ---- table build (own slice) ----
                    for t in range(c.NPT):
                        if l == 0:
                            xt = x_bf[:, t, :]
                            tp = tps.tile([P, P], BF16, tag="tp")
                            hT = tl.tile([P, 2, P], BF16, tag="hT")
                            nc.tensor.transpose(tp[0:c.IN, :], xt, ident[:])
                            nc.vector.tensor_copy(hT[0:c.IN, 0, :], tp[0:c.IN, :])
                            xw_ps = tps.tile([P, 2 * HID], F32, tag="xwps")
                            nc.tensor.matmul(xw_ps[:, :], hT[0:c.IN, 0, :],
                                             wc0_sb[:, :], start=True, stop=True)
                            hs = tl.tile([P, HID], F32, tag="hs")
                            nc.vector.tensor_add(hs[:], xw_ps[:, 0:HID], bin_bc[:])
                            nc.sync.dma_start(hshort[t * P:(t + 1) * P, :], hs[:])
                            xw = xw_ps[:, HID:2 * HID]
                        else:
                            ht = h_bf[:, t, :]
                            wsb = w1_sb if l == 1 else w2_sb
                            xw_ps = tps.tile([P, 2 * HID], F32, tag="xwps")
                            hT = tl.tile([P, 2, P], BF16, tag="hT")
                            for hf in range(2):
                                tp = tps.tile([P, P], BF16, tag="tp")
                                nc.tensor.transpose(
                                    tp[:, :], ht[:, hf * P:(hf + 1) * P], ident[:])
                                nc.vector.tensor_copy(hT[:, hf, :], tp[:, :])
                            for hf in range(2):
                                nc.tensor.matmul(xw_ps[:, 0:HID], hT[:, hf, :],
                                                 wsb[:, hf, :], start=(hf == 0),
                                                 stop=(hf == 1))
                            xw = xw_ps[:, 0:HID]

                        xwb = tl.tile([P, HID], BF16, tag="xwb")
                        if t == c.NPT - 1:
                            nc.vector.tensor_scalar_mul(xwb[:], xw, m39[:, 0:1])
                        else:
                            nc.vector.tensor_copy(xwb[:], xw)
                        nc.sync.dma_start(tbl_own[t * P:(t + 1) * P, 0:HID],
                                          xwb[:])

                        sd = tl.tile([P, 8], F32, tag="sd")
                        sc = tl.tile([P, c.C], F32, tag="sc")
                        for hd in range(4):
                            sl = slice(hd * c.C, (hd + 1) * c.C)
                            nc.vector.tensor_tensor(
                                out=sc[:], in0=xw[:, sl], in1=as_bc[l][:, sl],
                                op=ALU.mult)
                            nc.vector.tensor_reduce(
                                sd[:, hd:hd + 1], sc[:], axis=AX.X, op=ALU.add)
                            nc.vector.tensor_tensor(
                                out=sc[:], in0=xw[:, sl], in1=ad_bc[l][:, sl],
                                op=ALU.mult)
                            nc.vector.tensor_reduce(
                                sd[:, 4 + hd:5 + hd], sc[:], axis=AX.X,
                                op=ALU.add)
                        nc.sync.dma_start(tblS_own[t * P:(t + 1) * P, :], sd[:])
                        nc.sync.dma_start(
                            tbl_own[t * P:(t + 1) * P, HID:HID + 16],
                            sd[:].bitcast(BF16))

                    nc.gpsimd.collective_compute(
                        "AllGather", ALU.bypass, replica_groups=RG,
                        ins=[tbl_own[:, :].opt()], outs=[tbl[:, :].opt()])

                    # ---- edge phase: gather / attention / segment matmul ----
                    s_o = sm.tile([P, HID], F32, name=f"s_o{l}")
                    s_q = sm.tile([P, HID], F32, name=f"s_q{l}")
                    nc.vector.memset(s_o[:], 0.0)
                    nc.vector.memset(s_q[:], 0.0)
                    for t in range(c.NPT):
                        g1 = ep.tile([P, c.EB, HID + 16], BF16, tag="g1")
                        g2 = ep.tile([P, c.EB, 8], F32, tag="g2")
                        for b in range(c.EB):
                            col = t * c.EB + b
                            nc.gpsimd.indirect_dma_start(
                                out=g1[:, b, :], out_offset=None, in_=tbl[:, :],
                                in_offset=bass.IndirectOffsetOnAxis(
                                    ap=esrc_sb[:, col:col + 1], axis=0))
                            nc.gpsimd.indirect_dma_start(
                                out=g2[:, b, :], out_offset=None,
                                in_=tblS_own[:, :],
                                in_offset=bass.IndirectOffsetOnAxis(
                                    ap=edst32_sb[:, col:col + 1], axis=0))

                        g1sd = g1[:, :, HID:HID + 16].bitcast(F32)
                        ee = ep.tile([P, c.EB, 4], F32, tag="ee")
                        nc.vector.tensor_add(ee[:], g1sd[:, :, 0:4],
                                             g2[:, :, 4:8])
                        e2 = ep.tile([P, c.EB, 4], F32, tag="e2")
                        nc.scalar.mul(e2[:], ee[:], SLOPE)
                        nc.vector.tensor_tensor(out=ee[:], in0=ee[:], in1=e2[:],
                                                op=ALU.max)
                        ex = ep.tile([P, c.EB, 4], F32, tag="ex")
                        nc.scalar.activation(ex[:], ee[:], ACT.Exp)

                        pay = ep.tile([P, c.EB, 260], BF16, tag="pay")
                        for hd in range(4):
                            sl = slice(hd * c.C, (hd + 1) * c.C)
                            nc.vector.tensor_tensor(
                                out=pay[:, :, sl], in0=g1[:, :, sl],
                                in1=ex[:, :, hd:hd + 1].to_broadcast(
                                    [P, c.EB, c.C]),
                                op=ALU.mult)
                        nc.vector.tensor_copy(pay[:, :, 256:260], ex[:])

                        pt = tps.tile([P, 260], F32, tag="pt")
                        for b in range(c.EB):
                            S = ep.tile([P, P], BF16, tag="S")
                            nc.vector.tensor_scalar(
                                out=S[:], in0=iota_bc[:],
                                scalar1=dstf_sb[:, t * c.EB + b:
                                                t * c.EB + b + 1],
                                scalar2=None, op0=ALU.is_equal)
                            nc.tensor.matmul(pt[:, :], S[:], pay[:, b, :],
                                             start=(b == 0),
                                             stop=(b == c.EB - 1))

                        if dbg and l == 0 and t == 0:
                            dcp = tl.tile([P, c.EB, c.HID + 16], F32, tag="dcp")
                            nc.vector.tensor_copy(dcp[:], g1[:])
                            nc.sync.dma_start(d_g1[:, :, :], dcp[:])
                            nc.sync.dma_start(d_g2[:, :, :], g2[:])
                            nc.sync.dma_start(d_ex[:, :, :], ex[:])
                            dcq = tl.tile([P, c.EB, 260], F32, tag="dcq")
                            nc.vector.tensor_copy(dcq[:], pay[:])
                            nc.sync.dma_start(d_pay[:, :, :], dcq[:])
                            dcr = tl.tile([P, 260], F32, tag="dcr")
                            nc.vector.tensor_copy(dcr[:], pt[:])
                            nc.sync.dma_start(d_pt[:, :], dcr[:])

                        # ---- finalize tile: alpha-div, bias, stats ----
                        den = tl.tile([P, 4], F32, tag="den")
                        nc.vector.tensor_scalar_max(den[:], pt[:, 256:260],
                                                    1e-20)
                        rec = tl.tile([P, 4], F32, tag="rec")
                        nc.vector.reciprocal(rec[:], den[:])
                        ot = o_sb[:, t, :]
                        for hd in range(4):
                            sl = slice(hd * c.C, (hd + 1) * c.C)
                            nc.vector.tensor_scalar_mul(ot[:, sl], pt[:, sl],
                                                        rec[:, hd:hd + 1])
                        nc.vector.tensor_add(ot, ot, gb_bc[l][:])
                        if t == c.NPT - 1:
                            om = tl.tile([P, HID], F32, tag="om")
                            nc.vector.tensor_scalar_mul(om[:], ot, m39[:, 0:1])
                            stat_in = om[:]
                        else:
                            stat_in = ot
                        sq = tl.tile([P, HID], F32, tag="sq")
                        nc.vector.tensor_tensor(out=sq[:], in0=stat_in,
                                                in1=stat_in, op=ALU.mult)
                        nc.vector.tensor_add(s_o[:], s_o[:], stat_in)
                        nc.vector.tensor_add(s_q[:], s_q[:], sq[:])

                    if dbg and l == 0:
                        for t in range(c.NPT):
                            nc.sync.dma_start(d_o[t * P:(t + 1) * P, :],
                                              o_sb[:, t, :])

                    # ---- BN stats reduce + collective ----
                    st_ps = sps.tile([P, 4], F32, name=f"stp{l}")
                    for hf in range(2):
                        nc.tensor.matmul(
                            st_ps[:, hf:hf + 1], s_o[:, hf * P:(hf + 1) * P],
                            ones[:], start=True, stop=True,
                            skip_group_check=True)
                        nc.tensor.matmul(
                            st_ps[:, 2 + hf:3 + hf], s_q[:, hf * P:(hf + 1) * P],
                            ones[:], start=True, stop=True,
                            skip_group_check=True)
                    stq = sm.tile([P, 4], F32, name=f"stq{l}")
                    nc.vector.tensor_copy(stq[:], st_ps[:])
                    nc.sync.dma_start(stats_own[:, :].transpose([1, 0]), stq[:])
                    nc.gpsimd.collective_compute(
                        "AllReduce", ALU.add, replica_groups=RG,
                        ins=[stats_own[:, :].opt()], outs=[stats_all[:, :].opt()])

                    st_s = sm.tile([2, P], F32, name=f"sts{l}")
                    nc.sync.dma_start(st_s[:], stats_all[0:2, :])
                    st_q = sm.tile([2, P], F32, name=f"stq2{l}")
                    nc.sync.dma_start(st_q[:], stats_all[2:4, :])
                    g2t = sm.tile([2, P], F32, name=f"g2t{l}")
                    nc.sync.dma_start(g2t[:], pvec[4 + 6 * l:6 + 6 * l, :])
                    be2t = sm.tile([2, P], F32, name=f"be2t{l}")
                    nc.sync.dma_start(be2t[:], pvec[6 + 6 * l:8 + 6 * l, :])
                    mu = sm.tile([2, P], F32, name=f"mu{l}")
                    nc.scalar.mul(mu[:], st_s[:], 1.0 / c.N)
                    va = sm.tile([2, P], F32, name=f"va{l}")
                    nc.scalar.mul(va[:], st_q[:], 1.0 / c.N)
                    mu2 = sm.tile([2, P], F32, name=f"mu2{l}")
                    nc.vector.tensor_tensor(out=mu2[:], in0=mu[:], in1=mu[:],
                                            op=ALU.mult)
                    nc.vector.tensor_tensor(out=va[:], in0=va[:], in1=mu2[:],
                                            op=ALU.subtract)
                    nc.vector.tensor_scalar_add(va[:], va[:], EPS)
                    sdv = sm.tile([2, P], F32, name=f"sdv{l}")
                    nc.scalar.sqrt(sdv[:], va[:])
                    rs = sm.tile([2, P], F32, name=f"rs{l}")
                    nc.vector.reciprocal(rs[:], sdv[:])
                    A2 = sm.tile([2, P], F32, name=f"A2{l}")
                    nc.vector.tensor_tensor(out=A2[:], in0=rs[:], in1=g2t[:],
                                            op=ALU.mult)
                    B2 = sm.tile([2, P], F32, name=f"B2{l}")
                    nc.vector.tensor_tensor(out=B2[:], in0=mu[:], in1=A2[:],
                                            op=ALU.mult)
                    nc.vector.tensor_tensor(out=B2[:], in0=be2t[:],
                                            in1=B2[:], op=ALU.subtract)
                    A_bc = sm.tile([P, HID], F32, name=f"Abc{l}")
                    B_bc = sm.tile([P, HID], F32, name=f"Bbc{l}")
                    bcast256(sm, A_bc, A2[0:1, :], A2[1:2, :])
                    bcast256(sm, B_bc, B2[0:1, :], B2[1:2, :])

                    # ---- apply pass ----
                    for t in range(c.NPT):
                        u1 = tl.tile([P, HID], F32, tag="u1")
                        nc.vector.tensor_tensor(out=u1[:], in0=o_sb[:, t, :],
                                                in1=A_bc[:], op=ALU.mult)
                        nc.vector.tensor_add(u1[:], u1[:], B_bc[:])
                        u2 = tl.tile([P, HID], F32, tag="u2")
                        nc.vector.tensor_scalar_min(u2[:], u1[:], 0.0)
                        nc.scalar.activation(u2[:], u2[:], ACT.Exp)
                        rl = tl.tile([P, HID], F32, tag="rl")
                        nc.scalar.activation(rl[:], u1[:], ACT.Relu)
                        nc.vector.tensor_add(u2[:], u2[:], rl[:])
                        nc.vector.tensor_scalar_add(u2[:], u2[:], -1.0)
                        hp = tl.tile([P, HID], F32, tag="hp")
                        src = hshort if l == 0 else h_cur[l - 1]
                        nc.sync.dma_start(hp[:], src[t * P:(t + 1) * P, :])
                        nc.vector.tensor_add(u2[:], u2[:], hp[:])
                        nc.sync.dma_start(h_cur[l][t * P:(t + 1) * P, :], u2[:])
                        if dbg:
                            nc.sync.dma_start(d_h[l][t * P:(t + 1) * P, :],
                                              u2[:])
                        nc.vector.tensor_copy(h_bf[:, t, :], u2[:])

            # ---------------- readout ----------------
            h3tbl = dram.tile([c.NP + P, HID], BF16)
            for t in range(c.NPT):
                nc.sync.dma_start(h3tbl[t * P:(t + 1) * P, :], h_bf[:, t, :])
            with (
                tc.tile_pool(name="rd", bufs=2) as rd,
                tc.tile_pool(name="rs1", bufs=1) as rs1,
            ):
                sent0 = rs1.tile([1, HID], BF16, name="sent0")
                nc.gpsimd.memset(sent0[:], 0.0)
                sent1 = rs1.tile([1, HID], BF16, name="sent1")
                nc.gpsimd.memset(sent1[:], -1e30)
                nc.sync.dma_start(h3tbl[c.NP:c.NP + 1, :], sent0[:])
                nc.sync.dma_start(h3tbl[c.NP + 1:c.NP + 2, :], sent1[:])

                rsum_sb = rs1.tile([P, 2, c.GW], F32, name="rsum_sb")
                rmax_sb = rs1.tile([P, 2, c.GW], F32, name="rmax_sb")
                GSZ = 512
                gpg = GSZ // c.KS
                for j in range(c.RN // GSZ):
                    isl = slice(j * (GSZ // 16), (j + 1) * (GSZ // 16))
                    gsl = slice(j * gpg, (j + 1) * gpg)
                    gr = rd.tile([P, 2, GSZ], BF16, tag="gr")
                    nc.gpsimd.dma_gather(
                        out_ap=gr[:], in_ap=h3tbl[:, :],
                        idxs_ap=ridxs_sb[:, isl],
                        num_idxs=GSZ, num_idxs_reg=GSZ, elem_size=HID,
                        transpose=True)
                    nc.vector.tensor_reduce(
                        rsum_sb[:, :, gsl],
                        gr[:].rearrange("p b (g k) -> p b g k", k=c.KS),
                        axis=AX.X, op=ALU.add)
                    gm = rd.tile([P, 2, GSZ], BF16, tag="gm")
                    nc.gpsimd.dma_gather(
                        out_ap=gm[:], in_ap=h3tbl[:, :],
                        idxs_ap=ridxm_sb[:, isl],
                        num_idxs=GSZ, num_idxs_reg=GSZ, elem_size=HID,
                        transpose=True)
                    nc.vector.tensor_reduce(
                        rmax_sb[:, :, gsl],
                        gm[:].rearrange("p b (g k) -> p b g k", k=c.KS),
                        axis=AX.X, op=ALU.max)

                rsum_bf = rs1.tile([P, 2, c.GW], BF16, name="rsum_bf")
                nc.vector.tensor_copy(rsum_bf[:], rsum_sb[:])
                rmax_bf = rs1.tile([P, 2, c.GW], BF16, name="rmax_bf")
                nc.vector.tensor_copy(rmax_bf[:], rmax_sb[:])
                nc.sync.dma_start(rsum_o[:, :, :].transpose([1, 0, 2]),
                                  rsum_bf[:])
                nc.sync.dma_start(rmax_o[:, :, :].transpose([1, 0, 2]),
                                  rmax_bf[:])

    nc.compile()
    return nc


# ---------------------------------------------------------------------------
# host-side prep
# ---------------------------------------------------------------------------

def host_prep(cfg: Cfg, x, edge_index, batch):
    c = cfg
    bf = ml_dtypes.bfloat16
    n = c.N
    loop = np.arange(n, dtype=np.int64)
    src = np.concatenate([np.asarray(edge_index[0], np.int64), loop])
    dst = np.concatenate([np.asarray(edge_index[1], np.int64), loop])
    src_slot = ((src // c.NR) * c.NP + src % c.NR).astype(np.int64)
    dst_core = dst // c.NR
    dst_local = (dst % c.NR).astype(np.int64)
    batch = np.asarray(batch, np.int64)

    per_core = []
    gfirsts = []
    for cc in range(c.NCORES):
        m = dst_core == cc
        es = src_slot[m]
        ed = dst_local[m]
        order = np.argsort(ed, kind="stable")
        es, ed = es[order], ed[order]
        tile_id = ed // P
        counts = np.bincount(tile_id, minlength=c.NPT)
        if counts.max() > c.ET:
            raise OverflowError("edge tile capacity exceeded")
        starts = np.zeros(c.NPT, np.int64)
        np.cumsum(counts[:-1], out=starts[1:])
        pos_in_tile = np.arange(len(ed)) - starts[tile_id]
        es_f = np.full(c.NPT * c.ET, c.PAD_SLOT, np.int64)
        ed_f = np.zeros(c.NPT * c.ET, np.int64)
        df_f = np.full(c.NPT * c.ET, -1, np.int64)
        slot = tile_id * c.ET + pos_in_tile
        es_f[slot] = es
        ed_f[slot] = ed
        df_f[slot] = ed % P
        # slot s of tile t -> (p = s % 128, col = t*EB + s//128)
        es2d = np.ascontiguousarray(
            es_f.reshape(c.NPT, c.EB, P).transpose(2, 0, 1).reshape(
                P, c.ECB)).astype(np.uint16)
        ed2d = np.ascontiguousarray(
            ed_f.reshape(c.NPT, c.EB, P).transpose(2, 0, 1).reshape(
                P, c.ECB)).astype(np.int16)
        df2d = np.ascontiguousarray(
            df_f.reshape(c.NPT, c.EB, P).transpose(2, 0, 1).reshape(
                P, c.ECB)).astype(np.int8)

        bsl = batch[cc * c.NR:(cc + 1) * c.NR]
        gfirst = int(bsl[0])
        gfirsts.append(gfirst)
        w = (bsl - gfirst).astype(np.int64)
        uniq, first_idx = np.unique(w, return_index=True)
        fi = np.zeros(int(w[-1]) + 1, np.int64)
        fi[uniq] = first_idx
        kwi = np.arange(c.NR) - fi[w]
        if int(w[-1]) >= c.GW or int(kwi.max()) >= c.KS:
            raise OverflowError("readout window exceeded")
        sidx_s = np.full(c.RN, c.NP, np.int64)
        sidx_m = np.full(c.RN, c.NP + 1, np.int64)
        pos = w * c.KS + kwi
        sidx_s[pos] = np.arange(c.NR)
        sidx_m[pos] = np.arange(c.NR)
        ridxs_a = sidx_s.reshape(c.RN // 16, 16).T.astype(np.int16)
        ridxm_a = sidx_m.reshape(c.RN // 16, 16).T.astype(np.int16)

        xp = np.zeros((c.NP, c.IN), np.float32)
        xp[:c.NR] = x[cc * c.NR:(cc + 1) * c.NR]

        per_core.append(dict(
            x_in=xp.astype(bf), esrc=es2d, edst32=ed2d, dstf=df2d,
            ridxs=ridxs_a, ridxm=ridxm_a))
    return per_core, gfirsts


def host_weights(cfg: Cfg, W_in, gW0, gW1, gW2, b_in, gb, bng, bnb, a_s, a_d):
    bf = ml_dtypes.bfloat16
    wcat0 = np.concatenate([np.asarray(W_in, np.float32),
                            np.asarray(gW0, np.float32)], axis=1).astype(bf)
    pvec = np.zeros((21, P), np.float32)
    pvec[0:2] = np.asarray(b_in, np.float32).reshape(2, P)
    for l in range(3):
        pvec[2 + 6 * l:4 + 6 * l] = np.asarray(gb[l], np.float32).reshape(2, P)
        pvec[4 + 6 * l:6 + 6 * l] = np.asarray(bng[l], np.float32).reshape(2, P)
        pvec[6 + 6 * l:8 + 6 * l] = np.asarray(bnb[l], np.float32).reshape(2, P)
    pvec[20] = np.arange(P, dtype=np.float32)
    asd = np.zeros((6, cfg.HID), np.float32)
    for l in range(3):
        asd[2 * l] = np.asarray(a_s[l], np.float32).reshape(-1)
        asd[2 * l + 1] = np.asarray(a_d[l], np.float32).reshape(-1)
    return dict(wcat0=wcat0, w1=np.asarray(gW1, np.float32).astype(bf),
                w2=np.asarray(gW2, np.float32).astype(bf), pvec=pvec, asd=asd)


def host_finish(cfg: Cfg, outs, gfirsts, batch, mW1, mb1, mg1, mbeta1,
                mW2, mb2, mg2, mbeta2, hW, hb):
    c = cfg
    batch = np.asarray(batch, np.int64)
    cnt = np.bincount(batch, minlength=c.G).astype(np.float32)
    hsum = np.zeros((c.G, c.HID), np.float32)
    hmax = np.full((c.G, c.HID), -np.inf, np.float32)
    for cc in range(c.NCORES):
        g0 = gfirsts[cc]
        ng = min(c.GW, c.G - g0)
        rs = np.asarray(outs[cc]["rsum_o"], np.float32).reshape(
            2 * P, c.GW)[:c.HID, :ng].T
        rm = np.asarray(outs[cc]["rmax_o"], np.float32).reshape(
            2 * P, c.GW)[:c.HID, :ng].T
        hsum[g0:g0 + ng] += rs
        hmax[g0:g0 + ng] = np.maximum(hmax[g0:g0 + ng], rm)
    hmean = hsum / np.maximum(cnt, 1.0)[:, None]
    hmax = np.where((cnt[:, None] > 0) & (hmax > -1e29), hmax, 0.0)
    hg = np.concatenate([hmean, hmax], axis=1).astype(np.float32)

    def bn(h, g, b):
        mu = h.mean(0, dtype=np.float32)
        v = ((h - mu) ** 2).mean(0, dtype=np.float32)
        return (h - mu) / np.sqrt(v + EPS) * g + b

    s = np.maximum(bn(hg @ np.asarray(mW1, np.float32) + mb1, mg1, mbeta1), 0.0)
    s = np.maximum(bn(s @ np.asarray(mW2, np.float32) + mb2, mg2, mbeta2), 0.0)
    return (s @ np.asarray(hW, np.float32) + hb).astype(np.float32)


# ---------------------------------------------------------------------------
# persistent PJRT runner (compile once, reuse)
# ---------------------------------------------------------------------------

class Runner:
    def __init__(self, nc, n_cores):
        import jax
        from jax.sharding import Mesh, PartitionSpec
        from jax.experimental.shard_map import shard_map
        from concourse import bass2jax
        try:
            jax.config.update("jax_compilation_cache_dir", "/tmp/jax_pcc")
            jax.config.update("jax_persistent_cache_min_entry_size_bytes", -1)
            jax.config.update("jax_persistent_cache_min_compile_time_secs", 0)
        except Exception:
            pass
        bass2jax.install_neuronx_cc_hook()
        self.nc = nc
        self.n_cores = n_cores
        partition_name = (nc.partition_id_tensor.name
                          if getattr(nc, "partition_id_tensor", None) is not None
                          else None)
        in_names, out_names, out_avals, zero_shapes = [], [], [], []
        self.in_specs = {}
        for alloc in nc.m.functions[0].allocations:
            if not isinstance(alloc, mybir.MemoryLocationSet):
                continue
            name = alloc.memorylocations[0].name
            if alloc.kind == "ExternalInput":
                if name == partition_name:
                    continue
                in_names.append(name)
                self.in_specs[name] = (tuple(alloc.tensor_shape),
                                       mybir.dt.np(alloc.dtype))
            elif alloc.kind == "ExternalOutput":
                shape = tuple(alloc.tensor_shape)
                dtype = mybir.dt.np(alloc.dtype)
                out_names.append(name)
                out_avals.append(jax.core.ShapedArray(shape, dtype))
                zero_shapes.append((shape, dtype))
        self.in_names = in_names
        self.out_names = out_names
        self.out_avals = out_avals
        self.zero_shapes = zero_shapes
        n_params = len(in_names)
        all_names = list(in_names) + list(out_names)
        if partition_name is not None:
            all_names.append(partition_name)
        donate = tuple(range(n_params, n_params + len(out_names)))

        def _body(*args):
            operands = list(args)
            if partition_name is not None:
                operands.append(bass2jax.partition_id_tensor())
            outs = bass2jax._bass_exec_p.bind(
                *operands,
                out_avals=tuple(out_avals),
                in_names=tuple(all_names),
                out_names=tuple(out_names),
                lowering_input_output_aliases=(),
                sim_require_finite=False,
                sim_require_nnan=False,
                nc=nc,
            )
            return tuple(outs)

        devices = jax.devices()[:n_cores]
        mesh = Mesh(np.asarray(devices), ("core",))
        from jax.sharding import NamedSharding
        self._zero_sh = NamedSharding(mesh, PartitionSpec("core"))
        nin = n_params + len(out_names)
        self._fn = jax.jit(
            shard_map(_body, mesh=mesh,
                      in_specs=(PartitionSpec("core"),) * nin,
                      out_specs=(PartitionSpec("core"),) * len(out_names),
                      check_rep=False),
            donate_argnums=donate, keep_unused=True)

    def run(self, in_maps):
        concat = [np.concatenate([np.asarray(m[nm]) for m in in_maps], axis=0)
                  for nm in self.in_names]
        zeros = [np.zeros((self.n_cores * s[0], *s[1:]), d)
                 for s, d in self.zero_shapes]
        out_arrs = self._fn(*concat, *zeros)
        res = []
        for cc in range(self.n_cores):
            res.append({nm: np.asarray(out_arrs[i]).reshape(
                self.n_cores, *self.out_avals[i].shape)[cc]
                for i, nm in enumerate(self.out_names)})
        return res

    def warm(self):
        in_maps = []
        for cc in range(self.n_cores):
            m = {nm: np.zeros(sh, dt)
                 for nm, (sh, dt) in self.in_specs.items()}
            in_maps.append(m)
        self.run(in_maps)


_RUNNER = None


def _ensure_runner():
    global _RUNNER
    if _RUNNER is None:
        nc = build_nc(FULL)
        _RUNNER = Runner(nc, FULL.NCORES)
        _RUNNER.warm()
    return _RUNNER


# ---------------------------------------------------------------------------
# entry point
# ---------------------------------------------------------------------------

def _kernel_numpy(x, edge_index, batch, W_in, b_in, gW, gas, gad, gb, bng,
                  bnb, mW1, mb1, mg1, mbeta1, mW2, mb2, mg2, mbeta2, hW, hb):
    # pure-host fallback (slow) in case device capacity assumptions fail
    n = x.shape[0]
    G = FULL.G
    loop = np.arange(n)
    src = np.concatenate([edge_index[0], loop])
    dst = np.concatenate([edge_index[1], loop])
    order = np.argsort(dst, kind="stable")
    srcs, dsts = src[order], dst[order]
    counts = np.bincount(dsts, minlength=n)
    starts = np.zeros(n, np.int64)
    np.cumsum(counts[:-1], out=starts[1:])

    def bn(h, g, b):
        mu = h.mean(0)
        v = ((h - mu) ** 2).mean(0)
        return (h - mu) / np.sqrt(v + EPS) * g + b

    h_short = x @ W_in + b_in
    h = x
    for i in range(3):
        xw = (h @ gW[i]).reshape(n, HEADS, -1)
        ssum = np.einsum("nhc,hc->nh", xw, gas[i])
        dsum = np.einsum("nhc,hc->nh", xw, gad[i])
        e = ssum[srcs] + dsum[dsts]
        e = np.where(e > 0, e, SLOPE * e)
        m = np.maximum.reduceat(e, starts, axis=0)
        ex = np.exp(e - m[dsts])
        den = np.add.reduceat(ex, starts, axis=0)
        alpha = ex / den[dsts]
        out = np.add.reduceat(xw[srcs] * alpha[:, :, None], starts,
                              axis=0).reshape(n, -1) + gb[i]
        hn = bn(out, bng[i], bnb[i])
        hn = np.where(hn > 0, hn, np.expm1(np.minimum(hn, 0)))
        h = hn + (h_short if i == 0 else h)
    cnt = np.bincount(batch, minlength=G).astype(np.float32)
    hsum = np.zeros((G, h.shape[1]), np.float32)
    np.add.at(hsum, batch, h)
    hmax = np.full((G, h.shape[1]), -np.inf, np.float32)
    np.maximum.at(hmax, batch, h)
    hmax = np.where(cnt[:, None] > 0, hmax, 0.0)
    hg = np.concatenate([hsum / np.maximum(cnt, 1.0)[:, None], hmax], axis=1)
    s = np.maximum(bn(hg @ mW1 + mb1, mg1, mbeta1), 0.0)
    s = np.maximum(bn(s @ mW2 + mb2, mg2, mbeta2), 0.0)
    return (s @ hW + hb).astype(np.float32)


def kernel(x, edge_index, batch, W_in, b_in, gW0, gas0, gad0, gb0, bng0, bnb0,
           gW1, gas1, gad1, gb1, bng1, bnb1, gW2, gas2, gad2, gb2, bng2, bnb2,
           mW1, mb1, mg1, mbeta1, mW2, mb2, mg2, mbeta2, hW, hb):
    c = FULL
    x = np.asarray(x, np.float32)
    edge_index = np.asarray(edge_index)
    batch = np.asarray(batch)
    try:
        runner = _ensure_runner()
        per_core, gfirsts = host_prep(c, x, edge_index, batch)
        wmap = host_weights(c, W_in, gW0, gW1, gW2, b_in,
                            [gb0, gb1, gb2], [bng0, bng1, bng2],
                            [bnb0, bnb1, bnb2],
                            [gas0, gas1, gas2], [gad0, gad1, gad2])
        in_maps = [dict(pc, **wmap) for pc in per_core]
        outs = runner.run(in_maps)
        return host_finish(c, outs, gfirsts, batch, mW1, mb1, mg1, mbeta1,
                           mW2, mb2, mg2, mbeta2, hW, hb)
    except OverflowError:
        return _kernel_numpy(
            x, edge_index, batch,
            np.asarray(W_in, np.float32), np.asarray(b_in, np.float32),
            [np.asarray(w, np.float32) for w in (gW0, gW1, gW2)],
            [np.asarray(w, np.float32) for w in (gas0, gas1, gas2)],
            [np.asarray(w, np.float32) for w in (gad0, gad1, gad2)],
            [np.asarray(w, np.float32) for w in (gb0, gb1, gb2)],
            [np.asarray(w, np.float32) for w in (bng0, bng1, bng2)],
            [np.asarray(w, np.float32) for w in (bnb0, bnb1, bnb2)],
            np.asarray(mW1, np.float32), mb1, mg1, mbeta1,
            np.asarray(mW2, np.float32), mb2, mg2, mbeta2,
            np.asarray(hW, np.float32), hb)


if os.environ.get("BASS_GNN_LAZY", "") != "1":
    _ensure_runner()


# revision 35
# speedup vs baseline: 1.3224x; 1.0194x over previous
import os
import sys

for _p in ("/opt/trn_rl_repo", "/root/.axon_site/_ro/trn_rl_repo"):
    if _p not in sys.path:
        sys.path.insert(0, _p)

import numpy as np
import ml_dtypes

import concourse.bass as bass
import concourse.bacc as bacc
import concourse.mybir as mybir
import concourse.tile as tile
from concourse.masks import make_identity

F32 = mybir.dt.float32
BF16 = mybir.dt.bfloat16
I32 = mybir.dt.int32
I16 = mybir.dt.int16
ALU = mybir.AluOpType
ACT = mybir.ActivationFunctionType
AX = mybir.AxisListType

P = 128
EPS = 1e-5
SLOPE = 0.2
HEADS = 4


class Cfg:
    def __init__(self, ncores=8, nreal=40000, npc=5120, in_dim=64, hid=256,
                 eb=11, gw=224, ks=64, g=1500):
        self.NCORES = ncores
        self.N = nreal                      # real nodes total
        self.NR = nreal // ncores           # real nodes per core
        self.NP = npc                       # padded node slots per core
        self.NT = npc * ncores              # total table slots
        self.IN = in_dim
        self.HID = hid
        self.C = hid // HEADS
        self.EB = eb                        # edge blocks per dst-tile
        self.ET = eb * P                    # edge slot capacity per dst-tile
        self.NPT = npc // P                 # dst-tiles per core
        self.ECB = self.NPT * eb            # edge array free blocks
        self.GW = gw                        # graph window per core
        self.KS = ks                        # slots per graph
        self.RN = gw * ks                   # readout gather count
        self.G = g
        assert npc % P == 0 and self.RN % 512 == 0
        self.PAD_SLOT = self.NR             # global slot with zeroed table row


FULL = Cfg()


# ---------------------------------------------------------------------------
# device program
# ---------------------------------------------------------------------------

def build_nc(cfg: Cfg, dbg=False):
    c = cfg
    HID = c.HID
    nc = bacc.Bacc("TRN2", target_bir_lowering=False, debug=False,
                   num_devices=c.NCORES, num_swdge_queues=1)

    x_in = nc.dram_tensor("x_in", [c.NP, c.IN], BF16, kind="ExternalInput")
    wcat0 = nc.dram_tensor("wcat0", [c.IN, 2 * HID], BF16, kind="ExternalInput")
    w1 = nc.dram_tensor("w1", [HID, HID], BF16, kind="ExternalInput")
    w2 = nc.dram_tensor("w2", [HID, HID], BF16, kind="ExternalInput")
    pvec = nc.dram_tensor("pvec", [22, P], F32, kind="ExternalInput")
    asd = nc.dram_tensor("asd", [6, HID], F32, kind="ExternalInput")
    esrc = nc.dram_tensor("esrc", [P, c.ECB], mybir.dt.uint16, kind="ExternalInput")
    edst32 = nc.dram_tensor("edst32", [P, c.ECB], I16, kind="ExternalInput")
    dstf = nc.dram_tensor("dstf", [P, c.ECB], mybir.dt.int8, kind="ExternalInput")
    ridxs = nc.dram_tensor("ridxs", [16, c.RN // 16], I16, kind="ExternalInput")
    ridxm = nc.dram_tensor("ridxm", [16, c.RN // 16], I16, kind="ExternalInput")

    rsum_o = nc.dram_tensor("rsum_o", [2, P, c.GW], BF16, kind="ExternalOutput")
    rmax_o = nc.dram_tensor("rmax_o", [2, P, c.GW], BF16, kind="ExternalOutput")
    if dbg:
        d_h = [nc.dram_tensor(f"d_h{l}", [c.NP, c.HID], F32,
                              kind="ExternalOutput") for l in range(3)]
        d_o = nc.dram_tensor("d_o", [c.NP, c.HID], F32, kind="ExternalOutput")
        d_g1 = nc.dram_tensor("d_g1", [P, c.EB, c.HID + 16], F32, kind="ExternalOutput")
        d_g2 = nc.dram_tensor("d_g2", [P, c.EB, 8], F32, kind="ExternalOutput")
        d_ex = nc.dram_tensor("d_ex", [P, c.EB, 4], F32, kind="ExternalOutput")
        d_pay = nc.dram_tensor("d_pay", [P, c.EB, 260], F32, kind="ExternalOutput")
        d_pt = nc.dram_tensor("d_pt", [P, 260], F32, kind="ExternalOutput")

    RG = [list(range(c.NCORES))]

    with tile.TileContext(nc) as tc:
        with (
            tc.tile_pool(name="dram", bufs=1, space="DRAM") as dram,
            tc.tile_pool(name="const", bufs=1) as cpool,
            tc.tile_pool(name="persist", bufs=1) as pers,
        ):
            # ---------------- constants / loads ----------------
            ident = cpool.tile([P, P], BF16)
            make_identity(nc, ident[:])
            ones = cpool.tile([P, 1], F32)
            nc.gpsimd.memset(ones[:], 1.0)
            m39 = cpool.tile([P, 1], F32)
            nc.gpsimd.memset(m39[:], 0.0)
            npad = c.NR - (c.NPT - 1) * P   # real rows in the last tile
            nc.gpsimd.memset(m39[0:npad, :], 1.0)

            wc0_sb = cpool.tile([c.IN, 2 * HID], BF16)
            nc.sync.dma_start(wc0_sb[:], wcat0[:, :])
            w1_sb = cpool.tile([P, 2, HID], BF16)
            w2_sb = cpool.tile([P, 2, HID], BF16)
            for hf in range(2):
                nc.sync.dma_start(w1_sb[:, hf, :], w1[hf * P:(hf + 1) * P, :])
                nc.sync.dma_start(w2_sb[:, hf, :], w2[hf * P:(hf + 1) * P, :])

            def bcast256(pool, dst, src0, src1):
                scr = pool.tile([1, 2 * P], F32, tag="bscr", bufs=2)
                nc.sync.dma_start(scr[0:1, 0:P], src0)
                nc.sync.dma_start(scr[0:1, P:2 * P], src1)
                nc.gpsimd.partition_broadcast(dst[:, :], scr[0:1, :])

            as_bc, ad_bc, gb_bc = [], [], []
            for l in range(3):
                a_s = pers.tile([P, HID], F32, name=f"as_bc{l}")
                a_d = pers.tile([P, HID], F32, name=f"ad_bc{l}")
                bcast256(pers, a_s, asd[2 * l:2 * l + 1, 0:P],
                         asd[2 * l:2 * l + 1, P:2 * P])
                bcast256(pers, a_d, asd[2 * l + 1:2 * l + 2, 0:P],
                         asd[2 * l + 1:2 * l + 2, P:2 * P])
                as_bc.append(a_s)
                ad_bc.append(a_d)
                g_b = pers.tile([P, HID], F32, name=f"gb_bc{l}")
                bcast256(pers, g_b, pvec[2 + 6 * l:3 + 6 * l, :],
                         pvec[3 + 6 * l:4 + 6 * l, :])
                gb_bc.append(g_b)
            bin_bc = pers.tile([P, HID], F32)
            bcast256(pers, bin_bc, pvec[0:1, :], pvec[1:2, :])
            iota_bc = pers.tile([P, P], F32)
            scr0 = pers.tile([1, P], F32, name="scr0")
            nc.sync.dma_start(scr0[0:1, :], pvec[20:21, :])
            nc.gpsimd.partition_broadcast(iota_bc[:, :], scr0[0:1, :])

            esrc_16 = pers.tile([P, c.ECB], mybir.dt.uint16, name="esrc16")
            nc.sync.dma_start(esrc_16[:], esrc[:, :])
            esrc_sb = pers.tile([P, c.ECB], I32)
            nc.vector.tensor_copy(esrc_sb[:], esrc_16[:])
            edst_16 = pers.tile([P, c.ECB], I16, name="edst16t")
            nc.sync.dma_start(edst_16[:], edst32[:, :])
            edst32_sb = pers.tile([P, c.ECB], I32)
            nc.vector.tensor_copy(edst32_sb[:], edst_16[:])
            dstf_8 = pers.tile([P, c.ECB], mybir.dt.int8, name="dstf8")
            nc.sync.dma_start(dstf_8[:], dstf[:, :])
            dstf_sb = pers.tile([P, c.ECB], F32)
            nc.vector.tensor_copy(dstf_sb[:], dstf_8[:])
            ridxs_sb = pers.tile([P, c.RN // 16], I16)
            ridxm_sb = pers.tile([P, c.RN // 16], I16)
            for k in range(8):
                nc.sync.dma_start(ridxs_sb[16 * k:16 * (k + 1), :], ridxs[:, :])
                nc.sync.dma_start(ridxm_sb[16 * k:16 * (k + 1), :], ridxm[:, :])

            x_bf = pers.tile([P, c.NPT, c.IN], BF16)
            for t in range(c.NPT):
                nc.sync.dma_start(x_bf[:, t, :], x_in[t * P:(t + 1) * P, :])

            h_bf = pers.tile([P, c.NPT, HID], BF16)
            o_sb = pers.tile([P, c.NPT, HID], F32)

            hshort = dram.tile([c.NP, HID], F32)
            h_cur = [dram.tile([c.NP, HID], F32, name=f"h{l}") for l in range(3)]

            # ---------------- layers ----------------
            for l in range(3):
                tbl_own = dram.tile([c.NP, HID + 16], BF16, name=f"tblo{l}")
                tblS_own = dram.tile([c.NP, 8], F32, name=f"tblso{l}")
                tbl = dram.tile([c.NT, HID + 16], BF16, name=f"tbl{l}",
                                addr_space="Shared")
                stats_own = dram.tile([4, P], F32, name=f"sto{l}")
                stats_all = dram.tile([4, P], F32, name=f"sta{l}",
                                      addr_space="Shared")

                with (
                    tc.tile_pool(name=f"tl{l}", bufs=2) as tl,
                    tc.tile_pool(name=f"tps{l}", bufs=2, space="PSUM") as tps,
                    tc.tile_pool(name=f"sps{l}", bufs=1, space="PSUM") as sps,
                    tc.tile_pool(name=f"ep{l}", bufs=2) as ep,
                    tc.tile_pool(name=f"sm{l}", bufs=1) as sm,
                ):
                    # ---- table build (own slice) ----
                    for t in range(c.NPT):
                        if l == 0:
                            xt = x_bf[:, t, :]
                            tp = tps.tile([P, P], BF16, tag="tp")
                            hT = tl.tile([P, 2, P], BF16, tag="hT")
                            nc.tensor.transpose(tp[0:c.IN, :], xt, ident[:])
                            nc.vector.tensor_copy(hT[0:c.IN, 0, :], tp[0:c.IN, :])
                            xw_ps = tps.tile([P, 2 * HID], F32, tag="xwps")
                            nc.tensor.matmul(xw_ps[:, :], hT[0:c.IN, 0, :],
                                             wc0_sb[:, :], start=True, stop=True)
                            hs = tl.tile([P, HID], F32, tag="hs")
                            nc.vector.tensor_add(hs[:], xw_ps[:, 0:HID], bin_bc[:])
                            nc.sync.dma_start(hshort[t * P:(t + 1) * P, :], hs[:])
                            xw = xw_ps[:, HID:2 * HID]
                        else:
                            ht = h_bf[:, t, :]
                            wsb = w1_sb if l == 1 else w2_sb
                            xw_ps = tps.tile([P, 2 * HID], F32, tag="xwps")
                            hT = tl.tile([P, 2, P], BF16, tag="hT")
                            for hf in range(2):
                                tp = tps.tile([P, P], BF16, tag="tp")
                                nc.tensor.transpose(
                                    tp[:, :], ht[:, hf * P:(hf + 1) * P], ident[:])
                                nc.vector.tensor_copy(hT[:, hf, :], tp[:, :])
                            for hf in range(2):
                                nc.tensor.matmul(xw_ps[:, 0:HID], hT[:, hf, :],
                                                 wsb[:, hf, :], start=(hf == 0),
                                                 stop=(hf == 1))
                            xw = xw_ps[:, 0:HID]

                        xwb = tl.tile([P, HID], BF16, tag="xwb")
                        if t == c.NPT - 1:
                            nc.vector.tensor_scalar_mul(xwb[:], xw, m39[:, 0:1])
                        else:
                            nc.vector.tensor_copy(xwb[:], xw)
                        nc.sync.dma_start(tbl_own[t * P:(t + 1) * P, 0:HID],
                                          xwb[:])

                        sd = tl.tile([P, 8], F32, tag="sd")
                        sc = tl.tile([P, c.C], F32, tag="sc")
                        for hd in range(4):
                            sl = slice(hd * c.C, (hd + 1) * c.C)
                            nc.vector.tensor_tensor(
                                out=sc[:], in0=xw[:, sl], in1=as_bc[l][:, sl],
                                op=ALU.mult)
                            nc.vector.tensor_reduce(
                                sd[:, hd:hd + 1], sc[:], axis=AX.X, op=ALU.add)
                            nc.vector.tensor_tensor(
                                out=sc[:], in0=xw[:, sl], in1=ad_bc[l][:, sl],
                                op=ALU.mult)
                            nc.vector.tensor_reduce(
                                sd[:, 4 + hd:5 + hd], sc[:], axis=AX.X,
                                op=ALU.add)
                        nc.sync.dma_start(tblS_own[t * P:(t + 1) * P, :], sd[:])
                        nc.sync.dma_start(
                            tbl_own[t * P:(t + 1) * P, HID:HID + 16],
                            sd[:].bitcast(BF16))

                    nc.gpsimd.collective_compute(
                        "AllGather", ALU.bypass, replica_groups=RG,
                        ins=[tbl_own[:, :].opt()], outs=[tbl[:, :].opt()])

                    # ---- edge phase: gather / attention / segment matmul ----
                    s_o = sm.tile([P, HID], F32, name=f"s_o{l}")
                    s_q = sm.tile([P, HID], F32, name=f"s_q{l}")
                    nc.vector.memset(s_o[:], 0.0)
                    nc.vector.memset(s_q[:], 0.0)
                    for t in range(c.NPT):
                        g1 = ep.tile([P, c.EB, HID + 16], BF16, tag="g1")
                        g2 = ep.tile([P, c.EB, 8], F32, tag="g2")
                        for b in range(c.EB):
                            col = t * c.EB + b
                            nc.gpsimd.indirect_dma_start(
                                out=g1[:, b, :], out_offset=None, in_=tbl[:, :],
                                in_offset=bass.IndirectOffsetOnAxis(
                                    ap=esrc_sb[:, col:col + 1], axis=0))
                            nc.gpsimd.indirect_dma_start(
                                out=g2[:, b, :], out_offset=None,
                                in_=tblS_own[:, :],
                                in_offset=bass.IndirectOffsetOnAxis(
                                    ap=edst32_sb[:, col:col + 1], axis=0))

                        g1sd = g1[:, :, HID:HID + 16].bitcast(F32)
                        ee = ep.tile([P, c.EB, 4], F32, tag="ee")
                        nc.vector.tensor_add(ee[:], g1sd[:, :, 0:4],
                                             g2[:, :, 4:8])
                        e2 = ep.tile([P, c.EB, 4], F32, tag="e2")
                        nc.scalar.mul(e2[:], ee[:], SLOPE)
                        nc.vector.tensor_tensor(out=ee[:], in0=ee[:], in1=e2[:],
                                                op=ALU.max)
                        ex = ep.tile([P, c.EB, 4], F32, tag="ex")
                        nc.scalar.activation(ex[:], ee[:], ACT.Exp)

                        pay = ep.tile([P, c.EB, 260], BF16, tag="pay")
                        for hd in range(4):
                            sl = slice(hd * c.C, (hd + 1) * c.C)
                            nc.vector.tensor_tensor(
                                out=pay[:, :, sl], in0=g1[:, :, sl],
                                in1=ex[:, :, hd:hd + 1].to_broadcast(
                                    [P, c.EB, c.C]),
                                op=ALU.mult)
                        nc.vector.tensor_copy(pay[:, :, 256:260], ex[:])

                        pt = tps.tile([P, 260], F32, tag="pt")
                        for b in range(c.EB):
                            S = ep.tile([P, P], BF16, tag="S")
                            nc.vector.tensor_scalar(
                                out=S[:], in0=iota_bc[:],
                                scalar1=dstf_sb[:, t * c.EB + b:
                                                t * c.EB + b + 1],
                                scalar2=None, op0=ALU.is_equal)
                            nc.tensor.matmul(pt[:, :], S[:], pay[:, b, :],
                                             start=(b == 0),
                                             stop=(b == c.EB - 1))

                        if dbg and l == 0 and t == 0:
                            dcp = tl.tile([P, c.EB, c.HID + 16], F32, tag="dcp")
                            nc.vector.tensor_copy(dcp[:], g1[:])
                            nc.sync.dma_start(d_g1[:, :, :], dcp[:])
                            nc.sync.dma_start(d_g2[:, :, :], g2[:])
                            nc.sync.dma_start(d_ex[:, :, :], ex[:])
                            dcq = tl.tile([P, c.EB, 260], F32, tag="dcq")
                            nc.vector.tensor_copy(dcq[:], pay[:])
                            nc.sync.dma_start(d_pay[:, :, :], dcq[:])
                            dcr = tl.tile([P, 260], F32, tag="dcr")
                            nc.vector.tensor_copy(dcr[:], pt[:])
                            nc.sync.dma_start(d_pt[:, :], dcr[:])

                        # ---- finalize tile: alpha-div, bias, stats ----
                        den = tl.tile([P, 4], F32, tag="den")
                        nc.vector.tensor_scalar_max(den[:], pt[:, 256:260],
                                                    1e-20)
                        rec = tl.tile([P, 4], F32, tag="rec")
                        nc.vector.reciprocal(rec[:], den[:])
                        ot = o_sb[:, t, :]
                        for hd in range(4):
                            sl = slice(hd * c.C, (hd + 1) * c.C)
                            nc.vector.tensor_scalar_mul(ot[:, sl], pt[:, sl],
                                                        rec[:, hd:hd + 1])
                        nc.vector.tensor_add(ot, ot, gb_bc[l][:])
                        if t == c.NPT - 1:
                            om = tl.tile([P, HID], F32, tag="om")
                            nc.vector.tensor_scalar_mul(om[:], ot, m39[:, 0:1])
                            stat_in = om[:]
                        else:
                            stat_in = ot
                        sq = tl.tile([P, HID], F32, tag="sq")
                        nc.vector.tensor_tensor(out=sq[:], in0=stat_in,
                                                in1=stat_in, op=ALU.mult)
                        nc.vector.tensor_add(s_o[:], s_o[:], stat_in)
                        nc.vector.tensor_add(s_q[:], s_q[:], sq[:])

                    if dbg and l == 0:
                        for t in range(c.NPT):
                            nc.sync.dma_start(d_o[t * P:(t + 1) * P, :],
                                              o_sb[:, t, :])

                    # ---- BN stats reduce + collective ----
                    st_ps = sps.tile([P, 4], F32, name=f"stp{l}")
                    for hf in range(2):
                        nc.tensor.matmul(
                            st_ps[:, hf:hf + 1], s_o[:, hf * P:(hf + 1) * P],
                            ones[:], start=True, stop=True,
                            skip_group_check=True)
                        nc.tensor.matmul(
                            st_ps[:, 2 + hf:3 + hf], s_q[:, hf * P:(hf + 1) * P],
                            ones[:], start=True, stop=True,
                            skip_group_check=True)
                    stq = sm.tile([P, 4], F32, name=f"stq{l}")
                    nc.vector.tensor_copy(stq[:], st_ps[:])
                    nc.sync.dma_start(stats_own[:, :].transpose([1, 0]), stq[:])
                    nc.gpsimd.collective_compute(
                        "AllReduce", ALU.add, replica_groups=RG,
                        ins=[stats_own[:, :].opt()], outs=[stats_all[:, :].opt()])

                    st_s = sm.tile([2, P], F32, name=f"sts{l}")
                    nc.sync.dma_start(st_s[:], stats_all[0:2, :])
                    st_q = sm.tile([2, P], F32, name=f"stq2{l}")
                    nc.sync.dma_start(st_q[:], stats_all[2:4, :])
                    g2t = sm.tile([2, P], F32, name=f"g2t{l}")
                    nc.sync.dma_start(g2t[:], pvec[4 + 6 * l:6 + 6 * l, :])
                    be2t = sm.tile([2, P], F32, name=f"be2t{l}")
                    nc.sync.dma_start(be2t[:], pvec[6 + 6 * l:8 + 6 * l, :])
                    mu = sm.tile([2, P], F32, name=f"mu{l}")
                    nc.scalar.mul(mu[:], st_s[:], 1.0 / c.N)
                    va = sm.tile([2, P], F32, name=f"va{l}")
                    nc.scalar.mul(va[:], st_q[:], 1.0 / c.N)
                    mu2 = sm.tile([2, P], F32, name=f"mu2{l}")
                    nc.vector.tensor_tensor(out=mu2[:], in0=mu[:], in1=mu[:],
                                            op=ALU.mult)
                    nc.vector.tensor_tensor(out=va[:], in0=va[:], in1=mu2[:],
                                            op=ALU.subtract)
                    nc.vector.tensor_scalar_add(va[:], va[:], EPS)
                    sdv = sm.tile([2, P], F32, name=f"sdv{l}")
                    nc.scalar.sqrt(sdv[:], va[:])
                    rs = sm.tile([2, P], F32, name=f"rs{l}")
                    nc.vector.reciprocal(rs[:], sdv[:])
                    A2 = sm.tile([2, P], F32, name=f"A2{l}")
                    nc.vector.tensor_tensor(out=A2[:], in0=rs[:], in1=g2t[:],
                                            op=ALU.mult)
                    B2 = sm.tile([2, P], F32, name=f"B2{l}")
                    nc.vector.tensor_tensor(out=B2[:], in0=mu[:], in1=A2[:],
                                            op=ALU.mult)
                    nc.vector.tensor_tensor(out=B2[:], in0=be2t[:],
                                            in1=B2[:], op=ALU.subtract)
                    A_bc = sm.tile([P, HID], F32, name=f"Abc{l}")
                    B_bc = sm.tile([P, HID], F32, name=f"Bbc{l}")
                    bcast256(sm, A_bc, A2[0:1, :], A2[1:2, :])
                    bcast256(sm, B_bc, B2[0:1, :], B2[1:2, :])

                    # ---- apply pass ----
                    for t in range(c.NPT):
                        u1 = tl.tile([P, HID], F32, tag="u1")
                        nc.vector.tensor_tensor(out=u1[:], in0=o_sb[:, t, :],
                                                in1=A_bc[:], op=ALU.mult)
                        nc.vector.tensor_add(u1[:], u1[:], B_bc[:])
                        u2 = tl.tile([P, HID], F32, tag="u2")
                        nc.vector.tensor_scalar_min(u2[:], u1[:], 0.0)
                        nc.scalar.activation(u2[:], u2[:], ACT.Exp)
                        rl = tl.tile([P, HID], F32, tag="rl")
                        nc.scalar.activation(rl[:], u1[:], ACT.Relu)
                        nc.vector.tensor_add(u2[:], u2[:], rl[:])
                        nc.vector.tensor_scalar_add(u2[:], u2[:], -1.0)
                        hp = tl.tile([P, HID], F32, tag="hp")
                        src = hshort if l == 0 else h_cur[l - 1]
                        nc.sync.dma_start(hp[:], src[t * P:(t + 1) * P, :])
                        nc.vector.tensor_add(u2[:], u2[:], hp[:])
                        nc.sync.dma_start(h_cur[l][t * P:(t + 1) * P, :], u2[:])
                        if dbg:
                            nc.sync.dma_start(d_h[l][t * P:(t + 1) * P, :],
                                              u2[:])
                        nc.vector.tensor_copy(h_bf[:, t, :], u2[:])

            # ---------------- readout ----------------
            h3tbl = dram.tile([c.NP + P, HID], BF16)
            for t in range(c.NPT):
                nc.sync.dma_start(h3tbl[t * P:(t + 1) * P, :], h_bf[:, t, :])
            with (
                tc.tile_pool(name="rd", bufs=2) as rd,
                tc.tile_pool(name="rs1", bufs=1) as rs1,
            ):
                sent0 = rs1.tile([1, HID], BF16, name="sent0")
                nc.gpsimd.memset(sent0[:], 0.0)
                sent1 = rs1.tile([1, HID], BF16, name="sent1")
                nc.gpsimd.memset(sent1[:], -1e30)
                nc.sync.dma_start(h3tbl[c.NP:c.NP + 1, :], sent0[:])
                nc.sync.dma_start(h3tbl[c.NP + 1:c.NP + 2, :], sent1[:])

                rsum_sb = rs1.tile([P, 2, c.GW], F32, name="rsum_sb")
                rmax_sb = rs1.tile([P, 2, c.GW], F32, name="rmax_sb")
                GSZ = 512
                gpg = GSZ // c.KS
                for j in range(c.RN // GSZ):
                    isl = slice(j * (GSZ // 16), (j + 1) * (GSZ // 16))
                    gsl = slice(j * gpg, (j + 1) * gpg)
                    gr = rd.tile([P, 2, GSZ], BF16, tag="gr")
                    nc.gpsimd.dma_gather(
                        out_ap=gr[:], in_ap=h3tbl[:, :],
                        idxs_ap=ridxs_sb[:, isl],
                        num_idxs=GSZ, num_idxs_reg=GSZ, elem_size=HID,
                        transpose=True)
                    nc.vector.tensor_reduce(
                        rsum_sb[:, :, gsl],
                        gr[:].rearrange("p b (g k) -> p b g k", k=c.KS),
                        axis=AX.X, op=ALU.add)
                    gm = rd.tile([P, 2, GSZ], BF16, tag="gm")
                    nc.gpsimd.dma_gather(
                        out_ap=gm[:], in_ap=h3tbl[:, :],
                        idxs_ap=ridxm_sb[:, isl],
                        num_idxs=GSZ, num_idxs_reg=GSZ, elem_size=HID,
                        transpose=True)
                    nc.vector.tensor_reduce(
                        rmax_sb[:, :, gsl],
                        gm[:].rearrange("p b (g k) -> p b g k", k=c.KS),
                        axis=AX.X, op=ALU.max)

                rsum_bf = rs1.tile([P, 2, c.GW], BF16, name="rsum_bf")
                nc.vector.tensor_copy(rsum_bf[:], rsum_sb[:])
                rmax_bf = rs1.tile([P, 2, c.GW], BF16, name="rmax_bf")
                nc.vector.tensor_copy(rmax_bf[:], rmax_sb[:])
                nc.sync.dma_start(rsum_o[:, :, :].transpose([1, 0, 2]),
                                  rsum_bf[:])
                nc.sync.dma_start(rmax_o[:, :, :].transpose([1, 0, 2]),
                                  rmax_bf[:])

    nc.compile()
    return nc


# ---------------------------------------------------------------------------
# host-side prep
# ---------------------------------------------------------------------------

def host_prep(cfg: Cfg, x, edge_index, batch):
    c = cfg
    bf = ml_dtypes.bfloat16
    n = c.N
    loop = np.arange(n, dtype=np.int64)
    src = np.concatenate([np.asarray(edge_index[0], np.int64), loop])
    dst = np.concatenate([np.asarray(edge_index[1], np.int64), loop])
    src_slot = ((src // c.NR) * c.NP + src % c.NR).astype(np.int64)
    dst_core = dst // c.NR
    dst_local = (dst % c.NR).astype(np.int64)
    batch = np.asarray(batch, np.int64)

    xscale = float(np.abs(x).max()) / 127.0 + 1e-30
    per_core = []
    gfirsts = []
    for cc in range(c.NCORES):
        m = dst_core == cc
        es = src_slot[m]
        ed = dst_local[m]
        order = np.argsort(ed, kind="stable")
        es, ed = es[order], ed[order]
        tile_id = ed // P
        counts = np.bincount(tile_id, minlength=c.NPT)
        if counts.max() > c.ET:
            raise OverflowError("edge tile capacity exceeded")
        starts = np.zeros(c.NPT, np.int64)
        np.cumsum(counts[:-1], out=starts[1:])
        pos_in_tile = np.arange(len(ed)) - starts[tile_id]
        es_f = np.full(c.NPT * c.ET, c.PAD_SLOT, np.int64)
        ed_f = np.zeros(c.NPT * c.ET, np.int64)
        df_f = np.full(c.NPT * c.ET, -1, np.int64)
        slot = tile_id * c.ET + pos_in_tile
        es_f[slot] = es
        ed_f[slot] = ed
        df_f[slot] = ed % P
        # slot s of tile t -> (p = s % 128, col = t*EB + s//128)
        es2d = np.ascontiguousarray(
            es_f.reshape(c.NPT, c.EB, P).transpose(2, 0, 1).reshape(
                P, c.ECB)).astype(np.uint16)
        ed2d = np.ascontiguousarray(
            ed_f.reshape(c.NPT, c.EB, P).transpose(2, 0, 1).reshape(
                P, c.ECB)).astype(np.int16)
        df2d = np.ascontiguousarray(
            df_f.reshape(c.NPT, c.EB, P).transpose(2, 0, 1).reshape(
                P, c.ECB)).astype(np.int8)

        bsl = batch[cc * c.NR:(cc + 1) * c.NR]
        gfirst = int(bsl[0])
        gfirsts.append(gfirst)
        w = (bsl - gfirst).astype(np.int64)
        uniq, first_idx = np.unique(w, return_index=True)
        fi = np.zeros(int(w[-1]) + 1, np.int64)
        fi[uniq] = first_idx
        kwi = np.arange(c.NR) - fi[w]
        if int(w[-1]) >= c.GW or int(kwi.max()) >= c.KS:
            raise OverflowError("readout window exceeded")
        sidx_s = np.full(c.RN, c.NP, np.int64)
        sidx_m = np.full(c.RN, c.NP + 1, np.int64)
        pos = w * c.KS + kwi
        sidx_s[pos] = np.arange(c.NR)
        sidx_m[pos] = np.arange(c.NR)
        ridxs_a = sidx_s.reshape(c.RN // 16, 16).T.astype(np.int16)
        ridxm_a = sidx_m.reshape(c.RN // 16, 16).T.astype(np.int16)

        xp = np.zeros((c.NP, c.IN), np.float32)
        xp[:c.NR] = x[cc * c.NR:(cc + 1) * c.NR]
        per_core.append(dict(
            x_in=xp.astype(bf), esrc=es2d, edst32=ed2d, dstf=df2d,
            ridxs=ridxs_a, ridxm=ridxm_a))
    return per_core, gfirsts, xscale


def host_weights(cfg: Cfg, W_in, gW0, gW1, gW2, b_in, gb, bng, bnb, a_s, a_d,
                 xscale=1.0):
    bf = ml_dtypes.bfloat16
    wcat0 = np.concatenate([np.asarray(W_in, np.float32),
                            np.asarray(gW0, np.float32)], axis=1).astype(bf)
    pvec = np.zeros((22, P), np.float32)
    pvec[21, 0] = xscale
    pvec[0:2] = np.asarray(b_in, np.float32).reshape(2, P)
    for l in range(3):
        pvec[2 + 6 * l:4 + 6 * l] = np.asarray(gb[l], np.float32).reshape(2, P)
        pvec[4 + 6 * l:6 + 6 * l] = np.asarray(bng[l], np.float32).reshape(2, P)
        pvec[6 + 6 * l:8 + 6 * l] = np.asarray(bnb[l], np.float32).reshape(2, P)
    pvec[20] = np.arange(P, dtype=np.float32)
    asd = np.zeros((6, cfg.HID), np.float32)
    for l in range(3):
        asd[2 * l] = np.asarray(a_s[l], np.float32).reshape(-1)
        asd[2 * l + 1] = np.asarray(a_d[l], np.float32).reshape(-1)
    return dict(wcat0=wcat0, w1=np.asarray(gW1, np.float32).astype(bf),
                w2=np.asarray(gW2, np.float32).astype(bf), pvec=pvec, asd=asd)


def host_finish(cfg: Cfg, outs, gfirsts, batch, mW1, mb1, mg1, mbeta1,
                mW2, mb2, mg2, mbeta2, hW, hb):
    c = cfg
    batch = np.asarray(batch, np.int64)
    cnt = np.bincount(batch, minlength=c.G).astype(np.float32)
    hsum = np.zeros((c.G, c.HID), np.float32)
    hmax = np.full((c.G, c.HID), -np.inf, np.float32)
    for cc in range(c.NCORES):
        g0 = gfirsts[cc]
        ng = min(c.GW, c.G - g0)
        rs = np.asarray(outs[cc]["rsum_o"], np.float32).reshape(
            2 * P, c.GW)[:c.HID, :ng].T
        rm = np.asarray(outs[cc]["rmax_o"], np.float32).reshape(
            2 * P, c.GW)[:c.HID, :ng].T
        hsum[g0:g0 + ng] += rs
        hmax[g0:g0 + ng] = np.maximum(hmax[g0:g0 + ng], rm)
    hmean = hsum / np.maximum(cnt, 1.0)[:, None]
    hmax = np.where((cnt[:, None] > 0) & (hmax > -1e29), hmax, 0.0)
    hg = np.concatenate([hmean, hmax], axis=1).astype(np.float32)

    def bn(h, g, b):
        mu = h.mean(0, dtype=np.float32)
        v = ((h - mu) ** 2).mean(0, dtype=np.float32)
        return (h - mu) / np.sqrt(v + EPS) * g + b

    s = np.maximum(bn(hg @ np.asarray(mW1, np.float32) + mb1, mg1, mbeta1), 0.0)
    s = np.maximum(bn(s @ np.asarray(mW2, np.float32) + mb2, mg2, mbeta2), 0.0)
    return (s @ np.asarray(hW, np.float32) + hb).astype(np.float32)


# ---------------------------------------------------------------------------
# persistent PJRT runner (compile once, reuse)
# ---------------------------------------------------------------------------

class Runner:
    def __init__(self, nc, n_cores):
        import jax
        from jax.sharding import Mesh, PartitionSpec
        from jax.experimental.shard_map import shard_map
        from concourse import bass2jax
        try:
            jax.config.update("jax_compilation_cache_dir", "/tmp/jax_pcc")
            jax.config.update("jax_persistent_cache_min_entry_size_bytes", -1)
            jax.config.update("jax_persistent_cache_min_compile_time_secs", 0)
        except Exception:
            pass
        bass2jax.install_neuronx_cc_hook()
        self.nc = nc
        self.n_cores = n_cores
        partition_name = (nc.partition_id_tensor.name
                          if getattr(nc, "partition_id_tensor", None) is not None
                          else None)
        in_names, out_names, out_avals, zero_shapes = [], [], [], []
        self.in_specs = {}
        for alloc in nc.m.functions[0].allocations:
            if not isinstance(alloc, mybir.MemoryLocationSet):
                continue
            name = alloc.memorylocations[0].name
            if alloc.kind == "ExternalInput":
                if name == partition_name:
                    continue
                in_names.append(name)
                self.in_specs[name] = (tuple(alloc.tensor_shape),
                                       mybir.dt.np(alloc.dtype))
            elif alloc.kind == "ExternalOutput":
                shape = tuple(alloc.tensor_shape)
                dtype = mybir.dt.np(alloc.dtype)
                out_names.append(name)
                out_avals.append(jax.core.ShapedArray(shape, dtype))
                zero_shapes.append((shape, dtype))
        self.in_names = in_names
        self.out_names = out_names
        self.out_avals = out_avals
        self.zero_shapes = zero_shapes
        n_params = len(in_names)
        all_names = list(in_names) + list(out_names)
        if partition_name is not None:
            all_names.append(partition_name)
        donate = tuple(range(n_params, n_params + len(out_names)))

        def _body(*args):
            operands = list(args)
            if partition_name is not None:
                operands.append(bass2jax.partition_id_tensor())
            outs = bass2jax._bass_exec_p.bind(
                *operands,
                out_avals=tuple(out_avals),
                in_names=tuple(all_names),
                out_names=tuple(out_names),
                lowering_input_output_aliases=(),
                sim_require_finite=False,
                sim_require_nnan=False,
                nc=nc,
            )
            return tuple(outs)

        devices = jax.devices()[:n_cores]
        mesh = Mesh(np.asarray(devices), ("core",))
        from jax.sharding import NamedSharding
        self._zero_sh = NamedSharding(mesh, PartitionSpec("core"))
        nin = n_params + len(out_names)
        self._fn = jax.jit(
            shard_map(_body, mesh=mesh,
                      in_specs=(PartitionSpec("core"),) * nin,
                      out_specs=(PartitionSpec("core"),) * len(out_names),
                      check_rep=False),
            donate_argnums=donate, keep_unused=True)

    def run(self, in_maps):
        concat = [np.concatenate([np.asarray(m[nm]) for m in in_maps], axis=0)
                  for nm in self.in_names]
        zeros = [np.zeros((self.n_cores * s[0], *s[1:]), d)
                 for s, d in self.zero_shapes]
        out_arrs = self._fn(*concat, *zeros)
        res = []
        for cc in range(self.n_cores):
            res.append({nm: np.asarray(out_arrs[i]).reshape(
                self.n_cores, *self.out_avals[i].shape)[cc]
                for i, nm in enumerate(self.out_names)})
        return res

    def warm(self):
        in_maps = []
        for cc in range(self.n_cores):
            m = {nm: np.zeros(sh, dt)
                 for nm, (sh, dt) in self.in_specs.items()}
            in_maps.append(m)
        self.run(in_maps)


_RUNNER = None


def _ensure_runner():
    global _RUNNER
    if _RUNNER is None:
        nc = build_nc(FULL)
        _RUNNER = Runner(nc, FULL.NCORES)
        _RUNNER.warm()
    return _RUNNER


# ---------------------------------------------------------------------------
# entry point
# ---------------------------------------------------------------------------

def _kernel_numpy(x, edge_index, batch, W_in, b_in, gW, gas, gad, gb, bng,
                  bnb, mW1, mb1, mg1, mbeta1, mW2, mb2, mg2, mbeta2, hW, hb):
    # pure-host fallback (slow) in case device capacity assumptions fail
    n = x.shape[0]
    G = FULL.G
    loop = np.arange(n)
    src = np.concatenate([edge_index[0], loop])
    dst = np.concatenate([edge_index[1], loop])
    order = np.argsort(dst, kind="stable")
    srcs, dsts = src[order], dst[order]
    counts = np.bincount(dsts, minlength=n)
    starts = np.zeros(n, np.int64)
    np.cumsum(counts[:-1], out=starts[1:])

    def bn(h, g, b):
        mu = h.mean(0)
        v = ((h - mu) ** 2).mean(0)
        return (h - mu) / np.sqrt(v + EPS) * g + b

    h_short = x @ W_in + b_in
    h = x
    for i in range(3):
        xw = (h @ gW[i]).reshape(n, HEADS, -1)
        ssum = np.einsum("nhc,hc->nh", xw, gas[i])
        dsum = np.einsum("nhc,hc->nh", xw, gad[i])
        e = ssum[srcs] + dsum[dsts]
        e = np.where(e > 0, e, SLOPE * e)
        m = np.maximum.reduceat(e, starts, axis=0)
        ex = np.exp(e - m[dsts])
        den = np.add.reduceat(ex, starts, axis=0)
        alpha = ex / den[dsts]
        out = np.add.reduceat(xw[srcs] * alpha[:, :, None], starts,
                              axis=0).reshape(n, -1) + gb[i]
        hn = bn(out, bng[i], bnb[i])
        hn = np.where(hn > 0, hn, np.expm1(np.minimum(hn, 0)))
        h = hn + (h_short if i == 0 else h)
    cnt = np.bincount(batch, minlength=G).astype(np.float32)
    hsum = np.zeros((G, h.shape[1]), np.float32)
    np.add.at(hsum, batch, h)
    hmax = np.full((G, h.shape[1]), -np.inf, np.float32)
    np.maximum.at(hmax, batch, h)
    hmax = np.where(cnt[:, None] > 0, hmax, 0.0)
    hg = np.concatenate([hsum / np.maximum(cnt, 1.0)[:, None], hmax], axis=1)
    s = np.maximum(bn(hg @ mW1 + mb1, mg1, mbeta1), 0.0)
    s = np.maximum(bn(s @ mW2 + mb2, mg2, mbeta2), 0.0)
    return (s @ hW + hb).astype(np.float32)


def kernel(x, edge_index, batch, W_in, b_in, gW0, gas0, gad0, gb0, bng0, bnb0,
           gW1, gas1, gad1, gb1, bng1, bnb1, gW2, gas2, gad2, gb2, bng2, bnb2,
           mW1, mb1, mg1, mbeta1, mW2, mb2, mg2, mbeta2, hW, hb):
    c = FULL
    x = np.asarray(x, np.float32)
    edge_index = np.asarray(edge_index)
    batch = np.asarray(batch)
    try:
        runner = _ensure_runner()
        per_core, gfirsts, xscale = host_prep(c, x, edge_index, batch)
        wmap = host_weights(c, W_in, gW0, gW1, gW2, b_in,
                            [gb0, gb1, gb2], [bng0, bng1, bng2],
                            [bnb0, bnb1, bnb2],
                            [gas0, gas1, gas2], [gad0, gad1, gad2],
                            xscale=xscale)
        in_maps = [dict(pc, **wmap) for pc in per_core]
        outs = runner.run(in_maps)
        return host_finish(c, outs, gfirsts, batch, mW1, mb1, mg1, mbeta1,
                           mW2, mb2, mg2, mbeta2, hW, hb)
    except OverflowError:
        return _kernel_numpy(
            x, edge_index, batch,
            np.asarray(W_in, np.float32), np.asarray(b_in, np.float32),
            [np.asarray(w, np.float32) for w in (gW0, gW1, gW2)],
            [np.asarray(w, np.float32) for w in (gas0, gas1, gas2)],
            [np.asarray(w, np.float32) for w in (gad0, gad1, gad2)],
            [np.asarray(w, np.float32) for w in (gb0, gb1, gb2)],
            [np.asarray(w, np.float32) for w in (bng0, bng1, bng2)],
            [np.asarray(w, np.float32) for w in (bnb0, bnb1, bnb2)],
            np.asarray(mW1, np.float32), mb1, mg1, mbeta1,
            np.asarray(mW2, np.float32), mb2, mg2, mbeta2,
            np.asarray(hW, np.float32), hb)


if os.environ.get("BASS_GNN_LAZY", "") != "1":
    _ensure_runner()


# revision 36
# speedup vs baseline: 1.3655x; 1.0325x over previous
import os
import sys

for _p in ("/opt/trn_rl_repo", "/root/.axon_site/_ro/trn_rl_repo"):
    if _p not in sys.path:
        sys.path.insert(0, _p)

import numpy as np
import ml_dtypes

import concourse.bass as bass
import concourse.bacc as bacc
import concourse.mybir as mybir
import concourse.tile as tile
from concourse.masks import make_identity

F32 = mybir.dt.float32
BF16 = mybir.dt.bfloat16
I32 = mybir.dt.int32
I16 = mybir.dt.int16
ALU = mybir.AluOpType
ACT = mybir.ActivationFunctionType
AX = mybir.AxisListType

P = 128
EPS = 1e-5
SLOPE = 0.2
HEADS = 4


class Cfg:
    def __init__(self, ncores=8, nreal=40000, npc=5120, in_dim=64, hid=256,
                 eb=11, gw=224, ks=64, g=1500):
        self.NCORES = ncores
        self.N = nreal                      # real nodes total
        self.NR = nreal // ncores           # real nodes per core
        self.NP = npc                       # padded node slots per core
        self.NT = npc * ncores              # total table slots
        self.IN = in_dim
        self.HID = hid
        self.C = hid // HEADS
        self.EB = eb                        # edge blocks per dst-tile
        self.ET = eb * P                    # edge slot capacity per dst-tile
        self.NPT = npc // P                 # dst-tiles per core
        self.ECB = self.NPT * eb            # edge array free blocks
        self.GW = gw                        # graph window per core
        self.KS = ks                        # slots per graph
        self.RN = gw * ks                   # readout gather count
        self.G = g
        assert npc % P == 0 and self.RN % 512 == 0
        self.PAD_SLOT = self.NR             # global slot with zeroed table row


FULL = Cfg()


# ---------------------------------------------------------------------------
# device program
# ---------------------------------------------------------------------------

def build_nc(cfg: Cfg, dbg=False):
    c = cfg
    HID = c.HID
    nc = bacc.Bacc("TRN2", target_bir_lowering=False, debug=False,
                   num_devices=c.NCORES, num_swdge_queues=1)

    x_in = nc.dram_tensor("x_in", [c.NP, c.IN], BF16, kind="ExternalInput")
    wpk = nc.dram_tensor("wpk", [P, 160], BF16, kind="ExternalInput")
    pvec = nc.dram_tensor("pvec", [22, P], F32, kind="ExternalInput")
    asd = nc.dram_tensor("asd", [6, HID], F32, kind="ExternalInput")
    esrc = nc.dram_tensor("esrc", [P, c.ECB], mybir.dt.uint16, kind="ExternalInput")
    edst32 = nc.dram_tensor("edst32", [P, c.ECB], I16, kind="ExternalInput")
    dstf = nc.dram_tensor("dstf", [P, c.ECB], mybir.dt.int8, kind="ExternalInput")
    ridxs = nc.dram_tensor("ridxs", [16, c.RN // 16], I16, kind="ExternalInput")
    ridxm = nc.dram_tensor("ridxm", [16, c.RN // 16], I16, kind="ExternalInput")

    rsum_o = nc.dram_tensor("rsum_o", [2, P, c.GW], BF16, kind="ExternalOutput")
    rmax_o = nc.dram_tensor("rmax_o", [2, P, c.GW], BF16, kind="ExternalOutput")
    if dbg:
        d_h = [nc.dram_tensor(f"d_h{l}", [c.NP, c.HID], F32,
                              kind="ExternalOutput") for l in range(3)]
        d_o = nc.dram_tensor("d_o", [c.NP, c.HID], F32, kind="ExternalOutput")
        d_g1 = nc.dram_tensor("d_g1", [P, c.EB, c.HID + 16], F32, kind="ExternalOutput")
        d_g2 = nc.dram_tensor("d_g2", [P, c.EB, 8], F32, kind="ExternalOutput")
        d_ex = nc.dram_tensor("d_ex", [P, c.EB, 4], F32, kind="ExternalOutput")
        d_pay = nc.dram_tensor("d_pay", [P, c.EB, 260], F32, kind="ExternalOutput")
        d_pt = nc.dram_tensor("d_pt", [P, 260], F32, kind="ExternalOutput")

    RG = [list(range(c.NCORES))]

    with tile.TileContext(nc) as tc:
        with (
            tc.tile_pool(name="dram", bufs=1, space="DRAM") as dram,
            tc.tile_pool(name="const", bufs=1) as cpool,
            tc.tile_pool(name="persist", bufs=1) as pers,
        ):
            # ---------------- constants / loads ----------------
            ident = cpool.tile([P, P], BF16)
            make_identity(nc, ident[:])
            ones = cpool.tile([P, 1], F32)
            nc.gpsimd.memset(ones[:], 1.0)
            m39 = cpool.tile([P, 1], F32)
            nc.gpsimd.memset(m39[:], 0.0)
            npad = c.NR - (c.NPT - 1) * P   # real rows in the last tile
            nc.gpsimd.memset(m39[0:npad, :], 1.0)

            wpk_sb = cpool.tile([P, 160], BF16)
            nc.sync.dma_start(wpk_sb[:], wpk[:, :])
            wpk_d = dram.tile([P, 160], BF16)
            nc.gpsimd.dma_start(wpk_d[:, :], wpk_sb[:])
            wfull = dram.tile([8 * P * 160], BF16, addr_space="Shared")
            nc.gpsimd.collective_compute(
                "AllGather", ALU.bypass, replica_groups=RG,
                ins=[wpk_d[:, :].opt()], outs=[wfull[:].opt()])
            # wfull layout: wcat0 (64*512) | w1 (256*256) | w2 (256*256)
            wc0_sb = cpool.tile([c.IN, 2 * HID], BF16)
            nc.sync.dma_start(
                wc0_sb[:],
                wfull[0:c.IN * 2 * HID].rearrange("(a b) -> a b", b=2 * HID))
            w1_sb = cpool.tile([P, 2, HID], BF16)
            w2_sb = cpool.tile([P, 2, HID], BF16)
            OF1 = c.IN * 2 * HID
            OF2 = OF1 + HID * HID
            for hf in range(2):
                nc.sync.dma_start(
                    w1_sb[:, hf, :],
                    wfull[OF1 + hf * P * HID:OF1 + (hf + 1) * P * HID].rearrange(
                        "(a b) -> a b", b=HID))
                nc.sync.dma_start(
                    w2_sb[:, hf, :],
                    wfull[OF2 + hf * P * HID:OF2 + (hf + 1) * P * HID].rearrange(
                        "(a b) -> a b", b=HID))

            def bcast256(pool, dst, src0, src1):
                scr = pool.tile([1, 2 * P], F32, tag="bscr", bufs=2)
                nc.sync.dma_start(scr[0:1, 0:P], src0)
                nc.sync.dma_start(scr[0:1, P:2 * P], src1)
                nc.gpsimd.partition_broadcast(dst[:, :], scr[0:1, :])

            as_bc, ad_bc, gb_bc = [], [], []
            for l in range(3):
                a_s = pers.tile([P, HID], F32, name=f"as_bc{l}")
                a_d = pers.tile([P, HID], F32, name=f"ad_bc{l}")
                bcast256(pers, a_s, asd[2 * l:2 * l + 1, 0:P],
                         asd[2 * l:2 * l + 1, P:2 * P])
                bcast256(pers, a_d, asd[2 * l + 1:2 * l + 2, 0:P],
                         asd[2 * l + 1:2 * l + 2, P:2 * P])
                as_bc.append(a_s)
                ad_bc.append(a_d)
                g_b = pers.tile([P, HID], F32, name=f"gb_bc{l}")
                bcast256(pers, g_b, pvec[2 + 6 * l:3 + 6 * l, :],
                         pvec[3 + 6 * l:4 + 6 * l, :])
                gb_bc.append(g_b)
            bin_bc = pers.tile([P, HID], F32)
            bcast256(pers, bin_bc, pvec[0:1, :], pvec[1:2, :])
            iota_bc = pers.tile([P, P], F32)
            scr0 = pers.tile([1, P], F32, name="scr0")
            nc.sync.dma_start(scr0[0:1, :], pvec[20:21, :])
            nc.gpsimd.partition_broadcast(iota_bc[:, :], scr0[0:1, :])

            esrc_16 = pers.tile([P, c.ECB], mybir.dt.uint16, name="esrc16")
            nc.sync.dma_start(esrc_16[:], esrc[:, :])
            esrc_sb = pers.tile([P, c.ECB], I32)
            nc.vector.tensor_copy(esrc_sb[:], esrc_16[:])
            edst_16 = pers.tile([P, c.ECB], I16, name="edst16t")
            nc.sync.dma_start(edst_16[:], edst32[:, :])
            edst32_sb = pers.tile([P, c.ECB], I32)
            nc.vector.tensor_copy(edst32_sb[:], edst_16[:])
            dstf_8 = pers.tile([P, c.ECB], mybir.dt.int8, name="dstf8")
            nc.sync.dma_start(dstf_8[:], dstf[:, :])
            dstf_sb = pers.tile([P, c.ECB], F32)
            nc.vector.tensor_copy(dstf_sb[:], dstf_8[:])
            ridxs_sb = pers.tile([P, c.RN // 16], I16)
            ridxm_sb = pers.tile([P, c.RN // 16], I16)
            for k in range(8):
                nc.sync.dma_start(ridxs_sb[16 * k:16 * (k + 1), :], ridxs[:, :])
                nc.sync.dma_start(ridxm_sb[16 * k:16 * (k + 1), :], ridxm[:, :])

            x_bf = pers.tile([P, c.NPT, c.IN], BF16)
            for t in range(c.NPT):
                nc.sync.dma_start(x_bf[:, t, :], x_in[t * P:(t + 1) * P, :])

            h_bf = pers.tile([P, c.NPT, HID], BF16)
            o_sb = pers.tile([P, c.NPT, HID], F32)

            hshort = dram.tile([c.NP, HID], F32)
            h_cur = [dram.tile([c.NP, HID], F32, name=f"h{l}") for l in range(3)]

            # ---------------- layers ----------------
            for l in range(3):
                tbl_own = dram.tile([c.NP, HID + 16], BF16, name=f"tblo{l}")
                tblS_own = dram.tile([c.NP, 8], F32, name=f"tblso{l}")
                tbl = dram.tile([c.NT, HID + 16], BF16, name=f"tbl{l}",
                                addr_space="Shared")
                stats_own = dram.tile([4, P], F32, name=f"sto{l}")
                stats_all = dram.tile([4, P], F32, name=f"sta{l}",
                                      addr_space="Shared")

                with (
                    tc.tile_pool(name=f"tl{l}", bufs=2) as tl,
                    tc.tile_pool(name=f"tps{l}", bufs=2, space="PSUM") as tps,
                    tc.tile_pool(name=f"sps{l}", bufs=1, space="PSUM") as sps,
                    tc.tile_pool(name=f"ep{l}", bufs=2) as ep,
                    tc.tile_pool(name=f"sm{l}", bufs=1) as sm,
                ):
                    # ---- table build (own slice) ----
                    for t in range(c.NPT):
                        if l == 0:
                            xt = x_bf[:, t, :]
                            tp = tps.tile([P, P], BF16, tag="tp")
                            hT = tl.tile([P, 2, P], BF16, tag="hT")
                            nc.tensor.transpose(tp[0:c.IN, :], xt, ident[:])
                            nc.vector.tensor_copy(hT[0:c.IN, 0, :], tp[0:c.IN, :])
                            xw_ps = tps.tile([P, 2 * HID], F32, tag="xwps")
                            nc.tensor.matmul(xw_ps[:, :], hT[0:c.IN, 0, :],
                                             wc0_sb[:, :], start=True, stop=True)
                            hs = tl.tile([P, HID], F32, tag="hs")
                            nc.vector.tensor_add(hs[:], xw_ps[:, 0:HID], bin_bc[:])
                            nc.sync.dma_start(hshort[t * P:(t + 1) * P, :], hs[:])
                            xw = xw_ps[:, HID:2 * HID]
                        else:
                            ht = h_bf[:, t, :]
                            wsb = w1_sb if l == 1 else w2_sb
                            xw_ps = tps.tile([P, 2 * HID], F32, tag="xwps")
                            hT = tl.tile([P, 2, P], BF16, tag="hT")
                            for hf in range(2):
                                tp = tps.tile([P, P], BF16, tag="tp")
                                nc.tensor.transpose(
                                    tp[:, :], ht[:, hf * P:(hf + 1) * P], ident[:])
                                nc.vector.tensor_copy(hT[:, hf, :], tp[:, :])
                            for hf in range(2):
                                nc.tensor.matmul(xw_ps[:, 0:HID], hT[:, hf, :],
                                                 wsb[:, hf, :], start=(hf == 0),
                                                 stop=(hf == 1))
                            xw = xw_ps[:, 0:HID]

                        xwb = tl.tile([P, HID], BF16, tag="xwb")
                        if t == c.NPT - 1:
                            nc.vector.tensor_scalar_mul(xwb[:], xw, m39[:, 0:1])
                        else:
                            nc.vector.tensor_copy(xwb[:], xw)
                        nc.sync.dma_start(tbl_own[t * P:(t + 1) * P, 0:HID],
                                          xwb[:])

                        sd = tl.tile([P, 8], F32, tag="sd")
                        sc = tl.tile([P, c.C], F32, tag="sc")
                        for hd in range(4):
                            sl = slice(hd * c.C, (hd + 1) * c.C)
                            nc.vector.tensor_tensor(
                                out=sc[:], in0=xw[:, sl], in1=as_bc[l][:, sl],
                                op=ALU.mult)
                            nc.vector.tensor_reduce(
                                sd[:, hd:hd + 1], sc[:], axis=AX.X, op=ALU.add)
                            nc.vector.tensor_tensor(
                                out=sc[:], in0=xw[:, sl], in1=ad_bc[l][:, sl],
                                op=ALU.mult)
                            nc.vector.tensor_reduce(
                                sd[:, 4 + hd:5 + hd], sc[:], axis=AX.X,
                                op=ALU.add)
                        nc.sync.dma_start(tblS_own[t * P:(t + 1) * P, :], sd[:])
                        nc.sync.dma_start(
                            tbl_own[t * P:(t + 1) * P, HID:HID + 16],
                            sd[:].bitcast(BF16))

                    nc.gpsimd.collective_compute(
                        "AllGather", ALU.bypass, replica_groups=RG,
                        ins=[tbl_own[:, :].opt()], outs=[tbl[:, :].opt()])

                    # ---- edge phase: gather / attention / segment matmul ----
                    s_o = sm.tile([P, HID], F32, name=f"s_o{l}")
                    s_q = sm.tile([P, HID], F32, name=f"s_q{l}")
                    nc.vector.memset(s_o[:], 0.0)
                    nc.vector.memset(s_q[:], 0.0)
                    for t in range(c.NPT):
                        g1 = ep.tile([P, c.EB, HID + 16], BF16, tag="g1")
                        g2 = ep.tile([P, c.EB, 8], F32, tag="g2")
                        for b in range(c.EB):
                            col = t * c.EB + b
                            nc.gpsimd.indirect_dma_start(
                                out=g1[:, b, :], out_offset=None, in_=tbl[:, :],
                                in_offset=bass.IndirectOffsetOnAxis(
                                    ap=esrc_sb[:, col:col + 1], axis=0))
                            nc.gpsimd.indirect_dma_start(
                                out=g2[:, b, :], out_offset=None,
                                in_=tblS_own[:, :],
                                in_offset=bass.IndirectOffsetOnAxis(
                                    ap=edst32_sb[:, col:col + 1], axis=0))

                        g1sd = g1[:, :, HID:HID + 16].bitcast(F32)
                        ee = ep.tile([P, c.EB, 4], F32, tag="ee")
                        nc.vector.tensor_add(ee[:], g1sd[:, :, 0:4],
                                             g2[:, :, 4:8])
                        e2 = ep.tile([P, c.EB, 4], F32, tag="e2")
                        nc.scalar.mul(e2[:], ee[:], SLOPE)
                        nc.vector.tensor_tensor(out=ee[:], in0=ee[:], in1=e2[:],
                                                op=ALU.max)
                        ex = ep.tile([P, c.EB, 4], F32, tag="ex")
                        nc.scalar.activation(ex[:], ee[:], ACT.Exp)

                        pay = ep.tile([P, c.EB, 260], BF16, tag="pay")
                        for hd in range(4):
                            sl = slice(hd * c.C, (hd + 1) * c.C)
                            nc.vector.tensor_tensor(
                                out=pay[:, :, sl], in0=g1[:, :, sl],
                                in1=ex[:, :, hd:hd + 1].to_broadcast(
                                    [P, c.EB, c.C]),
                                op=ALU.mult)
                        nc.vector.tensor_copy(pay[:, :, 256:260], ex[:])

                        pt = tps.tile([P, 260], F32, tag="pt")
                        for b in range(c.EB):
                            S = ep.tile([P, P], BF16, tag="S")
                            nc.vector.tensor_scalar(
                                out=S[:], in0=iota_bc[:],
                                scalar1=dstf_sb[:, t * c.EB + b:
                                                t * c.EB + b + 1],
                                scalar2=None, op0=ALU.is_equal)
                            nc.tensor.matmul(pt[:, :], S[:], pay[:, b, :],
                                             start=(b == 0),
                                             stop=(b == c.EB - 1))

                        if dbg and l == 0 and t == 0:
                            dcp = tl.tile([P, c.EB, c.HID + 16], F32, tag="dcp")
                            nc.vector.tensor_copy(dcp[:], g1[:])
                            nc.sync.dma_start(d_g1[:, :, :], dcp[:])
                            nc.sync.dma_start(d_g2[:, :, :], g2[:])
                            nc.sync.dma_start(d_ex[:, :, :], ex[:])
                            dcq = tl.tile([P, c.EB, 260], F32, tag="dcq")
                            nc.vector.tensor_copy(dcq[:], pay[:])
                            nc.sync.dma_start(d_pay[:, :, :], dcq[:])
                            dcr = tl.tile([P, 260], F32, tag="dcr")
                            nc.vector.tensor_copy(dcr[:], pt[:])
                            nc.sync.dma_start(d_pt[:, :], dcr[:])

                        # ---- finalize tile: alpha-div, bias, stats ----
                        den = tl.tile([P, 4], F32, tag="den")
                        nc.vector.tensor_scalar_max(den[:], pt[:, 256:260],
                                                    1e-20)
                        rec = tl.tile([P, 4], F32, tag="rec")
                        nc.vector.reciprocal(rec[:], den[:])
                        ot = o_sb[:, t, :]
                        for hd in range(4):
                            sl = slice(hd * c.C, (hd + 1) * c.C)
                            nc.vector.tensor_scalar_mul(ot[:, sl], pt[:, sl],
                                                        rec[:, hd:hd + 1])
                        nc.vector.tensor_add(ot, ot, gb_bc[l][:])
                        if t == c.NPT - 1:
                            om = tl.tile([P, HID], F32, tag="om")
                            nc.vector.tensor_scalar_mul(om[:], ot, m39[:, 0:1])
                            stat_in = om[:]
                        else:
                            stat_in = ot
                        sq = tl.tile([P, HID], F32, tag="sq")
                        nc.vector.tensor_tensor(out=sq[:], in0=stat_in,
                                                in1=stat_in, op=ALU.mult)
                        nc.vector.tensor_add(s_o[:], s_o[:], stat_in)
                        nc.vector.tensor_add(s_q[:], s_q[:], sq[:])

                    if dbg and l == 0:
                        for t in range(c.NPT):
                            nc.sync.dma_start(d_o[t * P:(t + 1) * P, :],
                                              o_sb[:, t, :])

                    # ---- BN stats reduce + collective ----
                    st_ps = sps.tile([P, 4], F32, name=f"stp{l}")
                    for hf in range(2):
                        nc.tensor.matmul(
                            st_ps[:, hf:hf + 1], s_o[:, hf * P:(hf + 1) * P],
                            ones[:], start=True, stop=True,
                            skip_group_check=True)
                        nc.tensor.matmul(
                            st_ps[:, 2 + hf:3 + hf], s_q[:, hf * P:(hf + 1) * P],
                            ones[:], start=True, stop=True,
                            skip_group_check=True)
                    stq = sm.tile([P, 4], F32, name=f"stq{l}")
                    nc.vector.tensor_copy(stq[:], st_ps[:])
                    nc.sync.dma_start(stats_own[:, :].transpose([1, 0]), stq[:])
                    nc.gpsimd.collective_compute(
                        "AllReduce", ALU.add, replica_groups=RG,
                        ins=[stats_own[:, :].opt()], outs=[stats_all[:, :].opt()])

                    st_s = sm.tile([2, P], F32, name=f"sts{l}")
                    nc.sync.dma_start(st_s[:], stats_all[0:2, :])
                    st_q = sm.tile([2, P], F32, name=f"stq2{l}")
                    nc.sync.dma_start(st_q[:], stats_all[2:4, :])
                    g2t = sm.tile([2, P], F32, name=f"g2t{l}")
                    nc.sync.dma_start(g2t[:], pvec[4 + 6 * l:6 + 6 * l, :])
                    be2t = sm.tile([2, P], F32, name=f"be2t{l}")
                    nc.sync.dma_start(be2t[:], pvec[6 + 6 * l:8 + 6 * l, :])
                    mu = sm.tile([2, P], F32, name=f"mu{l}")
                    nc.scalar.mul(mu[:], st_s[:], 1.0 / c.N)
                    va = sm.tile([2, P], F32, name=f"va{l}")
                    nc.scalar.mul(va[:], st_q[:], 1.0 / c.N)
                    mu2 = sm.tile([2, P], F32, name=f"mu2{l}")
                    nc.vector.tensor_tensor(out=mu2[:], in0=mu[:], in1=mu[:],
                                            op=ALU.mult)
                    nc.vector.tensor_tensor(out=va[:], in0=va[:], in1=mu2[:],
                                            op=ALU.subtract)
                    nc.vector.tensor_scalar_add(va[:], va[:], EPS)
                    sdv = sm.tile([2, P], F32, name=f"sdv{l}")
                    nc.scalar.sqrt(sdv[:], va[:])
                    rs = sm.tile([2, P], F32, name=f"rs{l}")
                    nc.vector.reciprocal(rs[:], sdv[:])
                    A2 = sm.tile([2, P], F32, name=f"A2{l}")
                    nc.vector.tensor_tensor(out=A2[:], in0=rs[:], in1=g2t[:],
                                            op=ALU.mult)
                    B2 = sm.tile([2, P], F32, name=f"B2{l}")
                    nc.vector.tensor_tensor(out=B2[:], in0=mu[:], in1=A2[:],
                                            op=ALU.mult)
                    nc.vector.tensor_tensor(out=B2[:], in0=be2t[:],
                                            in1=B2[:], op=ALU.subtract)
                    A_bc = sm.tile([P, HID], F32, name=f"Abc{l}")
                    B_bc = sm.tile([P, HID], F32, name=f"Bbc{l}")
                    bcast256(sm, A_bc, A2[0:1, :], A2[1:2, :])
                    bcast256(sm, B_bc, B2[0:1, :], B2[1:2, :])

                    # ---- apply pass ----
                    for t in range(c.NPT):
                        u1 = tl.tile([P, HID], F32, tag="u1")
                        nc.vector.tensor_tensor(out=u1[:], in0=o_sb[:, t, :],
                                                in1=A_bc[:], op=ALU.mult)
                        nc.vector.tensor_add(u1[:], u1[:], B_bc[:])
                        u2 = tl.tile([P, HID], F32, tag="u2")
                        nc.vector.tensor_scalar_min(u2[:], u1[:], 0.0)
                        nc.scalar.activation(u2[:], u2[:], ACT.Exp)
                        rl = tl.tile([P, HID], F32, tag="rl")
                        nc.scalar.activation(rl[:], u1[:], ACT.Relu)
                        nc.vector.tensor_add(u2[:], u2[:], rl[:])
                        nc.vector.tensor_scalar_add(u2[:], u2[:], -1.0)
                        hp = tl.tile([P, HID], F32, tag="hp")
                        src = hshort if l == 0 else h_cur[l - 1]
                        nc.sync.dma_start(hp[:], src[t * P:(t + 1) * P, :])
                        nc.vector.tensor_add(u2[:], u2[:], hp[:])
                        nc.sync.dma_start(h_cur[l][t * P:(t + 1) * P, :], u2[:])
                        if dbg:
                            nc.sync.dma_start(d_h[l][t * P:(t + 1) * P, :],
                                              u2[:])
                        nc.vector.tensor_copy(h_bf[:, t, :], u2[:])

            # ---------------- readout ----------------
            h3tbl = dram.tile([c.NP + P, HID], BF16)
            for t in range(c.NPT):
                nc.sync.dma_start(h3tbl[t * P:(t + 1) * P, :], h_bf[:, t, :])
            with (
                tc.tile_pool(name="rd", bufs=2) as rd,
                tc.tile_pool(name="rs1", bufs=1) as rs1,
            ):
                sent0 = rs1.tile([1, HID], BF16, name="sent0")
                nc.gpsimd.memset(sent0[:], 0.0)
                sent1 = rs1.tile([1, HID], BF16, name="sent1")
                nc.gpsimd.memset(sent1[:], -1e30)
                nc.sync.dma_start(h3tbl[c.NP:c.NP + 1, :], sent0[:])
                nc.sync.dma_start(h3tbl[c.NP + 1:c.NP + 2, :], sent1[:])

                rsum_sb = rs1.tile([P, 2, c.GW], F32, name="rsum_sb")
                rmax_sb = rs1.tile([P, 2, c.GW], F32, name="rmax_sb")
                GSZ = 512
                gpg = GSZ // c.KS
                for j in range(c.RN // GSZ):
                    isl = slice(j * (GSZ // 16), (j + 1) * (GSZ // 16))
                    gsl = slice(j * gpg, (j + 1) * gpg)
                    gr = rd.tile([P, 2, GSZ], BF16, tag="gr")
                    nc.gpsimd.dma_gather(
                        out_ap=gr[:], in_ap=h3tbl[:, :],
                        idxs_ap=ridxs_sb[:, isl],
                        num_idxs=GSZ, num_idxs_reg=GSZ, elem_size=HID,
                        transpose=True)
                    nc.vector.tensor_reduce(
                        rsum_sb[:, :, gsl],
                        gr[:].rearrange("p b (g k) -> p b g k", k=c.KS),
                        axis=AX.X, op=ALU.add)
                    gm = rd.tile([P, 2, GSZ], BF16, tag="gm")
                    nc.gpsimd.dma_gather(
                        out_ap=gm[:], in_ap=h3tbl[:, :],
                        idxs_ap=ridxm_sb[:, isl],
                        num_idxs=GSZ, num_idxs_reg=GSZ, elem_size=HID,
                        transpose=True)
                    nc.vector.tensor_reduce(
                        rmax_sb[:, :, gsl],
                        gm[:].rearrange("p b (g k) -> p b g k", k=c.KS),
                        axis=AX.X, op=ALU.max)

                rsum_bf = rs1.tile([P, 2, c.GW], BF16, name="rsum_bf")
                nc.vector.tensor_copy(rsum_bf[:], rsum_sb[:])
                rmax_bf = rs1.tile([P, 2, c.GW], BF16, name="rmax_bf")
                nc.vector.tensor_copy(rmax_bf[:], rmax_sb[:])
                nc.sync.dma_start(rsum_o[:, :, :].transpose([1, 0, 2]),
                                  rsum_bf[:])
                nc.sync.dma_start(rmax_o[:, :, :].transpose([1, 0, 2]),
                                  rmax_bf[:])

    nc.compile()
    return nc


# ---------------------------------------------------------------------------
# host-side prep
# ---------------------------------------------------------------------------

def host_prep(cfg: Cfg, x, edge_index, batch):
    c = cfg
    bf = ml_dtypes.bfloat16
    n = c.N
    loop = np.arange(n, dtype=np.int64)
    src = np.concatenate([np.asarray(edge_index[0], np.int64), loop])
    dst = np.concatenate([np.asarray(edge_index[1], np.int64), loop])
    src_slot = ((src // c.NR) * c.NP + src % c.NR).astype(np.int64)
    dst_core = dst // c.NR
    dst_local = (dst % c.NR).astype(np.int64)
    batch = np.asarray(batch, np.int64)

    xscale = float(np.abs(x).max()) / 127.0 + 1e-30
    per_core = []
    gfirsts = []
    for cc in range(c.NCORES):
        m = dst_core == cc
        es = src_slot[m]
        ed = dst_local[m]
        order = np.argsort(ed, kind="stable")
        es, ed = es[order], ed[order]
        tile_id = ed // P
        counts = np.bincount(tile_id, minlength=c.NPT)
        if counts.max() > c.ET:
            raise OverflowError("edge tile capacity exceeded")
        starts = np.zeros(c.NPT, np.int64)
        np.cumsum(counts[:-1], out=starts[1:])
        pos_in_tile = np.arange(len(ed)) - starts[tile_id]
        es_f = np.full(c.NPT * c.ET, c.PAD_SLOT, np.int64)
        ed_f = np.zeros(c.NPT * c.ET, np.int64)
        df_f = np.full(c.NPT * c.ET, -1, np.int64)
        slot = tile_id * c.ET + pos_in_tile
        es_f[slot] = es
        ed_f[slot] = ed
        df_f[slot] = ed % P
        # slot s of tile t -> (p = s % 128, col = t*EB + s//128)
        es2d = np.ascontiguousarray(
            es_f.reshape(c.NPT, c.EB, P).transpose(2, 0, 1).reshape(
                P, c.ECB)).astype(np.uint16)
        ed2d = np.ascontiguousarray(
            ed_f.reshape(c.NPT, c.EB, P).transpose(2, 0, 1).reshape(
                P, c.ECB)).astype(np.int16)
        df2d = np.ascontiguousarray(
            df_f.reshape(c.NPT, c.EB, P).transpose(2, 0, 1).reshape(
                P, c.ECB)).astype(np.int8)

        bsl = batch[cc * c.NR:(cc + 1) * c.NR]
        gfirst = int(bsl[0])
        gfirsts.append(gfirst)
        w = (bsl - gfirst).astype(np.int64)
        uniq, first_idx = np.unique(w, return_index=True)
        fi = np.zeros(int(w[-1]) + 1, np.int64)
        fi[uniq] = first_idx
        kwi = np.arange(c.NR) - fi[w]
        if int(w[-1]) >= c.GW or int(kwi.max()) >= c.KS:
            raise OverflowError("readout window exceeded")
        sidx_s = np.full(c.RN, c.NP, np.int64)
        sidx_m = np.full(c.RN, c.NP + 1, np.int64)
        pos = w * c.KS + kwi
        sidx_s[pos] = np.arange(c.NR)
        sidx_m[pos] = np.arange(c.NR)
        ridxs_a = sidx_s.reshape(c.RN // 16, 16).T.astype(np.int16)
        ridxm_a = sidx_m.reshape(c.RN // 16, 16).T.astype(np.int16)

        xp = np.zeros((c.NP, c.IN), np.float32)
        xp[:c.NR] = x[cc * c.NR:(cc + 1) * c.NR]
        per_core.append(dict(
            x_in=xp.astype(bf), esrc=es2d, edst32=ed2d, dstf=df2d,
            ridxs=ridxs_a, ridxm=ridxm_a))
    return per_core, gfirsts, xscale


def host_weights(cfg: Cfg, W_in, gW0, gW1, gW2, b_in, gb, bng, bnb, a_s, a_d,
                 xscale=1.0):
    bf = ml_dtypes.bfloat16
    wcat0 = np.concatenate([np.asarray(W_in, np.float32),
                            np.asarray(gW0, np.float32)], axis=1).astype(bf)
    pvec = np.zeros((22, P), np.float32)
    pvec[21, 0] = xscale
    pvec[0:2] = np.asarray(b_in, np.float32).reshape(2, P)
    for l in range(3):
        pvec[2 + 6 * l:4 + 6 * l] = np.asarray(gb[l], np.float32).reshape(2, P)
        pvec[4 + 6 * l:6 + 6 * l] = np.asarray(bng[l], np.float32).reshape(2, P)
        pvec[6 + 6 * l:8 + 6 * l] = np.asarray(bnb[l], np.float32).reshape(2, P)
    pvec[20] = np.arange(P, dtype=np.float32)
    asd = np.zeros((6, cfg.HID), np.float32)
    for l in range(3):
        asd[2 * l] = np.asarray(a_s[l], np.float32).reshape(-1)
        asd[2 * l + 1] = np.asarray(a_d[l], np.float32).reshape(-1)
    flat = np.concatenate([
        np.asarray(wcat0, bf).ravel(),
        np.asarray(gW1, np.float32).astype(bf).ravel(),
        np.asarray(gW2, np.float32).astype(bf).ravel()])
    wpk = flat.reshape(8, P, 160)
    return dict(pvec=pvec, asd=asd), wpk


def host_finish(cfg: Cfg, outs, gfirsts, batch, mW1, mb1, mg1, mbeta1,
                mW2, mb2, mg2, mbeta2, hW, hb):
    c = cfg
    batch = np.asarray(batch, np.int64)
    cnt = np.bincount(batch, minlength=c.G).astype(np.float32)
    hsum = np.zeros((c.G, c.HID), np.float32)
    hmax = np.full((c.G, c.HID), -np.inf, np.float32)
    for cc in range(c.NCORES):
        g0 = gfirsts[cc]
        ng = min(c.GW, c.G - g0)
        rs = np.asarray(outs[cc]["rsum_o"], np.float32).reshape(
            2 * P, c.GW)[:c.HID, :ng].T
        rm = np.asarray(outs[cc]["rmax_o"], np.float32).reshape(
            2 * P, c.GW)[:c.HID, :ng].T
        hsum[g0:g0 + ng] += rs
        hmax[g0:g0 + ng] = np.maximum(hmax[g0:g0 + ng], rm)
    hmean = hsum / np.maximum(cnt, 1.0)[:, None]
    hmax = np.where((cnt[:, None] > 0) & (hmax > -1e29), hmax, 0.0)
    hg = np.concatenate([hmean, hmax], axis=1).astype(np.float32)

    def bn(h, g, b):
        mu = h.mean(0, dtype=np.float32)
        v = ((h - mu) ** 2).mean(0, dtype=np.float32)
        return (h - mu) / np.sqrt(v + EPS) * g + b

    s = np.maximum(bn(hg @ np.asarray(mW1, np.float32) + mb1, mg1, mbeta1), 0.0)
    s = np.maximum(bn(s @ np.asarray(mW2, np.float32) + mb2, mg2, mbeta2), 0.0)
    return (s @ np.asarray(hW, np.float32) + hb).astype(np.float32)


# ---------------------------------------------------------------------------
# persistent PJRT runner (compile once, reuse)
# ---------------------------------------------------------------------------

class Runner:
    def __init__(self, nc, n_cores):
        import jax
        from jax.sharding import Mesh, PartitionSpec
        from jax.experimental.shard_map import shard_map
        from concourse import bass2jax
        try:
            jax.config.update("jax_compilation_cache_dir", "/tmp/jax_pcc")
            jax.config.update("jax_persistent_cache_min_entry_size_bytes", -1)
            jax.config.update("jax_persistent_cache_min_compile_time_secs", 0)
        except Exception:
            pass
        bass2jax.install_neuronx_cc_hook()
        self.nc = nc
        self.n_cores = n_cores
        partition_name = (nc.partition_id_tensor.name
                          if getattr(nc, "partition_id_tensor", None) is not None
                          else None)
        in_names, out_names, out_avals, zero_shapes = [], [], [], []
        self.in_specs = {}
        for alloc in nc.m.functions[0].allocations:
            if not isinstance(alloc, mybir.MemoryLocationSet):
                continue
            name = alloc.memorylocations[0].name
            if alloc.kind == "ExternalInput":
                if name == partition_name:
                    continue
                in_names.append(name)
                self.in_specs[name] = (tuple(alloc.tensor_shape),
                                       mybir.dt.np(alloc.dtype))
            elif alloc.kind == "ExternalOutput":
                shape = tuple(alloc.tensor_shape)
                dtype = mybir.dt.np(alloc.dtype)
                out_names.append(name)
                out_avals.append(jax.core.ShapedArray(shape, dtype))
                zero_shapes.append((shape, dtype))
        self.in_names = in_names
        self.out_names = out_names
        self.out_avals = out_avals
        self.zero_shapes = zero_shapes
        n_params = len(in_names)
        all_names = list(in_names) + list(out_names)
        if partition_name is not None:
            all_names.append(partition_name)
        donate = tuple(range(n_params, n_params + len(out_names)))

        def _body(*args):
            operands = list(args)
            if partition_name is not None:
                operands.append(bass2jax.partition_id_tensor())
            outs = bass2jax._bass_exec_p.bind(
                *operands,
                out_avals=tuple(out_avals),
                in_names=tuple(all_names),
                out_names=tuple(out_names),
                lowering_input_output_aliases=(),
                sim_require_finite=False,
                sim_require_nnan=False,
                nc=nc,
            )
            return tuple(outs)

        devices = jax.devices()[:n_cores]
        mesh = Mesh(np.asarray(devices), ("core",))
        from jax.sharding import NamedSharding
        self._zero_sh = NamedSharding(mesh, PartitionSpec("core"))
        nin = n_params + len(out_names)
        self._fn = jax.jit(
            shard_map(_body, mesh=mesh,
                      in_specs=(PartitionSpec("core"),) * nin,
                      out_specs=(PartitionSpec("core"),) * len(out_names),
                      check_rep=False),
            donate_argnums=donate, keep_unused=True)

    def run(self, in_maps):
        concat = [np.concatenate([np.asarray(m[nm]) for m in in_maps], axis=0)
                  for nm in self.in_names]
        zeros = [np.zeros((self.n_cores * s[0], *s[1:]), d)
                 for s, d in self.zero_shapes]
        out_arrs = self._fn(*concat, *zeros)
        res = []
        for cc in range(self.n_cores):
            res.append({nm: np.asarray(out_arrs[i]).reshape(
                self.n_cores, *self.out_avals[i].shape)[cc]
                for i, nm in enumerate(self.out_names)})
        return res

    def warm(self):
        in_maps = []
        for cc in range(self.n_cores):
            m = {nm: np.zeros(sh, dt)
                 for nm, (sh, dt) in self.in_specs.items()}
            in_maps.append(m)
        self.run(in_maps)


_RUNNER = None


def _ensure_runner():
    global _RUNNER
    if _RUNNER is None:
        nc = build_nc(FULL)
        _RUNNER = Runner(nc, FULL.NCORES)
        _RUNNER.warm()
    return _RUNNER


# ---------------------------------------------------------------------------
# entry point
# ---------------------------------------------------------------------------

def _kernel_numpy(x, edge_index, batch, W_in, b_in, gW, gas, gad, gb, bng,
                  bnb, mW1, mb1, mg1, mbeta1, mW2, mb2, mg2, mbeta2, hW, hb):
    # pure-host fallback (slow) in case device capacity assumptions fail
    n = x.shape[0]
    G = FULL.G
    loop = np.arange(n)
    src = np.concatenate([edge_index[0], loop])
    dst = np.concatenate([edge_index[1], loop])
    order = np.argsort(dst, kind="stable")
    srcs, dsts = src[order], dst[order]
    counts = np.bincount(dsts, minlength=n)
    starts = np.zeros(n, np.int64)
    np.cumsum(counts[:-1], out=starts[1:])

    def bn(h, g, b):
        mu = h.mean(0)
        v = ((h - mu) ** 2).mean(0)
        return (h - mu) / np.sqrt(v + EPS) * g + b

    h_short = x @ W_in + b_in
    h = x
    for i in range(3):
        xw = (h @ gW[i]).reshape(n, HEADS, -1)
        ssum = np.einsum("nhc,hc->nh", xw, gas[i])
        dsum = np.einsum("nhc,hc->nh", xw, gad[i])
        e = ssum[srcs] + dsum[dsts]
        e = np.where(e > 0, e, SLOPE * e)
        m = np.maximum.reduceat(e, starts, axis=0)
        ex = np.exp(e - m[dsts])
        den = np.add.reduceat(ex, starts, axis=0)
        alpha = ex / den[dsts]
        out = np.add.reduceat(xw[srcs] * alpha[:, :, None], starts,
                              axis=0).reshape(n, -1) + gb[i]
        hn = bn(out, bng[i], bnb[i])
        hn = np.where(hn > 0, hn, np.expm1(np.minimum(hn, 0)))
        h = hn + (h_short if i == 0 else h)
    cnt = np.bincount(batch, minlength=G).astype(np.float32)
    hsum = np.zeros((G, h.shape[1]), np.float32)
    np.add.at(hsum, batch, h)
    hmax = np.full((G, h.shape[1]), -np.inf, np.float32)
    np.maximum.at(hmax, batch, h)
    hmax = np.where(cnt[:, None] > 0, hmax, 0.0)
    hg = np.concatenate([hsum / np.maximum(cnt, 1.0)[:, None], hmax], axis=1)
    s = np.maximum(bn(hg @ mW1 + mb1, mg1, mbeta1), 0.0)
    s = np.maximum(bn(s @ mW2 + mb2, mg2, mbeta2), 0.0)
    return (s @ hW + hb).astype(np.float32)


def kernel(x, edge_index, batch, W_in, b_in, gW0, gas0, gad0, gb0, bng0, bnb0,
           gW1, gas1, gad1, gb1, bng1, bnb1, gW2, gas2, gad2, gb2, bng2, bnb2,
           mW1, mb1, mg1, mbeta1, mW2, mb2, mg2, mbeta2, hW, hb):
    c = FULL
    x = np.asarray(x, np.float32)
    edge_index = np.asarray(edge_index)
    batch = np.asarray(batch)
    try:
        runner = _ensure_runner()
        per_core, gfirsts, xscale = host_prep(c, x, edge_index, batch)
        wmap, wpk = host_weights(c, W_in, gW0, gW1, gW2, b_in,
                                 [gb0, gb1, gb2], [bng0, bng1, bng2],
                                 [bnb0, bnb1, bnb2],
                                 [gas0, gas1, gas2], [gad0, gad1, gad2],
                                 xscale=xscale)
        in_maps = [dict(pc, wpk=wpk[i], **wmap)
                   for i, pc in enumerate(per_core)]
        outs = runner.run(in_maps)
        return host_finish(c, outs, gfirsts, batch, mW1, mb1, mg1, mbeta1,
                           mW2, mb2, mg2, mbeta2, hW, hb)
    except OverflowError:
        return _kernel_numpy(
            x, edge_index, batch,
            np.asarray(W_in, np.float32), np.asarray(b_in, np.float32),
            [np.asarray(w, np.float32) for w in (gW0, gW1, gW2)],
            [np.asarray(w, np.float32) for w in (gas0, gas1, gas2)],
            [np.asarray(w, np.float32) for w in (gad0, gad1, gad2)],
            [np.asarray(w, np.float32) for w in (gb0, gb1, gb2)],
            [np.asarray(w, np.float32) for w in (bng0, bng1, bng2)],
            [np.asarray(w, np.float32) for w in (bnb0, bnb1, bnb2)],
            np.asarray(mW1, np.float32), mb1, mg1, mbeta1,
            np.asarray(mW2, np.float32), mb2, mg2, mbeta2,
            np.asarray(hW, np.float32), hb)


if os.environ.get("BASS_GNN_LAZY", "") != "1":
    _ensure_runner()


# revision 37
# speedup vs baseline: 1.4450x; 1.0583x over previous
import os
import sys

for _p in ("/opt/trn_rl_repo", "/root/.axon_site/_ro/trn_rl_repo"):
    if _p not in sys.path:
        sys.path.insert(0, _p)

import numpy as np
import ml_dtypes

import concourse.bass as bass
import concourse.bacc as bacc
import concourse.mybir as mybir
import concourse.tile as tile
from concourse.masks import make_identity

F32 = mybir.dt.float32
BF16 = mybir.dt.bfloat16
I32 = mybir.dt.int32
I16 = mybir.dt.int16
ALU = mybir.AluOpType
ACT = mybir.ActivationFunctionType
AX = mybir.AxisListType

P = 128
EPS = 1e-5
SLOPE = 0.2
HEADS = 4


class Cfg:
    def __init__(self, ncores=8, nreal=40000, npc=5120, in_dim=64, hid=256,
                 eb=11, gw=224, ks=64, g=1500):
        self.NCORES = ncores
        self.N = nreal                      # real nodes total
        self.NR = nreal // ncores           # real nodes per core
        self.NP = npc                       # padded node slots per core
        self.NT = npc * ncores              # total table slots
        self.IN = in_dim
        self.HID = hid
        self.C = hid // HEADS
        self.EB = eb                        # edge blocks per dst-tile
        self.ET = eb * P                    # edge slot capacity per dst-tile
        self.NPT = npc // P                 # dst-tiles per core
        self.ECB = self.NPT * eb            # edge array free blocks
        self.GW = gw                        # graph window per core
        self.KS = ks                        # slots per graph
        self.RN = gw * ks                   # readout gather count
        self.G = g
        assert npc % P == 0 and self.RN % 512 == 0
        self.PAD_SLOT = self.NR             # global slot with zeroed table row


FULL = Cfg()


# ---------------------------------------------------------------------------
# device program
# ---------------------------------------------------------------------------

def build_nc(cfg: Cfg, dbg=False):
    c = cfg
    HID = c.HID
    nc = bacc.Bacc("TRN2", target_bir_lowering=False, debug=False,
                   num_devices=c.NCORES, num_swdge_queues=1)

    x_in = nc.dram_tensor("x_in", [c.NP, c.IN], BF16, kind="ExternalInput")
    wpk = nc.dram_tensor("wpk", [P, 160], BF16, kind="ExternalInput")
    pvec = nc.dram_tensor("pvec", [22, P], F32, kind="ExternalInput")
    asd = nc.dram_tensor("asd", [6, HID], F32, kind="ExternalInput")
    esrc = nc.dram_tensor("esrc", [P, c.ECB], mybir.dt.uint16, kind="ExternalInput")
    edst32 = nc.dram_tensor("edst32", [P, c.ECB], I16, kind="ExternalInput")
    dstf = nc.dram_tensor("dstf", [P, c.ECB], mybir.dt.int8, kind="ExternalInput")
    ridxs = nc.dram_tensor("ridxs", [16, c.RN // 16], I16, kind="ExternalInput")
    ridxm = nc.dram_tensor("ridxm", [16, c.RN // 16], I16, kind="ExternalInput")

    rsum_o = nc.dram_tensor("rsum_o", [2, P, c.GW], BF16, kind="ExternalOutput")
    rmax_o = nc.dram_tensor("rmax_o", [2, P, c.GW], BF16, kind="ExternalOutput")
    if dbg:
        d_h = [nc.dram_tensor(f"d_h{l}", [c.NP, c.HID], F32,
                              kind="ExternalOutput") for l in range(3)]
        d_o = nc.dram_tensor("d_o", [c.NP, c.HID], F32, kind="ExternalOutput")
        d_g1 = nc.dram_tensor("d_g1", [P, c.EB, c.HID + 16], F32, kind="ExternalOutput")
        d_g2 = nc.dram_tensor("d_g2", [P, c.EB, 8], F32, kind="ExternalOutput")
        d_ex = nc.dram_tensor("d_ex", [P, c.EB, 4], F32, kind="ExternalOutput")
        d_pay = nc.dram_tensor("d_pay", [P, c.EB, 260], F32, kind="ExternalOutput")
        d_pt = nc.dram_tensor("d_pt", [P, 260], F32, kind="ExternalOutput")

    RG = [list(range(c.NCORES))]

    with tile.TileContext(nc) as tc:
        with (
            tc.tile_pool(name="dram", bufs=1, space="DRAM") as dram,
            tc.tile_pool(name="const", bufs=1) as cpool,
            tc.tile_pool(name="persist", bufs=1) as pers,
        ):
            # ---------------- constants / loads ----------------
            ident = cpool.tile([P, P], BF16)
            make_identity(nc, ident[:])
            ones = cpool.tile([P, 1], F32)
            nc.gpsimd.memset(ones[:], 1.0)
            m39 = cpool.tile([P, 1], F32)
            nc.gpsimd.memset(m39[:], 0.0)
            npad = c.NR - (c.NPT - 1) * P   # real rows in the last tile
            nc.gpsimd.memset(m39[0:npad, :], 1.0)

            wpk_sb = cpool.tile([P, 160], BF16)
            nc.sync.dma_start(wpk_sb[:], wpk[:, :])
            wpk_d = dram.tile([P, 160], BF16)
            nc.gpsimd.dma_start(wpk_d[:, :], wpk_sb[:])
            wfull = dram.tile([8 * P * 160], BF16, addr_space="Shared")
            nc.gpsimd.collective_compute(
                "AllGather", ALU.bypass, replica_groups=RG,
                ins=[wpk_d[:, :].opt()], outs=[wfull[:].opt()])
            # wfull layout: wcat0 (64*512) | w1 (256*256) | w2 (256*256)
            wc0_sb = cpool.tile([c.IN, 2 * HID], BF16)
            nc.sync.dma_start(
                wc0_sb[:],
                wfull[0:c.IN * 2 * HID].rearrange("(a b) -> a b", b=2 * HID))
            w1_sb = cpool.tile([P, 2, HID], BF16)
            w2_sb = cpool.tile([P, 2, HID], BF16)
            OF1 = c.IN * 2 * HID
            OF2 = OF1 + HID * HID
            for hf in range(2):
                nc.sync.dma_start(
                    w1_sb[:, hf, :],
                    wfull[OF1 + hf * P * HID:OF1 + (hf + 1) * P * HID].rearrange(
                        "(a b) -> a b", b=HID))
                nc.sync.dma_start(
                    w2_sb[:, hf, :],
                    wfull[OF2 + hf * P * HID:OF2 + (hf + 1) * P * HID].rearrange(
                        "(a b) -> a b", b=HID))

            def bcast256(pool, dst, src0, src1):
                scr = pool.tile([1, 2 * P], F32, tag="bscr", bufs=2)
                nc.sync.dma_start(scr[0:1, 0:P], src0)
                nc.sync.dma_start(scr[0:1, P:2 * P], src1)
                nc.gpsimd.partition_broadcast(dst[:, :], scr[0:1, :])

            as_bc, ad_bc, gb_bc = [], [], []
            for l in range(3):
                a_s = pers.tile([P, HID], F32, name=f"as_bc{l}")
                a_d = pers.tile([P, HID], F32, name=f"ad_bc{l}")
                bcast256(pers, a_s, asd[2 * l:2 * l + 1, 0:P],
                         asd[2 * l:2 * l + 1, P:2 * P])
                bcast256(pers, a_d, asd[2 * l + 1:2 * l + 2, 0:P],
                         asd[2 * l + 1:2 * l + 2, P:2 * P])
                as_bc.append(a_s)
                ad_bc.append(a_d)
                g_b = pers.tile([P, HID], F32, name=f"gb_bc{l}")
                bcast256(pers, g_b, pvec[2 + 6 * l:3 + 6 * l, :],
                         pvec[3 + 6 * l:4 + 6 * l, :])
                gb_bc.append(g_b)
            bin_bc = pers.tile([P, HID], F32)
            bcast256(pers, bin_bc, pvec[0:1, :], pvec[1:2, :])
            iota_bc = pers.tile([P, P], F32)
            scr0 = pers.tile([1, P], F32, name="scr0")
            nc.sync.dma_start(scr0[0:1, :], pvec[20:21, :])
            nc.gpsimd.partition_broadcast(iota_bc[:, :], scr0[0:1, :])

            esrc_16 = pers.tile([P, c.ECB], mybir.dt.uint16, name="esrc16")
            nc.sync.dma_start(esrc_16[:], esrc[:, :])
            esrc_sb = pers.tile([P, c.ECB], I32)
            nc.vector.tensor_copy(esrc_sb[:], esrc_16[:])
            edst_16 = pers.tile([P, c.ECB], I16, name="edst16t")
            nc.sync.dma_start(edst_16[:], edst32[:, :])
            edst32_sb = pers.tile([P, c.ECB], I32)
            nc.vector.tensor_copy(edst32_sb[:], edst_16[:])
            dstf_8 = pers.tile([P, c.ECB], mybir.dt.int8, name="dstf8")
            nc.sync.dma_start(dstf_8[:], dstf[:, :])
            dstf_sb = pers.tile([P, c.ECB], F32)
            nc.vector.tensor_copy(dstf_sb[:], dstf_8[:])
            ridxs_sb = pers.tile([P, c.RN // 16], I16)
            ridxm_sb = pers.tile([P, c.RN // 16], I16)
            for k in range(8):
                nc.sync.dma_start(ridxs_sb[16 * k:16 * (k + 1), :], ridxs[:, :])
                nc.sync.dma_start(ridxm_sb[16 * k:16 * (k + 1), :], ridxm[:, :])

            x_bf = pers.tile([P, c.NPT, c.IN], BF16)
            for t in range(c.NPT):
                nc.sync.dma_start(x_bf[:, t, :], x_in[t * P:(t + 1) * P, :])

            h_bf = pers.tile([P, c.NPT, HID], BF16)
            o_sb = pers.tile([P, c.NPT, HID], F32)

            hshort = dram.tile([c.NP, HID], F32)
            h_cur = [dram.tile([c.NP, HID], F32, name=f"h{l}") for l in range(3)]

            # ---------------- layers ----------------
            for l in range(3):
                tbl_own = dram.tile([c.NP, HID + 16], BF16, name=f"tblo{l}")
                tblS_own = dram.tile([c.NP, 8], F32, name=f"tblso{l}")
                tbl = dram.tile([c.NT, HID + 16], BF16, name=f"tbl{l}",
                                addr_space="Shared")
                stats_own = dram.tile([4, P], F32, name=f"sto{l}")
                stats_all = dram.tile([4, P], F32, name=f"sta{l}",
                                      addr_space="Shared")

                with (
                    tc.tile_pool(name=f"tl{l}", bufs=2) as tl,
                    tc.tile_pool(name=f"tps{l}", bufs=2, space="PSUM") as tps,
                    tc.tile_pool(name=f"sps{l}", bufs=1, space="PSUM") as sps,
                    tc.tile_pool(name=f"ep{l}", bufs=2) as ep,
                    tc.tile_pool(name=f"sm{l}", bufs=1) as sm,
                ):
                    # ---- table build (own slice) ----
                    for t in range(c.NPT):
                        if l == 0:
                            xt = x_bf[:, t, :]
                            tp = tps.tile([P, P], BF16, tag="tp")
                            hT = tl.tile([P, 2, P], BF16, tag="hT")
                            nc.tensor.transpose(tp[0:c.IN, :], xt, ident[:])
                            nc.vector.tensor_copy(hT[0:c.IN, 0, :], tp[0:c.IN, :])
                            xw_ps = tps.tile([P, 2 * HID], F32, tag="xwps")
                            nc.tensor.matmul(xw_ps[:, :], hT[0:c.IN, 0, :],
                                             wc0_sb[:, :], start=True, stop=True)
                            hs = tl.tile([P, HID], F32, tag="hs")
                            nc.vector.tensor_add(hs[:], xw_ps[:, 0:HID], bin_bc[:])
                            nc.sync.dma_start(hshort[t * P:(t + 1) * P, :], hs[:])
                            xw = xw_ps[:, HID:2 * HID]
                        else:
                            ht = h_bf[:, t, :]
                            wsb = w1_sb if l == 1 else w2_sb
                            xw_ps = tps.tile([P, 2 * HID], F32, tag="xwps")
                            hT = tl.tile([P, 2, P], BF16, tag="hT")
                            for hf in range(2):
                                tp = tps.tile([P, P], BF16, tag="tp")
                                nc.tensor.transpose(
                                    tp[:, :], ht[:, hf * P:(hf + 1) * P], ident[:])
                                nc.vector.tensor_copy(hT[:, hf, :], tp[:, :])
                            for hf in range(2):
                                nc.tensor.matmul(xw_ps[:, 0:HID], hT[:, hf, :],
                                                 wsb[:, hf, :], start=(hf == 0),
                                                 stop=(hf == 1))
                            xw = xw_ps[:, 0:HID]

                        xwb = tl.tile([P, HID], BF16, tag="xwb")
                        if t == c.NPT - 1:
                            nc.vector.tensor_scalar_mul(xwb[:], xw, m39[:, 0:1])
                        else:
                            nc.vector.tensor_copy(xwb[:], xw)
                        nc.sync.dma_start(tbl_own[t * P:(t + 1) * P, 0:HID],
                                          xwb[:])

                        sd = tl.tile([P, 8], F32, tag="sd")
                        sc = tl.tile([P, c.C], F32, tag="sc")
                        for hd in range(4):
                            sl = slice(hd * c.C, (hd + 1) * c.C)
                            nc.vector.tensor_tensor(
                                out=sc[:], in0=xw[:, sl], in1=as_bc[l][:, sl],
                                op=ALU.mult)
                            nc.vector.tensor_reduce(
                                sd[:, hd:hd + 1], sc[:], axis=AX.X, op=ALU.add)
                            nc.vector.tensor_tensor(
                                out=sc[:], in0=xw[:, sl], in1=ad_bc[l][:, sl],
                                op=ALU.mult)
                            nc.vector.tensor_reduce(
                                sd[:, 4 + hd:5 + hd], sc[:], axis=AX.X,
                                op=ALU.add)
                        nc.sync.dma_start(tblS_own[t * P:(t + 1) * P, :], sd[:])
                        nc.sync.dma_start(
                            tbl_own[t * P:(t + 1) * P, HID:HID + 16],
                            sd[:].bitcast(BF16))

                    nc.gpsimd.collective_compute(
                        "AllGather", ALU.bypass, replica_groups=RG,
                        ins=[tbl_own[:, :].opt()], outs=[tbl[:, :].opt()])

                    # ---- edge phase: gather / attention / segment matmul ----
                    s_o = sm.tile([P, HID], F32, name=f"s_o{l}")
                    s_q = sm.tile([P, HID], F32, name=f"s_q{l}")
                    nc.vector.memset(s_o[:], 0.0)
                    nc.vector.memset(s_q[:], 0.0)
                    for t in range(c.NPT):
                        g1 = ep.tile([P, c.EB, HID + 16], BF16, tag="g1")
                        g2 = ep.tile([P, c.EB, 8], F32, tag="g2")
                        for b in range(c.EB):
                            col = t * c.EB + b
                            nc.gpsimd.indirect_dma_start(
                                out=g1[:, b, :], out_offset=None, in_=tbl[:, :],
                                in_offset=bass.IndirectOffsetOnAxis(
                                    ap=esrc_sb[:, col:col + 1], axis=0))
                            nc.gpsimd.indirect_dma_start(
                                out=g2[:, b, :], out_offset=None,
                                in_=tblS_own[:, :],
                                in_offset=bass.IndirectOffsetOnAxis(
                                    ap=edst32_sb[:, col:col + 1], axis=0))

                        g1sd = g1[:, :, HID:HID + 16].bitcast(F32)
                        ee = ep.tile([P, c.EB, 4], F32, tag="ee")
                        nc.vector.tensor_add(ee[:], g1sd[:, :, 0:4],
                                             g2[:, :, 4:8])
                        e2 = ep.tile([P, c.EB, 4], F32, tag="e2")
                        nc.scalar.mul(e2[:], ee[:], SLOPE)
                        nc.vector.tensor_tensor(out=ee[:], in0=ee[:], in1=e2[:],
                                                op=ALU.max)
                        ex = ep.tile([P, c.EB, 4], F32, tag="ex")
                        nc.scalar.activation(ex[:], ee[:], ACT.Exp)

                        pay = ep.tile([P, c.EB, 260], BF16, tag="pay")
                        for hd in range(4):
                            sl = slice(hd * c.C, (hd + 1) * c.C)
                            nc.vector.tensor_tensor(
                                out=pay[:, :, sl], in0=g1[:, :, sl],
                                in1=ex[:, :, hd:hd + 1].to_broadcast(
                                    [P, c.EB, c.C]),
                                op=ALU.mult)
                        nc.vector.tensor_copy(pay[:, :, 256:260], ex[:])

                        pt = tps.tile([P, 260], F32, tag="pt")
                        for b in range(c.EB):
                            S = ep.tile([P, P], BF16, tag="S")
                            nc.vector.tensor_scalar(
                                out=S[:], in0=iota_bc[:],
                                scalar1=dstf_sb[:, t * c.EB + b:
                                                t * c.EB + b + 1],
                                scalar2=None, op0=ALU.is_equal)
                            nc.tensor.matmul(pt[:, :], S[:], pay[:, b, :],
                                             start=(b == 0),
                                             stop=(b == c.EB - 1))

                        if dbg and l == 0 and t == 0:
                            dcp = tl.tile([P, c.EB, c.HID + 16], F32, tag="dcp")
                            nc.vector.tensor_copy(dcp[:], g1[:])
                            nc.sync.dma_start(d_g1[:, :, :], dcp[:])
                            nc.sync.dma_start(d_g2[:, :, :], g2[:])
                            nc.sync.dma_start(d_ex[:, :, :], ex[:])
                            dcq = tl.tile([P, c.EB, 260], F32, tag="dcq")
                            nc.vector.tensor_copy(dcq[:], pay[:])
                            nc.sync.dma_start(d_pay[:, :, :], dcq[:])
                            dcr = tl.tile([P, 260], F32, tag="dcr")
                            nc.vector.tensor_copy(dcr[:], pt[:])
                            nc.sync.dma_start(d_pt[:, :], dcr[:])

                        # ---- finalize tile: alpha-div, bias, stats ----
                        den = tl.tile([P, 4], F32, tag="den")
                        nc.vector.tensor_scalar_max(den[:], pt[:, 256:260],
                                                    1e-20)
                        rec = tl.tile([P, 4], F32, tag="rec")
                        nc.vector.reciprocal(rec[:], den[:])
                        ot = o_sb[:, t, :]
                        for hd in range(4):
                            sl = slice(hd * c.C, (hd + 1) * c.C)
                            nc.vector.tensor_scalar_mul(ot[:, sl], pt[:, sl],
                                                        rec[:, hd:hd + 1])
                        nc.vector.tensor_add(ot, ot, gb_bc[l][:])
                        if t == c.NPT - 1:
                            om = tl.tile([P, HID], F32, tag="om")
                            nc.vector.tensor_scalar_mul(om[:], ot, m39[:, 0:1])
                            stat_in = om[:]
                        else:
                            stat_in = ot
                        sq = tl.tile([P, HID], F32, tag="sq")
                        nc.vector.tensor_tensor(out=sq[:], in0=stat_in,
                                                in1=stat_in, op=ALU.mult)
                        nc.vector.tensor_add(s_o[:], s_o[:], stat_in)
                        nc.vector.tensor_add(s_q[:], s_q[:], sq[:])

                    if dbg and l == 0:
                        for t in range(c.NPT):
                            nc.sync.dma_start(d_o[t * P:(t + 1) * P, :],
                                              o_sb[:, t, :])

                    # ---- BN stats reduce + collective ----
                    st_ps = sps.tile([P, 4], F32, name=f"stp{l}")
                    for hf in range(2):
                        nc.tensor.matmul(
                            st_ps[:, hf:hf + 1], s_o[:, hf * P:(hf + 1) * P],
                            ones[:], start=True, stop=True,
                            skip_group_check=True)
                        nc.tensor.matmul(
                            st_ps[:, 2 + hf:3 + hf], s_q[:, hf * P:(hf + 1) * P],
                            ones[:], start=True, stop=True,
                            skip_group_check=True)
                    stq = sm.tile([P, 4], F32, name=f"stq{l}")
                    nc.vector.tensor_copy(stq[:], st_ps[:])
                    nc.sync.dma_start(stats_own[:, :].transpose([1, 0]), stq[:])
                    nc.gpsimd.collective_compute(
                        "AllReduce", ALU.add, replica_groups=RG,
                        ins=[stats_own[:, :].opt()], outs=[stats_all[:, :].opt()])

                    st_s = sm.tile([2, P], F32, name=f"sts{l}")
                    nc.sync.dma_start(st_s[:], stats_all[0:2, :])
                    st_q = sm.tile([2, P], F32, name=f"stq2{l}")
                    nc.sync.dma_start(st_q[:], stats_all[2:4, :])
                    g2t = sm.tile([2, P], F32, name=f"g2t{l}")
                    nc.sync.dma_start(g2t[:], pvec[4 + 6 * l:6 + 6 * l, :])
                    be2t = sm.tile([2, P], F32, name=f"be2t{l}")
                    nc.sync.dma_start(be2t[:], pvec[6 + 6 * l:8 + 6 * l, :])
                    mu = sm.tile([2, P], F32, name=f"mu{l}")
                    nc.scalar.mul(mu[:], st_s[:], 1.0 / c.N)
                    va = sm.tile([2, P], F32, name=f"va{l}")
                    nc.scalar.mul(va[:], st_q[:], 1.0 / c.N)
                    mu2 = sm.tile([2, P], F32, name=f"mu2{l}")
                    nc.vector.tensor_tensor(out=mu2[:], in0=mu[:], in1=mu[:],
                                            op=ALU.mult)
                    nc.vector.tensor_tensor(out=va[:], in0=va[:], in1=mu2[:],
                                            op=ALU.subtract)
                    nc.vector.tensor_scalar_add(va[:], va[:], EPS)
                    sdv = sm.tile([2, P], F32, name=f"sdv{l}")
                    nc.scalar.sqrt(sdv[:], va[:])
                    rs = sm.tile([2, P], F32, name=f"rs{l}")
                    nc.vector.reciprocal(rs[:], sdv[:])
                    A2 = sm.tile([2, P], F32, name=f"A2{l}")
                    nc.vector.tensor_tensor(out=A2[:], in0=rs[:], in1=g2t[:],
                                            op=ALU.mult)
                    B2 = sm.tile([2, P], F32, name=f"B2{l}")
                    nc.vector.tensor_tensor(out=B2[:], in0=mu[:], in1=A2[:],
                                            op=ALU.mult)
                    nc.vector.tensor_tensor(out=B2[:], in0=be2t[:],
                                            in1=B2[:], op=ALU.subtract)
                    A_bc = sm.tile([P, HID], F32, name=f"Abc{l}")
                    B_bc = sm.tile([P, HID], F32, name=f"Bbc{l}")
                    bcast256(sm, A_bc, A2[0:1, :], A2[1:2, :])
                    bcast256(sm, B_bc, B2[0:1, :], B2[1:2, :])

                    # ---- apply pass ----
                    for t in range(c.NPT):
                        u1 = tl.tile([P, HID], F32, tag="u1")
                        nc.vector.tensor_tensor(out=u1[:], in0=o_sb[:, t, :],
                                                in1=A_bc[:], op=ALU.mult)
                        nc.vector.tensor_add(u1[:], u1[:], B_bc[:])
                        u2 = tl.tile([P, HID], F32, tag="u2")
                        nc.vector.tensor_scalar_min(u2[:], u1[:], 0.0)
                        nc.scalar.activation(u2[:], u2[:], ACT.Exp)
                        rl = tl.tile([P, HID], F32, tag="rl")
                        nc.scalar.activation(rl[:], u1[:], ACT.Relu)
                        nc.vector.tensor_add(u2[:], u2[:], rl[:])
                        nc.vector.tensor_scalar_add(u2[:], u2[:], -1.0)
                        hp = tl.tile([P, HID], F32, tag="hp")
                        src = hshort if l == 0 else h_cur[l - 1]
                        nc.sync.dma_start(hp[:], src[t * P:(t + 1) * P, :])
                        nc.vector.tensor_add(u2[:], u2[:], hp[:])
                        nc.sync.dma_start(h_cur[l][t * P:(t + 1) * P, :], u2[:])
                        if dbg:
                            nc.sync.dma_start(d_h[l][t * P:(t + 1) * P, :],
                                              u2[:])
                        nc.vector.tensor_copy(h_bf[:, t, :], u2[:])

            # ---------------- readout ----------------
            h3tbl = dram.tile([c.NP + P, HID], BF16)
            for t in range(c.NPT):
                nc.sync.dma_start(h3tbl[t * P:(t + 1) * P, :], h_bf[:, t, :])
            with (
                tc.tile_pool(name="rd", bufs=2) as rd,
                tc.tile_pool(name="rs1", bufs=1) as rs1,
            ):
                sent0 = rs1.tile([1, HID], BF16, name="sent0")
                nc.gpsimd.memset(sent0[:], 0.0)
                sent1 = rs1.tile([1, HID], BF16, name="sent1")
                nc.gpsimd.memset(sent1[:], -1e30)
                nc.sync.dma_start(h3tbl[c.NP:c.NP + 1, :], sent0[:])
                nc.sync.dma_start(h3tbl[c.NP + 1:c.NP + 2, :], sent1[:])

                rsum_sb = rs1.tile([P, 2, c.GW], F32, name="rsum_sb")
                rmax_sb = rs1.tile([P, 2, c.GW], F32, name="rmax_sb")
                GSZ = 512
                gpg = GSZ // c.KS
                for j in range(c.RN // GSZ):
                    isl = slice(j * (GSZ // 16), (j + 1) * (GSZ // 16))
                    gsl = slice(j * gpg, (j + 1) * gpg)
                    gr = rd.tile([P, 2, GSZ], BF16, tag="gr")
                    nc.gpsimd.dma_gather(
                        out_ap=gr[:], in_ap=h3tbl[:, :],
                        idxs_ap=ridxs_sb[:, isl],
                        num_idxs=GSZ, num_idxs_reg=GSZ, elem_size=HID,
                        transpose=True)
                    nc.vector.tensor_reduce(
                        rsum_sb[:, :, gsl],
                        gr[:].rearrange("p b (g k) -> p b g k", k=c.KS),
                        axis=AX.X, op=ALU.add)
                    gm = rd.tile([P, 2, GSZ], BF16, tag="gm")
                    nc.gpsimd.dma_gather(
                        out_ap=gm[:], in_ap=h3tbl[:, :],
                        idxs_ap=ridxm_sb[:, isl],
                        num_idxs=GSZ, num_idxs_reg=GSZ, elem_size=HID,
                        transpose=True)
                    nc.vector.tensor_reduce(
                        rmax_sb[:, :, gsl],
                        gm[:].rearrange("p b (g k) -> p b g k", k=c.KS),
                        axis=AX.X, op=ALU.max)

                rsum_bf = rs1.tile([P, 2, c.GW], BF16, name="rsum_bf")
                nc.vector.tensor_copy(rsum_bf[:], rsum_sb[:])
                rmax_bf = rs1.tile([P, 2, c.GW], BF16, name="rmax_bf")
                nc.vector.tensor_copy(rmax_bf[:], rmax_sb[:])
                nc.sync.dma_start(rsum_o[:, :, :].transpose([1, 0, 2]),
                                  rsum_bf[:])
                nc.sync.dma_start(rmax_o[:, :, :].transpose([1, 0, 2]),
                                  rmax_bf[:])

    nc.compile()
    return nc


# ---------------------------------------------------------------------------
# host-side prep
# ---------------------------------------------------------------------------

def host_prep(cfg: Cfg, x, edge_index, batch):
    c = cfg
    bf = ml_dtypes.bfloat16
    n = c.N
    loop = np.arange(n, dtype=np.int64)
    src = np.concatenate([np.asarray(edge_index[0], np.int64), loop])
    dst = np.concatenate([np.asarray(edge_index[1], np.int64), loop])
    src_slot = ((src // c.NR) * c.NP + src % c.NR).astype(np.int64)
    dst_core = dst // c.NR
    dst_local = (dst % c.NR).astype(np.int64)
    batch = np.asarray(batch, np.int64)

    xscale = float(np.abs(x).max()) / 127.0 + 1e-30
    gorder = np.argsort(dst.astype(np.int32), kind="stable")
    src_slot_s = src_slot[gorder]
    dst_s = dst[gorder]
    cuts = np.searchsorted(dst_s, np.arange(0, c.N + c.NR, c.NR))
    per_core = []
    gfirsts = []
    for cc in range(c.NCORES):
        lo, hi = cuts[cc], cuts[cc + 1]
        es = src_slot_s[lo:hi]
        ed = dst_s[lo:hi] - cc * c.NR
        tile_id = ed // P
        counts = np.bincount(tile_id, minlength=c.NPT)
        if counts.max() > c.ET:
            raise OverflowError("edge tile capacity exceeded")
        starts = np.zeros(c.NPT, np.int64)
        np.cumsum(counts[:-1], out=starts[1:])
        pos_in_tile = np.arange(len(ed)) - starts[tile_id]
        es_f = np.full(c.NPT * c.ET, c.PAD_SLOT, np.int64)
        ed_f = np.zeros(c.NPT * c.ET, np.int64)
        df_f = np.full(c.NPT * c.ET, -1, np.int64)
        slot = tile_id * c.ET + pos_in_tile
        es_f[slot] = es
        ed_f[slot] = ed
        df_f[slot] = ed % P
        # slot s of tile t -> (p = s % 128, col = t*EB + s//128)
        es2d = np.ascontiguousarray(
            es_f.reshape(c.NPT, c.EB, P).transpose(2, 0, 1).reshape(
                P, c.ECB)).astype(np.uint16)
        ed2d = np.ascontiguousarray(
            ed_f.reshape(c.NPT, c.EB, P).transpose(2, 0, 1).reshape(
                P, c.ECB)).astype(np.int16)
        df2d = np.ascontiguousarray(
            df_f.reshape(c.NPT, c.EB, P).transpose(2, 0, 1).reshape(
                P, c.ECB)).astype(np.int8)

        bsl = batch[cc * c.NR:(cc + 1) * c.NR]
        gfirst = int(bsl[0])
        gfirsts.append(gfirst)
        w = (bsl - gfirst).astype(np.int64)
        uniq, first_idx = np.unique(w, return_index=True)
        fi = np.zeros(int(w[-1]) + 1, np.int64)
        fi[uniq] = first_idx
        kwi = np.arange(c.NR) - fi[w]
        if int(w[-1]) >= c.GW or int(kwi.max()) >= c.KS:
            raise OverflowError("readout window exceeded")
        sidx_s = np.full(c.RN, c.NP, np.int64)
        sidx_m = np.full(c.RN, c.NP + 1, np.int64)
        pos = w * c.KS + kwi
        sidx_s[pos] = np.arange(c.NR)
        sidx_m[pos] = np.arange(c.NR)
        ridxs_a = sidx_s.reshape(c.RN // 16, 16).T.astype(np.int16)
        ridxm_a = sidx_m.reshape(c.RN // 16, 16).T.astype(np.int16)

        xp = np.zeros((c.NP, c.IN), np.float32)
        xp[:c.NR] = x[cc * c.NR:(cc + 1) * c.NR]
        per_core.append(dict(
            x_in=xp.astype(bf), esrc=es2d, edst32=ed2d, dstf=df2d,
            ridxs=ridxs_a, ridxm=ridxm_a))
    return per_core, gfirsts, xscale


def host_weights(cfg: Cfg, W_in, gW0, gW1, gW2, b_in, gb, bng, bnb, a_s, a_d,
                 xscale=1.0):
    bf = ml_dtypes.bfloat16
    wcat0 = np.concatenate([np.asarray(W_in, np.float32),
                            np.asarray(gW0, np.float32)], axis=1).astype(bf)
    pvec = np.zeros((22, P), np.float32)
    pvec[21, 0] = xscale
    pvec[0:2] = np.asarray(b_in, np.float32).reshape(2, P)
    for l in range(3):
        pvec[2 + 6 * l:4 + 6 * l] = np.asarray(gb[l], np.float32).reshape(2, P)
        pvec[4 + 6 * l:6 + 6 * l] = np.asarray(bng[l], np.float32).reshape(2, P)
        pvec[6 + 6 * l:8 + 6 * l] = np.asarray(bnb[l], np.float32).reshape(2, P)
    pvec[20] = np.arange(P, dtype=np.float32)
    asd = np.zeros((6, cfg.HID), np.float32)
    for l in range(3):
        asd[2 * l] = np.asarray(a_s[l], np.float32).reshape(-1)
        asd[2 * l + 1] = np.asarray(a_d[l], np.float32).reshape(-1)
    flat = np.concatenate([
        np.asarray(wcat0, bf).ravel(),
        np.asarray(gW1, np.float32).astype(bf).ravel(),
        np.asarray(gW2, np.float32).astype(bf).ravel()])
    wpk = flat.reshape(8, P, 160)
    return dict(pvec=pvec, asd=asd), wpk


def host_finish(cfg: Cfg, outs, gfirsts, batch, mW1, mb1, mg1, mbeta1,
                mW2, mb2, mg2, mbeta2, hW, hb):
    c = cfg
    batch = np.asarray(batch, np.int64)
    cnt = np.bincount(batch, minlength=c.G).astype(np.float32)
    hsum = np.zeros((c.G, c.HID), np.float32)
    hmax = np.full((c.G, c.HID), -np.inf, np.float32)
    for cc in range(c.NCORES):
        g0 = gfirsts[cc]
        ng = min(c.GW, c.G - g0)
        rs = np.asarray(outs[cc]["rsum_o"], np.float32).reshape(
            2 * P, c.GW)[:c.HID, :ng].T
        rm = np.asarray(outs[cc]["rmax_o"], np.float32).reshape(
            2 * P, c.GW)[:c.HID, :ng].T
        hsum[g0:g0 + ng] += rs
        hmax[g0:g0 + ng] = np.maximum(hmax[g0:g0 + ng], rm)
    hmean = hsum / np.maximum(cnt, 1.0)[:, None]
    hmax = np.where((cnt[:, None] > 0) & (hmax > -1e29), hmax, 0.0)
    hg = np.concatenate([hmean, hmax], axis=1).astype(np.float32)

    def bn(h, g, b):
        mu = h.mean(0, dtype=np.float32)
        v = ((h - mu) ** 2).mean(0, dtype=np.float32)
        return (h - mu) / np.sqrt(v + EPS) * g + b

    s = np.maximum(bn(hg @ np.asarray(mW1, np.float32) + mb1, mg1, mbeta1), 0.0)
    s = np.maximum(bn(s @ np.asarray(mW2, np.float32) + mb2, mg2, mbeta2), 0.0)
    return (s @ np.asarray(hW, np.float32) + hb).astype(np.float32)


# ---------------------------------------------------------------------------
# persistent PJRT runner (compile once, reuse)
# ---------------------------------------------------------------------------

class Runner:
    def __init__(self, nc, n_cores):
        import jax
        from jax.sharding import Mesh, PartitionSpec
        from jax.experimental.shard_map import shard_map
        from concourse import bass2jax
        try:
            jax.config.update("jax_compilation_cache_dir", "/tmp/jax_pcc")
            jax.config.update("jax_persistent_cache_min_entry_size_bytes", -1)
            jax.config.update("jax_persistent_cache_min_compile_time_secs", 0)
        except Exception:
            pass
        bass2jax.install_neuronx_cc_hook()
        self.nc = nc
        self.n_cores = n_cores
        partition_name = (nc.partition_id_tensor.name
                          if getattr(nc, "partition_id_tensor", None) is not None
                          else None)
        in_names, out_names, out_avals, zero_shapes = [], [], [], []
        self.in_specs = {}
        for alloc in nc.m.functions[0].allocations:
            if not isinstance(alloc, mybir.MemoryLocationSet):
                continue
            name = alloc.memorylocations[0].name
            if alloc.kind == "ExternalInput":
                if name == partition_name:
                    continue
                in_names.append(name)
                self.in_specs[name] = (tuple(alloc.tensor_shape),
                                       mybir.dt.np(alloc.dtype))
            elif alloc.kind == "ExternalOutput":
                shape = tuple(alloc.tensor_shape)
                dtype = mybir.dt.np(alloc.dtype)
                out_names.append(name)
                out_avals.append(jax.core.ShapedArray(shape, dtype))
                zero_shapes.append((shape, dtype))
        self.in_names = in_names
        self.out_names = out_names
        self.out_avals = out_avals
        self.zero_shapes = zero_shapes
        n_params = len(in_names)
        all_names = list(in_names) + list(out_names)
        if partition_name is not None:
            all_names.append(partition_name)
        donate = tuple(range(n_params, n_params + len(out_names)))

        def _body(*args):
            operands = list(args)
            if partition_name is not None:
                operands.append(bass2jax.partition_id_tensor())
            outs = bass2jax._bass_exec_p.bind(
                *operands,
                out_avals=tuple(out_avals),
                in_names=tuple(all_names),
                out_names=tuple(out_names),
                lowering_input_output_aliases=(),
                sim_require_finite=False,
                sim_require_nnan=False,
                nc=nc,
            )
            return tuple(outs)

        devices = jax.devices()[:n_cores]
        mesh = Mesh(np.asarray(devices), ("core",))
        from jax.sharding import NamedSharding
        self._zero_sh = NamedSharding(mesh, PartitionSpec("core"))
        nin = n_params + len(out_names)
        self._fn = jax.jit(
            shard_map(_body, mesh=mesh,
                      in_specs=(PartitionSpec("core"),) * nin,
                      out_specs=(PartitionSpec("core"),) * len(out_names),
                      check_rep=False),
            donate_argnums=donate, keep_unused=True)

    def run(self, in_maps):
        concat = [np.concatenate([np.asarray(m[nm]) for m in in_maps], axis=0)
                  for nm in self.in_names]
        zeros = [np.zeros((self.n_cores * s[0], *s[1:]), d)
                 for s, d in self.zero_shapes]
        out_arrs = self._fn(*concat, *zeros)
        res = []
        for cc in range(self.n_cores):
            res.append({nm: np.asarray(out_arrs[i]).reshape(
                self.n_cores, *self.out_avals[i].shape)[cc]
                for i, nm in enumerate(self.out_names)})
        return res

    def warm(self):
        in_maps = []
        for cc in range(self.n_cores):
            m = {nm: np.zeros(sh, dt)
                 for nm, (sh, dt) in self.in_specs.items()}
            in_maps.append(m)
        self.run(in_maps)


_RUNNER = None


def _ensure_runner():
    global _RUNNER
    if _RUNNER is None:
        nc = build_nc(FULL)
        _RUNNER = Runner(nc, FULL.NCORES)
        _RUNNER.warm()
    return _RUNNER


# ---------------------------------------------------------------------------
# entry point
# ---------------------------------------------------------------------------

def _kernel_numpy(x, edge_index, batch, W_in, b_in, gW, gas, gad, gb, bng,
                  bnb, mW1, mb1, mg1, mbeta1, mW2, mb2, mg2, mbeta2, hW, hb):
    # pure-host fallback (slow) in case device capacity assumptions fail
    n = x.shape[0]
    G = FULL.G
    loop = np.arange(n)
    src = np.concatenate([edge_index[0], loop])
    dst = np.concatenate([edge_index[1], loop])
    order = np.argsort(dst, kind="stable")
    srcs, dsts = src[order], dst[order]
    counts = np.bincount(dsts, minlength=n)
    starts = np.zeros(n, np.int64)
    np.cumsum(counts[:-1], out=starts[1:])

    def bn(h, g, b):
        mu = h.mean(0)
        v = ((h - mu) ** 2).mean(0)
        return (h - mu) / np.sqrt(v + EPS) * g + b

    h_short = x @ W_in + b_in
    h = x
    for i in range(3):
        xw = (h @ gW[i]).reshape(n, HEADS, -1)
        ssum = np.einsum("nhc,hc->nh", xw, gas[i])
        dsum = np.einsum("nhc,hc->nh", xw, gad[i])
        e = ssum[srcs] + dsum[dsts]
        e = np.where(e > 0, e, SLOPE * e)
        m = np.maximum.reduceat(e, starts, axis=0)
        ex = np.exp(e - m[dsts])
        den = np.add.reduceat(ex, starts, axis=0)
        alpha = ex / den[dsts]
        out = np.add.reduceat(xw[srcs] * alpha[:, :, None], starts,
                              axis=0).reshape(n, -1) + gb[i]
        hn = bn(out, bng[i], bnb[i])
        hn = np.where(hn > 0, hn, np.expm1(np.minimum(hn, 0)))
        h = hn + (h_short if i == 0 else h)
    cnt = np.bincount(batch, minlength=G).astype(np.float32)
    hsum = np.zeros((G, h.shape[1]), np.float32)
    np.add.at(hsum, batch, h)
    hmax = np.full((G, h.shape[1]), -np.inf, np.float32)
    np.maximum.at(hmax, batch, h)
    hmax = np.where(cnt[:, None] > 0, hmax, 0.0)
    hg = np.concatenate([hsum / np.maximum(cnt, 1.0)[:, None], hmax], axis=1)
    s = np.maximum(bn(hg @ mW1 + mb1, mg1, mbeta1), 0.0)
    s = np.maximum(bn(s @ mW2 + mb2, mg2, mbeta2), 0.0)
    return (s @ hW + hb).astype(np.float32)


def kernel(x, edge_index, batch, W_in, b_in, gW0, gas0, gad0, gb0, bng0, bnb0,
           gW1, gas1, gad1, gb1, bng1, bnb1, gW2, gas2, gad2, gb2, bng2, bnb2,
           mW1, mb1, mg1, mbeta1, mW2, mb2, mg2, mbeta2, hW, hb):
    c = FULL
    x = np.asarray(x, np.float32)
    edge_index = np.asarray(edge_index)
    batch = np.asarray(batch)
    try:
        runner = _ensure_runner()
        per_core, gfirsts, xscale = host_prep(c, x, edge_index, batch)
        wmap, wpk = host_weights(c, W_in, gW0, gW1, gW2, b_in,
                                 [gb0, gb1, gb2], [bng0, bng1, bng2],
                                 [bnb0, bnb1, bnb2],
                                 [gas0, gas1, gas2], [gad0, gad1, gad2],
                                 xscale=xscale)
        in_maps = [dict(pc, wpk=wpk[i], **wmap)
                   for i, pc in enumerate(per_core)]
        outs = runner.run(in_maps)
        return host_finish(c, outs, gfirsts, batch, mW1, mb1, mg1, mbeta1,
                           mW2, mb2, mg2, mbeta2, hW, hb)
    except OverflowError:
        return _kernel_numpy(
            x, edge_index, batch,
            np.asarray(W_in, np.float32), np.asarray(b_in, np.float32),
            [np.asarray(w, np.float32) for w in (gW0, gW1, gW2)],
            [np.asarray(w, np.float32) for w in (gas0, gas1, gas2)],
            [np.asarray(w, np.float32) for w in (gad0, gad1, gad2)],
            [np.asarray(w, np.float32) for w in (gb0, gb1, gb2)],
            [np.asarray(w, np.float32) for w in (bng0, bng1, bng2)],
            [np.asarray(w, np.float32) for w in (bnb0, bnb1, bnb2)],
            np.asarray(mW1, np.float32), mb1, mg1, mbeta1,
            np.asarray(mW2, np.float32), mb2, mg2, mbeta2,
            np.asarray(hW, np.float32), hb)


if os.environ.get("BASS_GNN_LAZY", "") != "1":
    _ensure_runner()
